# revision 14
# baseline (speedup 1.0000x reference)
"""Trainium2 Bass kernel for nn_Block_89361089561275 (dense transformer block).

Sharding: data-parallel over batch B=8 -> one batch element per NeuronCore.
No collectives. Feature-transposed layout (features on SBUF partitions,
tokens on the free dim) throughout.

Key speed tricks over the bf16 baseline:
  * All big matmuls (qkv, v, proj, fc1, fc2, adapters) run as fp8e4m3
    DoubleRow matmuls: lhsT [128,2,M] + rhs [128,2,N] contract 256 K per
    instruction (~2x tensor-engine throughput). Weights are host-scaled
    (x32-ish) into fp8's sweet spot; inverse scales fold into downstream
    per-feature vectors / activation `scale=` operands.
  * Softmax exp is a Schraudolph bit-trick fused into the mandatory
    PSUM-evacuation op: the q/k weight scales carry alpha*beta =
    128*log2(e)*D^-0.5 so the score PSUM is already K2*s; one DVE
    scalar_tensor_tensor computes int16(round(st + 8192 + rpb_i8)) whose
    bf16 bit-pattern IS exp(s+r) up to a global power-of-2 scale that
    cancels in softmax. rpb ships as int8 = round(K2*rpb) (+8192 via the
    freed scalar slot). ScalarE does no exp at all.
  * The softmax denominator falls out of the AV matmul via a ones-column
    appended to v (M=65 accumulators, row 64 = colsum).
  * LN mean/var still come from ones-matmuls, but the finish chain runs on
    [1,N] rows and broadcasts rstd/-mean*rstd back across partitions with a
    K=1 ones matmul (no DRAM bounce round-trips).
  * fc1/fc2 weights are SBUF-resident (loaded once, streamed during
    attention), gelu/relu descale by 1/32 via the activation scale operand.
"""

import sys

for _p in ("/opt/trn_rl_repo",):
    if _p not in sys.path:
        sys.path.insert(0, _p)

import numpy as np
import ml_dtypes

BF16 = ml_dtypes.bfloat16
FP8 = ml_dtypes.float8_e4m3

B, N, C, H = 8, 1024, 768, 12
D = C // H            # 64
MLP = 4 * C           # 3072
RED = C // 3          # 256
EPS = 1e-5
P = 128
KC = C // P           # 6   c-chunks
KM = MLP // P         # 24  mlp-chunks
KR = RED // P         # 2   adapter chunks
NT = N // P           # 8   token tiles
HALF = 512
NSL = (slice(0, HALF), slice(HALF, N))

K2 = 128.0 / float(np.log(2.0))   # exponent-steps-per-unit-score for bf16
IBIAS = 8192.0                    # int16 exponent offset (2^-63 global scale)
SCALE = D ** -0.5
AB = K2 * SCALE                   # alpha*beta for the q/k weight pair
ALPHA = float(np.sqrt(AB))        # q-side host scale
BETA = float(np.sqrt(AB))         # k-side host scale
WS = 32.0                         # generic fp8 weight scale
OSC = 256.0                       # oT = OSC * o_true (8.0 folded in evac)

_PROG_CACHE: dict = {}

# indices into the packed [n, 128, KC] per-feature vector table
V_G1, V_G2, V_QB, V_PB, V_FB, V_L1G, V_L1B, V_L2G, V_L2B = range(9)
NVEC = 9


def _build(flags):
    """Build the single-core Bass program. flags is a tuple of bools:
    (has_mask, qb_nz, vb_nz, pb_nz, f1b_nz, fb_nz, adb_nz,
     ln1_triv, ln2_triv)
    """
    (has_mask, qb_nz, vb_nz, pb_nz, f1b_nz, fb_nz, adb_nz,
     ln1_triv, ln2_triv) = flags

    import concourse.tile as tile
    from concourse import bacc, mybir
    from contextlib import ExitStack

    f32 = mybir.dt.float32
    bf16 = mybir.dt.bfloat16
    fp8 = mybir.dt.float8e4
    i16 = mybir.dt.int16
    i8 = mybir.dt.int8
    AF = mybir.ActivationFunctionType
    OP = mybir.AluOpType
    PM = mybir.MatmulPerfMode

    nc = bacc.Bacc("TRN2")

    # ---- external I/O ----
    x_d = nc.declare_dram_parameter("xT", [P, KC, N], f32, isOutput=False)
    rpb_d = nc.declare_dram_parameter("rpbT", [H, N, N], i8, isOutput=False)
    wqk_d = nc.declare_dram_parameter("wqk", [12, P, KC, P], fp8, isOutput=False)
    wv_d = nc.declare_dram_parameter("wv", [P, KC, C], fp8, isOutput=False)
    pw_d = nc.declare_dram_parameter("projw", [KC, P, KC, P], fp8, isOutput=False)
    f1_d = nc.declare_dram_parameter("fc1w", [P, KM, KC, P], fp8, isOutput=False)
    f2_d = nc.declare_dram_parameter("fc2w", [P, KC, KM, P], fp8, isOutput=False)
    ad_d = nc.declare_dram_parameter("adw", [KR, P, KC, P], fp8, isOutput=False)
    au_d = nc.declare_dram_parameter("auw", [P, KC, KR, P], fp8, isOutput=False)
    vec_d = nc.declare_dram_parameter("vecs", [NVEC, P, KC], f32, isOutput=False)
    f1b_d = nc.declare_dram_parameter("fc1b", [P, KM], f32, isOutput=False)
    adb_d = nc.declare_dram_parameter("adb", [P, KR], f32, isOutput=False)
    vb_d = nc.declare_dram_parameter("vbias", [1, C], f32, isOutput=False)
    mb_d = nc.declare_dram_parameter("maskb", [P, NT], f32, isOutput=False)
    out_d = nc.declare_dram_parameter("outT", [P, KC, N], f32, isOutput=True)

    with tile.TileContext(nc) as tc, ExitStack() as ctx:
        sb = ctx.enter_context(tc.tile_pool(name="sb", bufs=1))
        pp = ctx.enter_context(tc.tile_pool(name="pp", bufs=1, space="PSUM"))
        dram = ctx.enter_context(tc.tile_pool(name="dram", bufs=2, space="DRAM"))

        # ---- persistent tiles ----
        xres = sb.tile([P, KC, N], f32, tag="xres", bufs=1)
        qkT = sb.tile([P, 12, N], bf16, tag="qkT", bufs=1)
        vaug = sb.tile([P, NT, H, D + 1], bf16, tag="vaug", bufs=1)
        oT = sb.tile([P, KC, N], fp8, tag="oT", bufs=1)
        xnT = sb.tile([P, KC, N], fp8, tag="xnT", bufs=1)
        xn2T = sb.tile([P, KC, N], fp8, tag="xn2T", bufs=1)
        a1T = sb.tile([P, KR, N], fp8, tag="a1", bufs=1)

        for ch in range(KC):  # per-chunk loads so LN1 stats start early
            nc.sync.dma_start(out=xres[:, ch], in_=x_d[:, ch])

        ones_bf = sb.tile([P, 1], bf16, tag="ones", bufs=1)
        nc.vector.memset(ones_bf, 1.0)
        ones_row = sb.tile([1, P], bf16, tag="onesr", bufs=1)
        nc.vector.memset(ones_row, 1.0)
        # evac broadcast weights live at base partition 64 (where the AV
        # colsum row lands) so lhsT/rhs partition bases match
        c8_64 = sb.tile([D + 1, P], bf16, tag="c8r", bufs=1)
        nc.vector.memset(c8_64[D:D + 1, :], float(OSC / WS))
        nc.vector.memset(vaug[:, :, :, D:D + 1], 1.0)

        zero_col = sb.tile([P, 1], f32, tag="zcol", bufs=1)
        nc.vector.memset(zero_col, 0.0)
        eps_col = sb.tile([P, 1], f32, tag="ecol", bufs=1)
        nc.vector.memset(eps_col, float(EPS))

        vecs = sb.tile([P, NVEC, KC], f32, tag="vecs", bufs=1)
        nc.sync.dma_start(out=vecs, in_=vec_d[:].rearrange("v p k -> p v k"))

        def vec(i):
            return vecs[:, i]  # [128, KC]

        if f1b_nz:
            f1b = sb.tile([P, KM], f32, tag="f1b", bufs=1)
            nc.sync.dma_start(out=f1b, in_=f1b_d[:])
        if adb_nz:
            adb = sb.tile([P, KR], f32, tag="adb", bufs=1)
            nc.sync.dma_start(out=adb, in_=adb_d[:])
        if vb_nz:
            vb1 = sb.tile([1, C], f32, tag="vb1", bufs=1)
            nc.sync.dma_start(out=vb1, in_=vb_d[:])
            vb_b = sb.tile([P, C], f32, tag="vb_b", bufs=1)
            scratch = dram.tile([1, C], f32, tag="bscratch", bufs=2)
            nc.sync.dma_start(out=scratch, in_=vb1)
            nc.sync.dma_start(out=vb_b, in_=scratch.to_broadcast(vb_b.shape))
        if has_mask:
            maskb = sb.tile([P, NT], f32, tag="maskb", bufs=1)
            nc.sync.dma_start(out=maskb, in_=mb_d[:])

        # ---------------- layernorm (feature-transposed) ----------------
        # stats via ones-matmuls into two base-0 psum rows (sum, sum-sq) --
        # engine ops need all operands on the same partitions.
        def ln_stats_ch(stt_s, stt_q, ch, xbs, src):
            xb = sb.tile([P, N], bf16, tag="xb", bufs=KC + 1)
            xbs.append(xb)
            nc.vector.tensor_copy(out=xb, in_=src[:, ch])
            x2 = sb.tile([P, N], bf16, tag="x2", bufs=2)
            nc.vector.tensor_mul(x2, xb, xb)
            for nk in range(2):
                nc.tensor.matmul(stt_s[0:1, NSL[nk]], lhsT=ones_bf,
                                 rhs=xb[:, NSL[nk]],
                                 start=(ch == 0), stop=(ch == KC - 1))
                nc.tensor.matmul(stt_q[0:1, NSL[nk]], lhsT=ones_bf,
                                 rhs=x2[:, NSL[nk]],
                                 start=(ch == 0), stop=(ch == KC - 1))

        def ln_finish(stt_s, stt_q, xbs, dst, g_i, b_i, triv):
            # [1,N] chain: mean, var, rstd, -mean*rstd; then K=1 PE broadcast
            mean = sb.tile([1, N], f32, tag="lnrow", bufs=4)
            nc.vector.tensor_scalar_mul(mean, stt_s[0:1, :], 1.0 / C)
            mm2 = sb.tile([1, N], f32, tag="lnrow", bufs=4)
            nc.vector.tensor_mul(mm2, mean, mean)
            var = sb.tile([1, N], f32, tag="lnrow", bufs=4)
            nc.vector.scalar_tensor_tensor(out=var, in0=stt_q[0:1, :],
                                           scalar=1.0 / C, in1=mm2,
                                           op0=OP.mult, op1=OP.subtract)
            sd = sb.tile([1, N], f32, tag="lnrow", bufs=4)
            nc.scalar.activation(out=sd, in_=var, func=AF.Sqrt,
                                 bias=eps_col[0:1])
            rn = sb.tile([1, 2, N], bf16, tag="lnrn", bufs=2)
            with nc.allow_low_precision(reason="bf16 rstd broadcast"):
                nc.vector.reciprocal(rn[0:1, 0], sd)           # rstd
            nc.vector.scalar_tensor_tensor(out=rn[0:1, 1], in0=mean,
                                           scalar=-1.0, in1=rn[0:1, 0],
                                           op0=OP.mult, op1=OP.mult)
            # broadcast both rows across partitions via K=1 matmuls
            ab = sb.tile([P, 2, N], bf16, tag="lnab", bufs=1)
            for i in range(2):
                rb = pp.tile([P, N], f32, tag="st", bufs=2, name="lnrb")
                for nk in range(2):
                    nc.tensor.matmul(rb[:, NSL[nk]], lhsT=ones_row,
                                     rhs=rn[0:1, i, NSL[nk]],
                                     start=True, stop=True)
                nc.vector.tensor_copy(out=ab[:, i], in_=rb)
            for ch in range(KC):
                t1 = sb.tile([P, N], bf16, tag="x2", bufs=2)
                nc.vector.tensor_mul(t1, xbs[ch], ab[:, 0])
                if triv:
                    nc.vector.tensor_add(dst[:, ch], t1, ab[:, 1])
                else:
                    nc.vector.tensor_add(t1, t1, ab[:, 1])
                    nc.vector.tensor_scalar(
                        out=dst[:, ch], in0=t1,
                        scalar1=vec(g_i)[:, ch:ch + 1],
                        scalar2=vec(b_i)[:, ch:ch + 1],
                        op0=OP.mult, op1=OP.add)

        # ---------------- LN1 -> xnT (fp8) ----------------
        stt1s = pp.tile([1, N], f32, tag="st", bufs=2, name="ln_s")
        stt1q = pp.tile([1, N], f32, tag="st", bufs=2, name="ln_q")
        xbs1 = []
        for ch in range(KC):
            ln_stats_ch(stt1s, stt1q, ch, xbs1, xres)
        ln_finish(stt1s, stt1q, xbs1, xnT, V_L1G, V_L1B, ln1_triv)

        # ---------------- QKV (fp8 DoubleRow) ----------------
        def dr_mms(acc, wt, rhs_t, nsl, kd=KC):
            for j in range(kd // 2):
                nc.tensor.matmul(acc, lhsT=wt[:, 2 * j:2 * j + 2, :],
                                 rhs=rhs_t[:, 2 * j:2 * j + 2, nsl],
                                 start=(j == 0), stop=(j == kd // 2 - 1),
                                 perf_mode=PM.DoubleRow)

        for blk in (0, 6, 1, 7, 2, 8, 3, 9, 4, 10, 5, 11):
            wt = sb.tile([P, KC, P], fp8, tag="w6", bufs=3)
            nc.sync.dma_start(out=wt, in_=wqk_d[blk])
            for nk in range(2):
                mm = pp.tile([P, HALF], f32, tag="acc", bufs=4)
                dr_mms(mm, wt, xnT, NSL[nk])
                dst = qkT[:, blk, NSL[nk]]
                if blk < 6 and qb_nz:
                    nc.vector.tensor_scalar_add(dst, mm, vec(V_QB)[:, blk:blk + 1])
                else:
                    nc.scalar.copy(out=dst, in_=mm)

        wv_sb = sb.tile([P, KC, C], fp8, tag="wv", bufs=1)
        nc.sync.dma_start(out=wv_sb, in_=wv_d[:])
        for t in range(NT):
            for off, cw in ((0, HALF), (HALF, C - HALF)):
                mm = pp.tile([P, HALF], f32, tag="acc", bufs=4)
                for j in range(KC // 2):
                    nc.tensor.matmul(
                        mm[:, :cw], lhsT=xnT[:, 2 * j:2 * j + 2, t * P:(t + 1) * P],
                        rhs=wv_sb[:, 2 * j:2 * j + 2, off:off + cw],
                        start=(j == 0), stop=(j == KC // 2 - 1),
                        perf_mode=PM.DoubleRow)
                dst = vaug[:, t, off // D:(off + cw) // D, :D]
                src = mm[:, :cw].rearrange("p (h d) -> p h d", d=D)
                if vb_nz:
                    nc.vector.tensor_add(
                        dst, src,
                        vb_b[:, off:off + cw].rearrange("p (h d) -> p h d", d=D))
                else:
                    nc.vector.tensor_copy(out=dst, in_=src)

        # resident MLP weights: queue the DMAs here so they land during
        # attention (after the first head pairs' rpb tiles)
        f1sb = sb.tile([P, KM, KC, P], fp8, tag="f1sb", bufs=1)
        f2sb = sb.tile([P, KC, KM, P], fp8, tag="f2sb", bufs=1)
        ausb = sb.tile([P, KC, KR, P], fp8, tag="ausb", bufs=1)

        # ---------------- attention ----------------
        def evac_head(o_ps, hp, hh):
            # 1/colsum at base partition 64 (aligned with the colsum row)
            rc = sb.tile([D + 1, N], bf16, tag="rc", bufs=2)
            with nc.allow_low_precision(reason="bf16 softmax denom"):
                for nk in range(2):
                    nc.vector.reciprocal(rc[D:D + 1, NSL[nk]],
                                         o_ps[nk][D:D + 1, :])
            rb = pp.tile([P, N], f32, tag="st", bufs=2, name="evrb")
            for nk in range(2):
                nc.tensor.matmul(rb[:, NSL[nk]], lhsT=c8_64[D:D + 1, :],
                                 rhs=rc[D:D + 1, NSL[nk]],
                                 start=True, stop=True)
            rbs = sb.tile([P, N], bf16, tag="rbs", bufs=2)
            nc.vector.tensor_copy(out=rbs, in_=rb)
            if hh == 0:
                for nk in range(2):
                    nc.vector.tensor_mul(oT[0:D, hp, NSL[nk]],
                                         o_ps[nk][0:D, :], rbs[0:D, NSL[nk]])
            else:
                ot_tmp = sb.tile([D, N], fp8, tag="ott", bufs=2)
                for nk in range(2):
                    nc.vector.tensor_mul(ot_tmp[:, NSL[nk]],
                                         o_ps[nk][0:D, :], rbs[0:D, NSL[nk]])
                nc.sync.dma_start(out=oT[D:P, hp, :], in_=ot_tmp)

        for hp in range(H // 2):
            qh = [qkT[hh * D:(hh + 1) * D, hp, :] for hh in range(2)]
            kh = [qkT[hh * D:(hh + 1) * D, 6 + hp, :] for hh in range(2)]
            o_ps = [[pp.tile([D + 1, HALF], f32, tag="acc", bufs=4,
                             name=f"o_ps{hh}{nk}") for nk in range(2)]
                    for hh in range(2)]
            if hp == 2:  # let early rpb tiles through the queue first
                nc.sync.dma_start(out=f1sb, in_=f1_d[:])
            if hp == 3:
                nc.sync.dma_start(out=f2sb, in_=f2_d[:])
                nc.sync.dma_start(out=ausb, in_=au_d[:])
            for mt in range(NT):
                sts, rps = [], []
                for hh in range(2):
                    rp = sb.tile([P, N], i8, tag="rpb", bufs=6)
                    nc.sync.dma_start(
                        out=rp, in_=rpb_d[2 * hp + hh, mt * P:(mt + 1) * P, :])
                    rps.append(rp)
                    sts.append(pp.tile([P, N], f32, tag="st", bufs=2, name="st"))
                for nk in range(2):  # hh-pairs issue adjacently -> row-group
                    for hh in range(2):  # concurrency on the PE
                        nc.tensor.matmul(sts[hh][:, NSL[nk]],
                                         lhsT=kh[hh][:, mt * P:(mt + 1) * P],
                                         rhs=qh[hh][:, NSL[nk]],
                                         start=True, stop=True)
                pts = []
                for hh in range(2):
                    pt = sb.tile([P, N], i16, tag="pt", bufs=4)
                    pts.append(pt)
                    # int16(st + 8192 + rpb) -- bit-pattern is bf16 exp(s+r)
                    nc.vector.scalar_tensor_tensor(
                        out=pt, in0=sts[hh], scalar=IBIAS, in1=rps[hh],
                        op0=OP.add, op1=OP.add)
                    if has_mask:
                        nc.vector.tensor_scalar_mul(pt, pt, maskb[:, mt:mt + 1])
                for hh in range(2):
                    for nk in range(2):
                        nc.tensor.matmul(
                            o_ps[hh][nk][:, :],
                            lhsT=vaug[:, mt, 2 * hp + hh, :],
                            rhs=pts[hh][:, NSL[nk]].bitcast(bf16),
                            start=(mt == 0), stop=(mt == NT - 1))
            evac_head(o_ps[0], hp, 0)
            evac_head(o_ps[1], hp, 1)

        # ---------------- proj + residual 1 + LN2 stats ----------------
        stt2s = pp.tile([1, N], f32, tag="st", bufs=2, name="ln2_s")
        stt2q = pp.tile([1, N], f32, tag="st", bufs=2, name="ln2_q")
        xbs2 = []
        for mt in range(KC):
            wt = sb.tile([P, KC, P], fp8, tag="w6", bufs=3)
            nc.sync.dma_start(out=wt, in_=pw_d[mt])
            for nk in range(2):
                mm = pp.tile([P, HALF], f32, tag="acc", bufs=4)
                dr_mms(mm, wt, oT, NSL[nk])
                if pb_nz:
                    nc.vector.tensor_scalar_add(mm, mm, vec(V_PB)[:, mt:mt + 1])
                nc.vector.scalar_tensor_tensor(
                    out=xres[:, mt, NSL[nk]], in0=mm,
                    scalar=vec(V_G1)[:, mt:mt + 1],
                    in1=xres[:, mt, NSL[nk]], op0=OP.mult, op1=OP.add)
            ln_stats_ch(stt2s, stt2q, mt, xbs2, xres)
        ln_finish(stt2s, stt2q, xbs2, xn2T, V_L2G, V_L2B, ln2_triv)

        # ---------------- adapter down (relu via ACT, 1/32 descale) -----
        for mt in range(KR):
            wt = sb.tile([P, KC, P], fp8, tag="w6", bufs=3)
            nc.sync.dma_start(out=wt, in_=ad_d[mt])
            for nk in range(2):
                mm = pp.tile([P, HALF], f32, tag="acc", bufs=4)
                dr_mms(mm, wt, xn2T, NSL[nk])
                nc.scalar.activation(
                    out=a1T[:, mt, NSL[nk]], in_=mm, func=AF.Relu,
                    scale=1.0 / WS,
                    bias=(adb[:, mt:mt + 1] if adb_nz else zero_col))

        # ---------------- MLP (fp8 DR, resident weights) ----------------
        for nk in range(2):
            h1 = sb.tile([P, KM, HALF], fp8, tag="h1", bufs=1)
            for mt in range(KM):
                mm = pp.tile([P, HALF], f32, tag="acc", bufs=4)
                dr_mms(mm, f1sb[:, mt], xn2T, NSL[nk])
                nc.scalar.activation(
                    out=h1[:, mt], in_=mm, func=AF.Gelu, scale=1.0 / WS,
                    bias=(f1b[:, mt:mt + 1] if f1b_nz else zero_col))
            for mt in range(KC):
                mm = pp.tile([P, HALF], f32, tag="acc", bufs=4)
                for j in range(KM // 2):
                    nc.tensor.matmul(mm, lhsT=f2sb[:, mt, 2 * j:2 * j + 2, :],
                                     rhs=h1[:, 2 * j:2 * j + 2, :],
                                     start=(j == 0), stop=False,
                                     perf_mode=PM.DoubleRow)
                nc.tensor.matmul(mm, lhsT=ausb[:, mt],
                                 rhs=a1T[:, 0:KR, NSL[nk]],
                                 start=False, stop=True,
                                 perf_mode=PM.DoubleRow)
                if fb_nz:
                    nc.vector.tensor_scalar_add(mm, mm, vec(V_FB)[:, mt:mt + 1])
                nc.vector.scalar_tensor_tensor(
                    out=xres[:, mt, NSL[nk]], in0=mm,
                    scalar=vec(V_G2)[:, mt:mt + 1],
                    in1=xres[:, mt, NSL[nk]], op0=OP.mult, op1=OP.add)
                nc.sync.dma_start(out=out_d[:, mt, NSL[nk]],
                                  in_=xres[:, mt, NSL[nk]])

    if not nc.is_finalized():
        nc.finalize()
    return nc


def _pack_w6(wT, km, kk):
    """[K, M] (K=contraction, M=out) -> [M//128, 128, K//128, 128] fp8 tiles
    laid out so each DMA partition read is contiguous."""
    K, M = wT.shape
    assert K == kk * P and M == km * P
    a = wT.reshape(kk, P, km, P)          # [ks, p, mt, col]
    return np.ascontiguousarray(a.transpose(2, 1, 0, 3)).astype(FP8)


def _pack_res(wT, km, kk):
    """[K, M] -> [128, M//128, K//128, 128] partition-major for one-shot
    SBUF-resident DMA (contiguous per-partition reads)."""
    K, M = wT.shape
    assert K == kk * P and M == km * P
    a = wT.reshape(kk, P, km, P)          # [ks, p, mt, col]
    return np.ascontiguousarray(a.transpose(1, 2, 0, 3)).astype(FP8)


def _stripe(v, k):
    """[k*128] -> [128, k] with v[ks*128+p] at [p, ks]."""
    return np.ascontiguousarray(v.reshape(k, P).T).astype(np.float32)


def prepare_core_inputs(x, mask, rpb, ln1_g, ln1_b, qkv_w, q_bias, v_bias,
                        proj_w, proj_b, gamma1, ln2_g, ln2_b, fc1_w, fc1_b,
                        fc2_w, fc2_b, ad_dw, ad_db, ad_uw, ad_ub, gamma2):
    """Host-side layout prep. Returns (per_core_maps, flags)."""
    f32 = np.float32

    qkv_w = np.asarray(qkv_w, f32)
    wq = qkv_w[:C] * ALPHA            # q-side carries alpha (incl. D^-.5*K2)
    wk = qkv_w[C:2 * C] * BETA
    wv = qkv_w[2 * C:] * WS
    wqkT = np.concatenate([wq, wk], 0).T          # [C, 1536]
    wqk = _pack_w6(wqkT, 12, KC)
    wv_packed = np.ascontiguousarray(
        wv.T.reshape(KC, P, C).transpose(1, 0, 2)).astype(FP8)

    projw = _pack_w6(np.asarray(proj_w, f32).T * WS, KC, KC)
    fc1w = _pack_res(np.asarray(fc1_w, f32).T * WS, KM, KC)
    fc2w = _pack_res(np.asarray(fc2_w, f32).T * WS, KC, KM)
    adw = _pack_w6(np.asarray(ad_dw, f32).T * WS, KR, KC)
    auw = _pack_res(np.asarray(ad_uw, f32).T * WS, KC, KR)

    # rpb as int8 exponent steps: round(K2 * rpb); +8192 rides the stt scalar
    rpbT = np.clip(np.round(K2 * np.asarray(rpb, f32).transpose(0, 2, 1)),
                   -127, 127).astype(np.int8)
    rpbT = np.ascontiguousarray(rpbT)

    q_bias_s = np.asarray(q_bias, f32) * ALPHA
    fb = (np.asarray(fc2_b, f32) + np.asarray(ad_ub, f32)) * WS

    vecs = np.stack([
        _stripe(np.asarray(gamma1, f32) / (OSC * WS), KC),
        _stripe(np.asarray(gamma2, f32) / WS, KC),
        _stripe(q_bias_s, KC),
        _stripe(np.asarray(proj_b, f32) * OSC * WS, KC),
        _stripe(fb, KC),
        _stripe(np.asarray(ln1_g, f32), KC),
        _stripe(np.asarray(ln1_b, f32), KC),
        _stripe(np.asarray(ln2_g, f32), KC),
        _stripe(np.asarray(ln2_b, f32), KC),
    ], 0)  # [NVEC, 128, KC]

    f1b = _stripe(np.asarray(fc1_b, f32), KM)
    adb = _stripe(np.asarray(ad_db, f32), KR)
    vb = (np.asarray(v_bias, f32) * WS).reshape(1, C).astype(f32)

    mask = np.asarray(mask)
    has_mask = not bool(mask.all())

    flags = (
        has_mask,
        bool(np.any(q_bias_s)),
        bool(np.any(v_bias)),
        bool(np.any(proj_b)),
        bool(np.any(fc1_b)),
        bool(np.any(fb)),
        bool(np.any(ad_db)),
        bool(np.all(ln1_g == 1.0) and not np.any(ln1_b)),
        bool(np.all(ln2_g == 1.0) and not np.any(ln2_b)),
    )

    shared = {
        "rpbT": rpbT, "wqk": wqk, "wv": wv_packed, "projw": projw,
        "fc1w": fc1w, "fc2w": fc2w, "adw": adw, "auw": auw,
        "vecs": vecs, "fc1b": f1b, "adb": adb, "vbias": vb,
    }

    x = np.asarray(x, f32)
    per_core = []
    for b in range(B):
        xT = np.ascontiguousarray(
            x[b].T.reshape(KC, P, N).transpose(1, 0, 2)).astype(f32)
        if has_mask:
            mb = np.where(mask[b], 1.0, 0.0).astype(f32)    # [N] over keys m
            mb = np.ascontiguousarray(mb.reshape(NT, P).T)  # [128, NT]
        else:
            mb = np.zeros((P, NT), f32)
        m = dict(shared)
        m["xT"] = xT
        m["maskb"] = mb
        per_core.append(m)
    return per_core, flags


def _ensure_ntff_hook():
    """The agent image lacks ``antenv.axon_hooks``; provide it and register
    the ctypes NTFF profile hook so trace=True works under axon."""
    import types
    try:
        from antenv.axon_hooks import get_axon_ntff_profile_hook  # noqa: F401
        return
    except ImportError:
        pass
    import antenv
    mod = types.ModuleType("antenv.axon_hooks")
    _h = {"hook": None}
    mod.set_axon_ntff_profile_hook = lambda h: _h.__setitem__("hook", h)
    mod.get_axon_ntff_profile_hook = lambda: _h["hook"]
    sys.modules["antenv.axon_hooks"] = mod
    antenv.axon_hooks = mod
    try:
        from trn_agent_boot.trn_boot import _ntff_profile_via_ctypes
        hook = _ntff_profile_via_ctypes("/opt/axon/libaxon_pjrt.so")
        if hook is not None:
            mod.set_axon_ntff_profile_hook(hook)
    except Exception as e:  # profiling degrades, run still works
        print("ntff hook setup failed:", e)


def run_sharded(inputs, trace=False, trace_kwargs=None):
    """Compile (cached) + run on 8 cores. Returns (out [B,N,C] f32, results)."""
    from concourse.bass_utils import run_bass_kernel_spmd
    if trace:
        _ensure_ntff_hook()

    per_core, flags = prepare_core_inputs(**inputs)
    if flags not in _PROG_CACHE:
        _PROG_CACHE[flags] = _build(flags)
    nc = _PROG_CACHE[flags]

    kw = {}
    if trace:
        kw["trace"] = True
        kw["trace_cores"] = [0]
        if trace_kwargs:
            kw["trace_kwargs"] = trace_kwargs
    res = run_bass_kernel_spmd(nc, per_core, core_ids=list(range(B)), **kw)

    out = np.empty((B, N, C), np.float32)
    for b in range(B):
        oT = res.results[b]["outT"]          # [128, KC, N]
        out[b] = oT.transpose(1, 0, 2).reshape(C, N).T
    return out, res


def kernel(**inputs):
    out, _ = run_sharded(inputs, trace=False)
    return out


# revision 23
# speedup vs baseline: 1.2424x; 1.2424x over previous
"""Trainium2 Bass kernel for nn_Block_89361089561275 (dense transformer block).

Sharding: data-parallel over batch B=8 -> one batch element per NeuronCore.
No collectives. Feature-transposed layout (features on SBUF partitions,
tokens on the free dim) throughout.

Key speed tricks over the bf16 baseline:
  * All big matmuls (qkv, v, proj, fc1, fc2, adapters) run as fp8e4m3
    DoubleRow matmuls: lhsT [128,2,M] + rhs [128,2,N] contract 256 K per
    instruction (~2x tensor-engine throughput). Weights are host-scaled
    (x32-ish) into fp8's sweet spot; inverse scales fold into downstream
    per-feature vectors / activation `scale=` operands.
  * Softmax exp is a Schraudolph bit-trick fused into the mandatory
    PSUM-evacuation op: the q/k weight scales carry alpha*beta =
    128*log2(e)*D^-0.5 so the score PSUM is already K2*s; one DVE
    scalar_tensor_tensor computes int16(round(st + 8192 + rpb_i8)) whose
    bf16 bit-pattern IS exp(s+r) up to a global power-of-2 scale that
    cancels in softmax. rpb ships as int8 = round(K2*rpb) (+8192 via the
    freed scalar slot). ScalarE does no exp at all.
  * The softmax denominator falls out of the AV matmul via a ones-column
    appended to v (M=65 accumulators, row 64 = colsum).
  * LN mean/var still come from ones-matmuls, but the finish chain runs on
    [1,N] rows and broadcasts rstd/-mean*rstd back across partitions with a
    K=1 ones matmul (no DRAM bounce round-trips).
  * fc1/fc2 weights are SBUF-resident (loaded once, streamed during
    attention), gelu/relu descale by 1/32 via the activation scale operand.
"""

import sys

for _p in ("/opt/trn_rl_repo",):
    if _p not in sys.path:
        sys.path.insert(0, _p)

import numpy as np
import ml_dtypes

BF16 = ml_dtypes.bfloat16
FP8 = ml_dtypes.float8_e4m3

B, N, C, H = 8, 1024, 768, 12
D = C // H            # 64
MLP = 4 * C           # 3072
RED = C // 3          # 256
EPS = 1e-5
P = 128
KC = C // P           # 6   c-chunks
KM = MLP // P         # 24  mlp-chunks
KR = RED // P         # 2   adapter chunks
NT = N // P           # 8   token tiles
HALF = 512
NSL = (slice(0, HALF), slice(HALF, N))

K2 = 128.0 / float(np.log(2.0))   # exponent-steps-per-unit-score for bf16
IBIAS = 8192.0                    # int16 exponent offset (2^-63 global scale)
SCALE = D ** -0.5
AB = K2 * SCALE                   # alpha*beta for the q/k weight pair
ALPHA = float(np.sqrt(AB))        # q-side host scale
BETA = float(np.sqrt(AB))         # k-side host scale
WS = 32.0                         # generic fp8 weight scale
OSC = 256.0                       # oT = OSC * o_true (8.0 folded in evac)

_PROG_CACHE: dict = {}

# indices into the packed [n, 128, KC] per-feature vector table
V_G1, V_G2, V_QB, V_PB, V_FB, V_L1G, V_L1B, V_L2G, V_L2B = range(9)
NVEC = 9


def _build(flags):
    """Build the single-core Bass program. flags is a tuple of bools:
    (has_mask, qb_nz, vb_nz, pb_nz, f1b_nz, fb_nz, adb_nz,
     ln1_triv, ln2_triv)
    """
    (has_mask, qb_nz, vb_nz, pb_nz, f1b_nz, fb_nz, adb_nz,
     ln1_triv, ln2_triv) = flags

    import concourse.tile as tile
    from concourse import bacc, mybir
    from contextlib import ExitStack

    f32 = mybir.dt.float32
    bf16 = mybir.dt.bfloat16
    fp8 = mybir.dt.float8e4
    i16 = mybir.dt.int16
    i8 = mybir.dt.int8
    AF = mybir.ActivationFunctionType
    OP = mybir.AluOpType
    PM = mybir.MatmulPerfMode

    nc = bacc.Bacc("TRN2")

    # ---- external I/O ----
    x_d = nc.declare_dram_parameter("xT", [P, KC, N], f32, isOutput=False)
    rpb8_d = nc.declare_dram_parameter("rpb8", [H, N, N], i8, isOutput=False)
    rpbf_d = nc.declare_dram_parameter("rpbf", [H, N, N], fp8, isOutput=False)
    id_d = nc.declare_dram_parameter("ident", [P, P], fp8, isOutput=False)
    wqk_d = nc.declare_dram_parameter("wqk", [12, P, KC, P], fp8, isOutput=False)
    wv_d = nc.declare_dram_parameter("wv", [P, KC, C], fp8, isOutput=False)
    pw_d = nc.declare_dram_parameter("projw", [KC, P, KC, P], fp8, isOutput=False)
    f1_d = nc.declare_dram_parameter("fc1w", [P, KM, KC, P], fp8, isOutput=False)
    f2_d = nc.declare_dram_parameter("fc2w", [P, KC, KM, P], fp8, isOutput=False)
    ad_d = nc.declare_dram_parameter("adw", [KR, P, KC, P], fp8, isOutput=False)
    au_d = nc.declare_dram_parameter("auw", [P, KC, KR, P], fp8, isOutput=False)
    vec_d = nc.declare_dram_parameter("vecs", [NVEC, P, KC], f32, isOutput=False)
    f1b_d = nc.declare_dram_parameter("fc1b", [P, KM], f32, isOutput=False)
    adb_d = nc.declare_dram_parameter("adb", [P, KR], f32, isOutput=False)
    vb_d = nc.declare_dram_parameter("vbias", [1, C], f32, isOutput=False)
    mb_d = nc.declare_dram_parameter("maskb", [P, NT], f32, isOutput=False)
    out_d = nc.declare_dram_parameter("outT", [P, KC, N], f32, isOutput=True)

    with tile.TileContext(nc) as tc, ExitStack() as ctx:
        sb = ctx.enter_context(tc.tile_pool(name="sb", bufs=1))
        pp = ctx.enter_context(tc.tile_pool(name="pp", bufs=1, space="PSUM"))
        dram = ctx.enter_context(tc.tile_pool(name="dram", bufs=2, space="DRAM"))

        # ---- persistent tiles ----
        xres = sb.tile([P, KC, N], f32, tag="xres", bufs=1)
        qkT = sb.tile([P, 12, N], bf16, tag="qkT", bufs=1)
        vaug = sb.tile([P, NT, H, D + 1], bf16, tag="vaug", bufs=1)
        oT = sb.tile([P, KC, N], fp8, tag="oT", bufs=1)
        xnT = sb.tile([P, KC, N], fp8, tag="xnT", bufs=1)
        xn2T = sb.tile([P, KC, N], fp8, tag="xn2T", bufs=1)
        a1T = sb.tile([P, KR, N], fp8, tag="a1", bufs=1)

        for ch in range(KC):  # per-chunk loads so LN1 stats start early
            nc.sync.dma_start(out=xres[:, ch], in_=x_d[:, ch])

        ones_bf = sb.tile([P, 1], bf16, tag="ones", bufs=1)
        nc.vector.memset(ones_bf, 1.0)
        ones_row = sb.tile([1, P], bf16, tag="onesr", bufs=1)
        nc.vector.memset(ones_row, 1.0)
        c8_row = sb.tile([1, P], bf16, tag="c8r", bufs=1)
        nc.vector.memset(c8_row, float(OSC / WS))
        nc.vector.memset(vaug[:, :, :, D:D + 1], 1.0)

        zero_col = sb.tile([P, 1], f32, tag="zcol", bufs=1)
        nc.vector.memset(zero_col, 0.0)
        eps_col = sb.tile([P, 1], f32, tag="ecol", bufs=1)
        nc.vector.memset(eps_col, float(EPS))

        vecs = sb.tile([P, NVEC, KC], f32, tag="vecs", bufs=1)
        nc.sync.dma_start(out=vecs, in_=vec_d[:].rearrange("v p k -> p v k"))

        def vec(i):
            return vecs[:, i]  # [128, KC]

        if f1b_nz:
            f1b = sb.tile([P, KM], f32, tag="f1b", bufs=1)
            nc.sync.dma_start(out=f1b, in_=f1b_d[:])
        if adb_nz:
            adb = sb.tile([P, KR], f32, tag="adb", bufs=1)
            nc.sync.dma_start(out=adb, in_=adb_d[:])
        if vb_nz:
            vb1 = sb.tile([1, C], f32, tag="vb1", bufs=1)
            nc.sync.dma_start(out=vb1, in_=vb_d[:])
            vb_b = sb.tile([P, C], f32, tag="vb_b", bufs=1)
            scratch = dram.tile([1, C], f32, tag="bscratch", bufs=2)
            nc.sync.dma_start(out=scratch, in_=vb1)
            nc.sync.dma_start(out=vb_b, in_=scratch.to_broadcast(vb_b.shape))
        if has_mask:
            maskb = sb.tile([P, NT], f32, tag="maskb", bufs=1)
            nc.sync.dma_start(out=maskb, in_=mb_d[:])

        # ---------------- layernorm (feature-transposed) ----------------
        # stats via ones-matmuls into two base-0 psum rows (sum, sum-sq) --
        # engine ops need all operands on the same partitions.
        def ln_stats_ch(stt_s, stt_q, ch, xbs, src):
            xb = sb.tile([P, N], bf16, tag="xb", bufs=KC)
            xbs.append(xb)
            nc.vector.tensor_copy(out=xb, in_=src[:, ch])
            x2 = sb.tile([P, N], bf16, tag="x2", bufs=2)
            nc.vector.tensor_mul(x2, xb, xb)
            for nk in range(2):
                nc.tensor.matmul(stt_s[0:1, NSL[nk]], lhsT=ones_bf,
                                 rhs=xb[:, NSL[nk]],
                                 start=(ch == 0), stop=(ch == KC - 1))
                nc.tensor.matmul(stt_q[0:1, NSL[nk]], lhsT=ones_bf,
                                 rhs=x2[:, NSL[nk]],
                                 start=(ch == 0), stop=(ch == KC - 1))

        def ln_finish(stt_s, stt_q, xbs, dst, g_i, b_i, triv):
            # evacuate sum rows via ACT (with the 1/C fold), scatter the
            # per-token stats across 128 lanes via DMA so sqrt/reciprocal
            # run wide, gather back, K=1 PE broadcast.
            mrow = sb.tile([1, N], f32, tag="lnrow", bufs=2)
            qrow = sb.tile([1, N], f32, tag="lnrow", bufs=2)
            nc.scalar.activation(out=mrow, in_=stt_s[0:1, :], func=AF.Copy,
                                 scale=1.0 / C)
            nc.scalar.activation(out=qrow, in_=stt_q[0:1, :], func=AF.Copy,
                                 scale=1.0 / C)
            s128 = sb.tile([P, 2, NT], f32, tag="s128", bufs=2)
            nc.sync.dma_start(out=s128[:, 0], in_=mrow)
            nc.sync.dma_start(out=s128[:, 1], in_=qrow)
            t128 = sb.tile([P, NT], f32, tag="t128", bufs=4)
            nc.vector.tensor_mul(t128, s128[:, 0], s128[:, 0])
            v128 = sb.tile([P, NT], f32, tag="t128", bufs=4)
            nc.vector.tensor_sub(v128, s128[:, 1], t128)
            sd = sb.tile([P, NT], f32, tag="t128", bufs=4)
            nc.scalar.activation(out=sd, in_=v128, func=AF.Sqrt,
                                 bias=eps_col)
            r2 = sb.tile([P, 2, NT], bf16, tag="r128", bufs=2)
            with nc.allow_low_precision(reason="bf16 rstd broadcast"):
                nc.vector.reciprocal(r2[:, 0], sd)             # rstd
            nc.vector.scalar_tensor_tensor(out=r2[:, 1], in0=s128[:, 0],
                                           scalar=-1.0, in1=r2[:, 0],
                                           op0=OP.mult, op1=OP.mult)
            rn = sb.tile([1, 2, N], bf16, tag="lnrn", bufs=1)
            nc.sync.dma_start(out=rn[0:1, 0], in_=r2[:, 0])
            nc.sync.dma_start(out=rn[0:1, 1], in_=r2[:, 1])
            # broadcast both rows across partitions via K=1 matmuls
            ab = sb.tile([P, 2, N], bf16, tag="lnab", bufs=1)
            for i in range(2):
                rb = pp.tile([P, N], f32, tag="st", bufs=2, name="lnrb")
                for nk in range(2):
                    nc.tensor.matmul(rb[:, NSL[nk]], lhsT=ones_row,
                                     rhs=rn[0:1, i, NSL[nk]],
                                     start=True, stop=True)
                nc.scalar.activation(out=ab[:, i], in_=rb, func=AF.Copy)
            for ch in range(KC):
                t1 = sb.tile([P, N], bf16, tag="x2", bufs=2)
                nc.vector.tensor_mul(t1, xbs[ch], ab[:, 0])
                if triv:
                    nc.vector.tensor_add(dst[:, ch], t1, ab[:, 1])
                else:
                    nc.vector.tensor_add(t1, t1, ab[:, 1])
                    nc.vector.tensor_scalar(
                        out=dst[:, ch], in0=t1,
                        scalar1=vec(g_i)[:, ch:ch + 1],
                        scalar2=vec(b_i)[:, ch:ch + 1],
                        op0=OP.mult, op1=OP.add)

        # ---------------- LN1 -> xnT (fp8) ----------------
        stt1s = pp.tile([1, N], f32, tag="st", bufs=2, name="ln_s")
        stt1q = pp.tile([1, N], f32, tag="st", bufs=2, name="ln_q")
        xbs1 = []
        for ch in range(KC):
            ln_stats_ch(stt1s, stt1q, ch, xbs1, xres)
        ln_finish(stt1s, stt1q, xbs1, xnT, V_L1G, V_L1B, ln1_triv)

        # ---------------- QKV (fp8 DoubleRow) ----------------
        def dr_mms(acc, wt, rhs_t, nsl, kd=KC):
            for j in range(kd // 2):
                nc.tensor.matmul(acc, lhsT=wt[:, 2 * j:2 * j + 2, :],
                                 rhs=rhs_t[:, 2 * j:2 * j + 2, nsl],
                                 start=(j == 0), stop=(j == kd // 2 - 1),
                                 perf_mode=PM.DoubleRow)

        for blk in (0, 6, 1, 7, 2, 8, 3, 9, 4, 10, 5, 11):
            wt = sb.tile([P, KC, P], fp8, tag="w6", bufs=3)
            nc.sync.dma_start(out=wt, in_=wqk_d[blk])
            for nk in range(2):
                mm = pp.tile([P, HALF], f32, tag="acc", bufs=4)
                dr_mms(mm, wt, xnT, NSL[nk])
                dst = qkT[:, blk, NSL[nk]]
                if blk < 6 and qb_nz:
                    nc.vector.tensor_scalar_add(dst, mm, vec(V_QB)[:, blk:blk + 1])
                else:
                    nc.scalar.copy(out=dst, in_=mm)

        wv_sb = sb.tile([P, KC, C], fp8, tag="h1", bufs=1)
        nc.sync.dma_start(out=wv_sb, in_=wv_d[:])
        for t in range(NT):
            for off, cw in ((0, HALF), (HALF, C - HALF)):
                mm = pp.tile([P, HALF], f32, tag="acc", bufs=4)
                for j in range(KC // 2):
                    nc.tensor.matmul(
                        mm[:, :cw], lhsT=xnT[:, 2 * j:2 * j + 2, t * P:(t + 1) * P],
                        rhs=wv_sb[:, 2 * j:2 * j + 2, off:off + cw],
                        start=(j == 0), stop=(j == KC // 2 - 1),
                        perf_mode=PM.DoubleRow)
                dst = vaug[:, t, off // D:(off + cw) // D, :D]
                src = mm[:, :cw].rearrange("p (h d) -> p h d", d=D)
                if vb_nz:
                    nc.vector.tensor_add(
                        dst, src,
                        vb_b[:, off:off + cw].rearrange("p (h d) -> p h d", d=D))
                else:
                    nc.vector.tensor_copy(out=dst, in_=src)

        # resident MLP weights: queue the DMAs here so they land during
        # attention (after the first head pairs' rpb tiles)
        f1sb = sb.tile([P, KM, KC, P], fp8, tag="f1sb", bufs=1)
        f2sb = sb.tile([P, KC, KM, P], fp8, tag="f2sb", bufs=1)
        ausb = sb.tile([P, KC, KR, P], fp8, tag="ausb", bufs=1)

        # ---------------- attention ----------------
        ident_sb = sb.tile([P, P], fp8, tag="ident", bufs=1)
        nc.sync.dma_start(out=ident_sb, in_=id_d[:])

        def evac_head(o_ps, hp, hh):
            # ACT evacuates the accumulators (frees PSUM fast); colsum row
            # scatters across 128 lanes for the reciprocal, gathers back,
            # K=1 PE broadcast, one bf16 2x DVE multiply.
            ou = sb.tile([D + 1, N], bf16, tag="ou", bufs=2)
            for nk in range(2):
                nc.scalar.activation(out=ou[:, NSL[nk]], in_=o_ps[nk],
                                     func=AF.Copy)
            cs = sb.tile([P, NT], bf16, tag="cs", bufs=2)
            nc.sync.dma_start(out=cs, in_=ou[D:D + 1, :])
            rcp = sb.tile([P, NT], bf16, tag="cs", bufs=2)
            with nc.allow_low_precision(reason="bf16 softmax denom"):
                nc.vector.reciprocal(rcp, cs)
            rc = sb.tile([1, N], bf16, tag="rc", bufs=2)
            nc.sync.dma_start(out=rc, in_=rcp)
            rb = pp.tile([P, N], f32, tag="st", bufs=2, name="evrb")
            for nk in range(2):
                nc.tensor.matmul(rb[:, NSL[nk]], lhsT=c8_row,
                                 rhs=rc[0:1, NSL[nk]], start=True, stop=True)
            rbs = sb.tile([P, N], bf16, tag="rbs", bufs=2)
            nc.scalar.activation(out=rbs, in_=rb, func=AF.Copy)
            if hh == 0:
                nc.vector.tensor_mul(oT[0:D, hp, :], ou[0:D, :], rbs[0:D, :])
            else:
                ot_tmp = sb.tile([D, N], fp8, tag="ott", bufs=2)
                nc.vector.tensor_mul(ot_tmp, ou[0:D, :], rbs[0:D, :])
                nc.sync.dma_start(out=oT[D:P, hp, :], in_=ot_tmp)

        for hp in range(H // 2):
            qh = [qkT[hh * D:(hh + 1) * D, hp, :] for hh in range(2)]
            kh = [qkT[hh * D:(hh + 1) * D, 6 + hp, :] for hh in range(2)]
            o_ps = [[pp.tile([D + 1, HALF], f32, tag="acc", bufs=4,
                             name=f"o_ps{hh}{nk}") for nk in range(2)]
                    for hh in range(2)]
            if hp == 2:  # let early rpb tiles through the queue first
                nc.sync.dma_start(out=f1sb, in_=f1_d[:])
            if hp == 3:
                nc.sync.dma_start(out=f2sb, in_=f2_d[:])
                nc.sync.dma_start(out=ausb, in_=au_d[:])
            for mt in range(NT):
                # hh=1 tiles mostly take the "ACT path": rpb is accumulated
                # into the score PSUM by an fp8 identity matmul, and the
                # +8192/int16 conversion runs on the (otherwise idle)
                # scalar engine; hh=0 (and the rest) use the DVE
                # scalar_tensor_tensor with int8 rpb. This splits the
                # unavoidable fp32-PSUM evacuation across both engines.
                act_path = [False, mt % 4 != 3]
                sts, rps = [], []
                for hh in range(2):
                    h = 2 * hp + hh
                    msl = slice(mt * P, (mt + 1) * P)
                    if act_path[hh]:
                        rp = sb.tile([P, N], fp8, tag="rpf", bufs=4)
                        nc.sync.dma_start(out=rp, in_=rpbf_d[h, msl, :])
                    else:
                        rp = sb.tile([P, N], i8, tag="rpb", bufs=6)
                        nc.sync.dma_start(out=rp, in_=rpb8_d[h, msl, :])
                    rps.append(rp)
                    sts.append(pp.tile([P, N], f32, tag="st", bufs=2, name="st"))
                for nk in range(2):  # hh-pairs issue adjacently -> row-group
                    for hh in range(2):  # concurrency on the PE
                        nc.tensor.matmul(sts[hh][:, NSL[nk]],
                                         lhsT=kh[hh][:, mt * P:(mt + 1) * P],
                                         rhs=qh[hh][:, NSL[nk]],
                                         start=True, stop=not act_path[hh])
                    for hh in range(2):
                        if act_path[hh]:
                            nc.tensor.matmul(sts[hh][:, NSL[nk]],
                                             lhsT=ident_sb,
                                             rhs=rps[hh][:, NSL[nk]],
                                             start=False, stop=True)
                pts = []
                for hh in range(2):
                    pt = sb.tile([P, N], i16, tag="pt", bufs=4)
                    pts.append(pt)
                    # int16(st + 8192 + rpb) -- bit-pattern is bf16 exp(s+r)
                    if act_path[hh]:
                        nc.scalar.activation(out=pt, in_=sts[hh],
                                             func=AF.Copy, bias=IBIAS)
                    else:
                        nc.vector.scalar_tensor_tensor(
                            out=pt, in0=sts[hh], scalar=IBIAS, in1=rps[hh],
                            op0=OP.add, op1=OP.add)
                    if has_mask:
                        nc.vector.tensor_scalar_mul(pt, pt, maskb[:, mt:mt + 1])
                for hh in range(2):
                    for nk in range(2):
                        nc.tensor.matmul(
                            o_ps[hh][nk][:, :],
                            lhsT=vaug[:, mt, 2 * hp + hh, :],
                            rhs=pts[hh][:, NSL[nk]].bitcast(bf16),
                            start=(mt == 0), stop=(mt == NT - 1))
            evac_head(o_ps[0], hp, 0)
            evac_head(o_ps[1], hp, 1)

        # ---------------- proj + residual 1 + LN2 stats ----------------
        stt2s = pp.tile([1, N], f32, tag="st", bufs=2, name="ln2_s")
        stt2q = pp.tile([1, N], f32, tag="st", bufs=2, name="ln2_q")
        xbs2 = []
        for mt in range(KC):
            wt = sb.tile([P, KC, P], fp8, tag="w6", bufs=3)
            nc.sync.dma_start(out=wt, in_=pw_d[mt])
            for nk in range(2):
                mm = pp.tile([P, HALF], f32, tag="acc", bufs=4)
                dr_mms(mm, wt, oT, NSL[nk])
                if pb_nz:
                    nc.vector.tensor_scalar_add(mm, mm, vec(V_PB)[:, mt:mt + 1])
                nc.vector.scalar_tensor_tensor(
                    out=xres[:, mt, NSL[nk]], in0=mm,
                    scalar=vec(V_G1)[:, mt:mt + 1],
                    in1=xres[:, mt, NSL[nk]], op0=OP.mult, op1=OP.add)
            ln_stats_ch(stt2s, stt2q, mt, xbs2, xres)
        ln_finish(stt2s, stt2q, xbs2, xn2T, V_L2G, V_L2B, ln2_triv)

        # ---------------- adapter down (relu via ACT, 1/32 descale) -----
        for mt in range(KR):
            wt = sb.tile([P, KC, P], fp8, tag="w6", bufs=3)
            nc.sync.dma_start(out=wt, in_=ad_d[mt])
            for nk in range(2):
                mm = pp.tile([P, HALF], f32, tag="acc", bufs=4)
                dr_mms(mm, wt, xn2T, NSL[nk])
                nc.scalar.activation(
                    out=a1T[:, mt, NSL[nk]], in_=mm, func=AF.Relu,
                    scale=1.0 / WS,
                    bias=(adb[:, mt:mt + 1] if adb_nz else zero_col))

        # ---------------- MLP (fp8 DR, resident weights) ----------------
        for nk in range(2):
            h1 = sb.tile([P, KM, HALF], fp8, tag="h1", bufs=1)
            for mt in range(KM):
                mm = pp.tile([P, HALF], f32, tag="acc", bufs=4)
                dr_mms(mm, f1sb[:, mt], xn2T, NSL[nk])
                nc.scalar.activation(
                    out=h1[:, mt], in_=mm, func=AF.Gelu, scale=1.0 / WS,
                    bias=(f1b[:, mt:mt + 1] if f1b_nz else zero_col))
            for mt in range(KC):
                mm = pp.tile([P, HALF], f32, tag="acc", bufs=4)
                for j in range(KM // 2):
                    nc.tensor.matmul(mm, lhsT=f2sb[:, mt, 2 * j:2 * j + 2, :],
                                     rhs=h1[:, 2 * j:2 * j + 2, :],
                                     start=(j == 0), stop=False,
                                     perf_mode=PM.DoubleRow)
                nc.tensor.matmul(mm, lhsT=ausb[:, mt],
                                 rhs=a1T[:, 0:KR, NSL[nk]],
                                 start=False, stop=True,
                                 perf_mode=PM.DoubleRow)
                if fb_nz:
                    nc.vector.tensor_scalar_add(mm, mm, vec(V_FB)[:, mt:mt + 1])
                nc.vector.scalar_tensor_tensor(
                    out=xres[:, mt, NSL[nk]], in0=mm,
                    scalar=vec(V_G2)[:, mt:mt + 1],
                    in1=xres[:, mt, NSL[nk]], op0=OP.mult, op1=OP.add)
                nc.sync.dma_start(out=out_d[:, mt, NSL[nk]],
                                  in_=xres[:, mt, NSL[nk]])

    if not nc.is_finalized():
        nc.finalize()
    return nc


def _pack_w6(wT, km, kk):
    """[K, M] (K=contraction, M=out) -> [M//128, 128, K//128, 128] fp8 tiles
    laid out so each DMA partition read is contiguous."""
    K, M = wT.shape
    assert K == kk * P and M == km * P
    a = wT.reshape(kk, P, km, P)          # [ks, p, mt, col]
    return np.ascontiguousarray(a.transpose(2, 1, 0, 3)).astype(FP8)


def _pack_res(wT, km, kk):
    """[K, M] -> [128, M//128, K//128, 128] partition-major for one-shot
    SBUF-resident DMA (contiguous per-partition reads)."""
    K, M = wT.shape
    assert K == kk * P and M == km * P
    a = wT.reshape(kk, P, km, P)          # [ks, p, mt, col]
    return np.ascontiguousarray(a.transpose(1, 2, 0, 3)).astype(FP8)


def _stripe(v, k):
    """[k*128] -> [128, k] with v[ks*128+p] at [p, ks]."""
    return np.ascontiguousarray(v.reshape(k, P).T).astype(np.float32)


def prepare_core_inputs(x, mask, rpb, ln1_g, ln1_b, qkv_w, q_bias, v_bias,
                        proj_w, proj_b, gamma1, ln2_g, ln2_b, fc1_w, fc1_b,
                        fc2_w, fc2_b, ad_dw, ad_db, ad_uw, ad_ub, gamma2):
    """Host-side layout prep. Returns (per_core_maps, flags)."""
    f32 = np.float32

    qkv_w = np.asarray(qkv_w, f32)
    wq = qkv_w[:C] * ALPHA            # q-side carries alpha (incl. D^-.5*K2)
    wk = qkv_w[C:2 * C] * BETA
    wv = qkv_w[2 * C:] * WS
    wqkT = np.concatenate([wq, wk], 0).T          # [C, 1536]
    wqk = _pack_w6(wqkT, 12, KC)
    wv_packed = np.ascontiguousarray(
        wv.T.reshape(KC, P, C).transpose(1, 0, 2)).astype(FP8)

    projw = _pack_w6(np.asarray(proj_w, f32).T * WS, KC, KC)
    fc1w = _pack_res(np.asarray(fc1_w, f32).T * WS, KM, KC)
    fc2w = _pack_res(np.asarray(fc2_w, f32).T * WS, KC, KM)
    adw = _pack_w6(np.asarray(ad_dw, f32).T * WS, KR, KC)
    auw = _pack_res(np.asarray(ad_uw, f32).T * WS, KC, KR)

    # rpb as exponent steps K2*rpb in two formats: int8 for the DVE
    # schraudolph path, fp8 for the identity-matmul (ACT) path
    rpbk = K2 * np.ascontiguousarray(np.asarray(rpb, f32).transpose(0, 2, 1))
    rpb8 = np.clip(np.round(rpbk), -127, 127).astype(np.int8)
    rpbf = rpbk.astype(FP8)
    ident = np.eye(P).astype(FP8)

    q_bias_s = np.asarray(q_bias, f32) * ALPHA
    fb = (np.asarray(fc2_b, f32) + np.asarray(ad_ub, f32)) * WS

    vecs = np.stack([
        _stripe(np.asarray(gamma1, f32) / (OSC * WS), KC),
        _stripe(np.asarray(gamma2, f32) / WS, KC),
        _stripe(q_bias_s, KC),
        _stripe(np.asarray(proj_b, f32) * OSC * WS, KC),
        _stripe(fb, KC),
        _stripe(np.asarray(ln1_g, f32), KC),
        _stripe(np.asarray(ln1_b, f32), KC),
        _stripe(np.asarray(ln2_g, f32), KC),
        _stripe(np.asarray(ln2_b, f32), KC),
    ], 0)  # [NVEC, 128, KC]

    f1b = _stripe(np.asarray(fc1_b, f32), KM)
    adb = _stripe(np.asarray(ad_db, f32), KR)
    vb = (np.asarray(v_bias, f32) * WS).reshape(1, C).astype(f32)

    mask = np.asarray(mask)
    has_mask = not bool(mask.all())

    flags = (
        has_mask,
        bool(np.any(q_bias_s)),
        bool(np.any(v_bias)),
        bool(np.any(proj_b)),
        bool(np.any(fc1_b)),
        bool(np.any(fb)),
        bool(np.any(ad_db)),
        bool(np.all(ln1_g == 1.0) and not np.any(ln1_b)),
        bool(np.all(ln2_g == 1.0) and not np.any(ln2_b)),
    )

    shared = {
        "rpb8": rpb8, "rpbf": rpbf, "ident": ident,
        "wqk": wqk, "wv": wv_packed, "projw": projw,
        "fc1w": fc1w, "fc2w": fc2w, "adw": adw, "auw": auw,
        "vecs": vecs, "fc1b": f1b, "adb": adb, "vbias": vb,
    }

    x = np.asarray(x, f32)
    per_core = []
    for b in range(B):
        xT = np.ascontiguousarray(
            x[b].T.reshape(KC, P, N).transpose(1, 0, 2)).astype(f32)
        if has_mask:
            mb = np.where(mask[b], 1.0, 0.0).astype(f32)    # [N] over keys m
            mb = np.ascontiguousarray(mb.reshape(NT, P).T)  # [128, NT]
        else:
            mb = np.zeros((P, NT), f32)
        m = dict(shared)
        m["xT"] = xT
        m["maskb"] = mb
        per_core.append(m)
    return per_core, flags


def _ensure_ntff_hook():
    """The agent image lacks ``antenv.axon_hooks``; provide it and register
    the ctypes NTFF profile hook so trace=True works under axon."""
    import types
    try:
        from antenv.axon_hooks import get_axon_ntff_profile_hook  # noqa: F401
        return
    except ImportError:
        pass
    import antenv
    mod = types.ModuleType("antenv.axon_hooks")
    _h = {"hook": None}
    mod.set_axon_ntff_profile_hook = lambda h: _h.__setitem__("hook", h)
    mod.get_axon_ntff_profile_hook = lambda: _h["hook"]
    sys.modules["antenv.axon_hooks"] = mod
    antenv.axon_hooks = mod
    try:
        from trn_agent_boot.trn_boot import _ntff_profile_via_ctypes
        hook = _ntff_profile_via_ctypes("/opt/axon/libaxon_pjrt.so")
        if hook is not None:
            mod.set_axon_ntff_profile_hook(hook)
    except Exception as e:  # profiling degrades, run still works
        print("ntff hook setup failed:", e)


def run_sharded(inputs, trace=False, trace_kwargs=None):
    """Compile (cached) + run on 8 cores. Returns (out [B,N,C] f32, results)."""
    from concourse.bass_utils import run_bass_kernel_spmd
    if trace:
        _ensure_ntff_hook()

    per_core, flags = prepare_core_inputs(**inputs)
    if flags not in _PROG_CACHE:
        _PROG_CACHE[flags] = _build(flags)
    nc = _PROG_CACHE[flags]

    kw = {}
    if trace:
        kw["trace"] = True
        kw["trace_cores"] = [0]
        if trace_kwargs:
            kw["trace_kwargs"] = trace_kwargs
    res = run_bass_kernel_spmd(nc, per_core, core_ids=list(range(B)), **kw)

    out = np.empty((B, N, C), np.float32)
    for b in range(B):
        oT = res.results[b]["outT"]          # [128, KC, N]
        out[b] = oT.transpose(1, 0, 2).reshape(C, N).T
    return out, res


def kernel(**inputs):
    out, _ = run_sharded(inputs, trace=False)
    return out


# revision 27
# speedup vs baseline: 1.5279x; 1.2298x over previous
"""Trainium2 Bass kernel for nn_Block_89361089561275 (dense transformer block).

Sharding: data-parallel over batch B=8 -> one batch element per NeuronCore.
No collectives. Feature-transposed layout (features on SBUF partitions,
tokens on the free dim) throughout.

Key speed tricks over the bf16 baseline:
  * All big matmuls (qkv, v, proj, fc1, fc2, adapters) run as fp8e4m3
    DoubleRow matmuls: lhsT [128,2,M] + rhs [128,2,N] contract 256 K per
    instruction (~2x tensor-engine throughput). Weights are host-scaled
    (x32-ish) into fp8's sweet spot; inverse scales fold into downstream
    per-feature vectors / activation `scale=` operands.
  * Softmax exp is a Schraudolph bit-trick fused into the mandatory
    PSUM-evacuation op: the q/k weight scales carry alpha*beta =
    128*log2(e)*D^-0.5 so the score PSUM is already K2*s; one DVE
    scalar_tensor_tensor computes int16(round(st + 8192 + rpb_i8)) whose
    bf16 bit-pattern IS exp(s+r) up to a global power-of-2 scale that
    cancels in softmax. rpb ships as int8 = round(K2*rpb) (+8192 via the
    freed scalar slot). ScalarE does no exp at all.
  * The softmax denominator falls out of the AV matmul via a ones-column
    appended to v (M=65 accumulators, row 64 = colsum).
  * LN mean/var still come from ones-matmuls, but the finish chain runs on
    [1,N] rows and broadcasts rstd/-mean*rstd back across partitions with a
    K=1 ones matmul (no DRAM bounce round-trips).
  * fc1/fc2 weights are SBUF-resident (loaded once, streamed during
    attention), gelu/relu descale by 1/32 via the activation scale operand.
"""

import sys

for _p in ("/opt/trn_rl_repo",):
    if _p not in sys.path:
        sys.path.insert(0, _p)

import numpy as np
import ml_dtypes

BF16 = ml_dtypes.bfloat16
FP8 = ml_dtypes.float8_e4m3

B, N, C, H = 8, 1024, 768, 12
D = C // H            # 64
MLP = 4 * C           # 3072
RED = C // 3          # 256
EPS = 1e-5
P = 128
KC = C // P           # 6   c-chunks
KM = MLP // P         # 24  mlp-chunks
KR = RED // P         # 2   adapter chunks
NT = N // P           # 8   token tiles
HALF = 512
NSL = (slice(0, HALF), slice(HALF, N))

K2 = 128.0 / float(np.log(2.0))   # exponent-steps-per-unit-score for bf16
IBIAS = 8192.0                    # int16 exponent offset (2^-63 global scale)
SCALE = D ** -0.5
AB = K2 * SCALE                   # alpha*beta for the q/k weight pair
ALPHA = float(np.sqrt(AB))        # q-side host scale
BETA = float(np.sqrt(AB))         # k-side host scale
WS = 32.0                         # generic fp8 weight scale
OSC = 256.0                       # oT = OSC * o_true (8.0 folded in evac)

_PROG_CACHE: dict = {}

# indices into the packed [n, 128, KC] per-feature vector table
V_G1, V_G2, V_QB, V_PB, V_FB, V_L1G, V_L1B, V_L2G, V_L2B = range(9)
NVEC = 9


def _build(flags):
    """Build the single-core Bass program. flags is a tuple of bools:
    (has_mask, qb_nz, vb_nz, pb_nz, f1b_nz, fb_nz, adb_nz,
     ln1_triv, ln2_triv)
    """
    (has_mask, qb_nz, vb_nz, pb_nz, f1b_nz, fb_nz, adb_nz,
     ln1_triv, ln2_triv) = flags

    import concourse.tile as tile
    from concourse import bacc, mybir
    from contextlib import ExitStack

    f32 = mybir.dt.float32
    bf16 = mybir.dt.bfloat16
    fp8 = mybir.dt.float8e4
    i16 = mybir.dt.int16
    i8 = mybir.dt.int8
    AF = mybir.ActivationFunctionType
    OP = mybir.AluOpType
    PM = mybir.MatmulPerfMode

    nc = bacc.Bacc("TRN2")

    # ---- external I/O ----
    x_d = nc.declare_dram_parameter("xT", [P, KC, N], f32, isOutput=False)
    rpb8_d = nc.declare_dram_parameter("rpb8", [H, N, N], i8, isOutput=False)
    rpbf_d = nc.declare_dram_parameter("rpbf", [H, N, N], fp8, isOutput=False)
    id_d = nc.declare_dram_parameter("ident", [P, P], fp8, isOutput=False)
    wqk_d = nc.declare_dram_parameter("wqk", [12, P, KC, P], fp8, isOutput=False)
    wv_d = nc.declare_dram_parameter("wv", [P, KC, C], fp8, isOutput=False)
    pw_d = nc.declare_dram_parameter("projw", [KC, P, KC, P], fp8, isOutput=False)
    f1_d = nc.declare_dram_parameter("fc1w", [P, KM, KC, P], fp8, isOutput=False)
    f2_d = nc.declare_dram_parameter("fc2w", [P, KC, KM, P], fp8, isOutput=False)
    ad_d = nc.declare_dram_parameter("adw", [KR, P, KC, P], fp8, isOutput=False)
    au_d = nc.declare_dram_parameter("auw", [P, KC, KR, P], fp8, isOutput=False)
    vec_d = nc.declare_dram_parameter("vecs", [NVEC, P, KC], f32, isOutput=False)
    f1b_d = nc.declare_dram_parameter("fc1b", [P, KM], f32, isOutput=False)
    adb_d = nc.declare_dram_parameter("adb", [P, KR], f32, isOutput=False)
    vb_d = nc.declare_dram_parameter("vbias", [1, C], f32, isOutput=False)
    mb_d = nc.declare_dram_parameter("maskb", [P, NT], f32, isOutput=False)
    out_d = nc.declare_dram_parameter("outT", [P, KC, N], f32, isOutput=True)

    with tile.TileContext(nc) as tc, ExitStack() as ctx:
        sb = ctx.enter_context(tc.tile_pool(name="sb", bufs=1))
        pp = ctx.enter_context(tc.tile_pool(name="pp", bufs=1, space="PSUM"))
        dram = ctx.enter_context(tc.tile_pool(name="dram", bufs=2, space="DRAM"))

        # ---- persistent tiles ----
        xres = sb.tile([P, KC, N], f32, tag="xres", bufs=1)
        qkT = sb.tile([P, 12, N], bf16, tag="qkT", bufs=1)
        vaug = sb.tile([P, NT, H, D + 1], bf16, tag="vaug", bufs=1)
        oT = sb.tile([P, KC, N], fp8, tag="oT", bufs=1)
        xnT = sb.tile([P, KC, N], fp8, tag="xnT", bufs=1)
        xn2T = sb.tile([P, KC, N], fp8, tag="xn2T", bufs=1)
        a1T = sb.tile([P, KR, N], fp8, tag="a1", bufs=1)

        for ch in range(KC):  # per-chunk loads so LN1 stats start early
            nc.sync.dma_start(out=xres[:, ch], in_=x_d[:, ch])

        ones_bf = sb.tile([P, 1], bf16, tag="ones", bufs=1)
        nc.vector.memset(ones_bf, 1.0)
        ones_row = sb.tile([1, P], bf16, tag="onesr", bufs=1)
        nc.vector.memset(ones_row, 1.0)
        c8_row = sb.tile([1, P], bf16, tag="c8r", bufs=1)
        nc.vector.memset(c8_row, float(OSC / WS))
        nc.vector.memset(vaug[:, :, :, D:D + 1], 1.0)

        zero_col = sb.tile([P, 1], f32, tag="zcol", bufs=1)
        nc.vector.memset(zero_col, 0.0)
        eps_col = sb.tile([P, 1], f32, tag="ecol", bufs=1)
        nc.vector.memset(eps_col, float(EPS))

        vecs = sb.tile([P, NVEC, KC], f32, tag="vecs", bufs=1)
        nc.sync.dma_start(out=vecs, in_=vec_d[:].rearrange("v p k -> p v k"))

        def vec(i):
            return vecs[:, i]  # [128, KC]

        if f1b_nz:
            f1b = sb.tile([P, KM], f32, tag="f1b", bufs=1)
            nc.sync.dma_start(out=f1b, in_=f1b_d[:])
        if adb_nz:
            adb = sb.tile([P, KR], f32, tag="adb", bufs=1)
            nc.sync.dma_start(out=adb, in_=adb_d[:])
        if vb_nz:
            vb1 = sb.tile([1, C], f32, tag="vb1", bufs=1)
            nc.sync.dma_start(out=vb1, in_=vb_d[:])
            vb_b = sb.tile([P, C], f32, tag="vb_b", bufs=1)
            scratch = dram.tile([1, C], f32, tag="bscratch", bufs=2)
            nc.sync.dma_start(out=scratch, in_=vb1)
            nc.sync.dma_start(out=vb_b, in_=scratch.to_broadcast(vb_b.shape))
        if has_mask:
            maskb = sb.tile([P, NT], f32, tag="maskb", bufs=1)
            nc.sync.dma_start(out=maskb, in_=mb_d[:])

        # ---------------- layernorm (feature-transposed) ----------------
        # stats via ones-matmuls into two base-0 psum rows (sum, sum-sq) --
        # engine ops need all operands on the same partitions.
        def ln_stats_ch(stt_s, stt_q, ch, xbs, src):
            xb = sb.tile([P, N], bf16, tag="xb", bufs=KC)
            xbs.append(xb)
            nc.vector.tensor_copy(out=xb, in_=src[:, ch])
            x2 = sb.tile([P, N], bf16, tag="x2", bufs=2)
            nc.vector.tensor_mul(x2, xb, xb)
            for nk in range(2):
                nc.tensor.matmul(stt_s[0:1, NSL[nk]], lhsT=ones_bf,
                                 rhs=xb[:, NSL[nk]],
                                 start=(ch == 0), stop=(ch == KC - 1))
                nc.tensor.matmul(stt_q[0:1, NSL[nk]], lhsT=ones_bf,
                                 rhs=x2[:, NSL[nk]],
                                 start=(ch == 0), stop=(ch == KC - 1))

        def ln_finish(stt_s, stt_q, xbs, dst, g_i, b_i, triv):
            # evacuate sum rows via ACT (with the 1/C fold), scatter the
            # per-token stats across 128 lanes via DMA so sqrt/reciprocal
            # run wide, gather back, K=1 PE broadcast.
            mrow = sb.tile([1, N], f32, tag="lnrow", bufs=2)
            qrow = sb.tile([1, N], f32, tag="lnrow", bufs=2)
            nc.scalar.activation(out=mrow, in_=stt_s[0:1, :], func=AF.Copy,
                                 scale=1.0 / C)
            nc.scalar.activation(out=qrow, in_=stt_q[0:1, :], func=AF.Copy,
                                 scale=1.0 / C)
            s128 = sb.tile([P, 2, NT], f32, tag="s128", bufs=2)
            nc.sync.dma_start(out=s128[:, 0], in_=mrow)
            nc.sync.dma_start(out=s128[:, 1], in_=qrow)
            t128 = sb.tile([P, NT], f32, tag="t128", bufs=4)
            nc.vector.tensor_mul(t128, s128[:, 0], s128[:, 0])
            v128 = sb.tile([P, NT], f32, tag="t128", bufs=4)
            nc.vector.tensor_sub(v128, s128[:, 1], t128)
            sd = sb.tile([P, NT], f32, tag="t128", bufs=4)
            nc.scalar.activation(out=sd, in_=v128, func=AF.Sqrt,
                                 bias=eps_col)
            r2 = sb.tile([P, 2, NT], bf16, tag="r128", bufs=2)
            with nc.allow_low_precision(reason="bf16 rstd broadcast"):
                nc.vector.reciprocal(r2[:, 0], sd)             # rstd
            nc.vector.scalar_tensor_tensor(out=r2[:, 1], in0=s128[:, 0],
                                           scalar=-1.0, in1=r2[:, 0],
                                           op0=OP.mult, op1=OP.mult)
            # gather to DRAM, broadcast across partitions (DMA-only chain —
            # no PSUM slots or engine time on the critical path)
            scr = dram.tile([1, 2, N], bf16, tag="lnscr", bufs=2)
            nc.sync.dma_start(out=scr[0:1, 0], in_=r2[:, 0])
            nc.sync.dma_start(out=scr[0:1, 1], in_=r2[:, 1])
            ab = sb.tile([P, 2, N], bf16, tag="lnab", bufs=1)
            nc.sync.dma_start(out=ab, in_=scr.to_broadcast(ab.shape))
            for ch in range(KC):
                t1 = sb.tile([P, N], bf16, tag="x2", bufs=2)
                nc.vector.tensor_mul(t1, xbs[ch], ab[:, 0])
                if triv:
                    nc.vector.tensor_add(dst[:, ch], t1, ab[:, 1])
                else:
                    nc.vector.tensor_add(t1, t1, ab[:, 1])
                    nc.vector.tensor_scalar(
                        out=dst[:, ch], in0=t1,
                        scalar1=vec(g_i)[:, ch:ch + 1],
                        scalar2=vec(b_i)[:, ch:ch + 1],
                        op0=OP.mult, op1=OP.add)

        # ---------------- LN1 -> xnT (fp8) ----------------
        stt1s = pp.tile([1, N], f32, tag="st", bufs=2, name="ln_s")
        stt1q = pp.tile([1, N], f32, tag="st", bufs=2, name="ln_q")
        xbs1 = []
        for ch in range(KC):
            ln_stats_ch(stt1s, stt1q, ch, xbs1, xres)
        ln_finish(stt1s, stt1q, xbs1, xnT, V_L1G, V_L1B, ln1_triv)

        # ---------------- QKV (fp8 DoubleRow) ----------------
        def dr_mms(acc, wt, rhs_t, nsl, kd=KC):
            for j in range(kd // 2):
                nc.tensor.matmul(acc, lhsT=wt[:, 2 * j:2 * j + 2, :],
                                 rhs=rhs_t[:, 2 * j:2 * j + 2, nsl],
                                 start=(j == 0), stop=(j == kd // 2 - 1),
                                 perf_mode=PM.DoubleRow)

        for blk in (0, 6, 1, 7, 2, 8, 3, 9, 4, 10, 5, 11):
            wt = sb.tile([P, KC, P], fp8, tag="w6", bufs=3)
            nc.sync.dma_start(out=wt, in_=wqk_d[blk])
            for nk in range(2):
                mm = pp.tile([P, HALF], f32, tag="acc", bufs=4)
                dr_mms(mm, wt, xnT, NSL[nk])
                dst = qkT[:, blk, NSL[nk]]
                if blk < 6 and qb_nz:
                    nc.vector.tensor_scalar_add(dst, mm, vec(V_QB)[:, blk:blk + 1])
                else:
                    nc.scalar.copy(out=dst, in_=mm)

        wv_sb = sb.tile([P, KC, C], fp8, tag="h1", bufs=1)
        nc.sync.dma_start(out=wv_sb, in_=wv_d[:])
        for t in range(NT):
            for off, cw in ((0, HALF), (HALF, C - HALF)):
                mm = pp.tile([P, HALF], f32, tag="acc", bufs=4)
                for j in range(KC // 2):
                    nc.tensor.matmul(
                        mm[:, :cw], lhsT=xnT[:, 2 * j:2 * j + 2, t * P:(t + 1) * P],
                        rhs=wv_sb[:, 2 * j:2 * j + 2, off:off + cw],
                        start=(j == 0), stop=(j == KC // 2 - 1),
                        perf_mode=PM.DoubleRow)
                dst = vaug[:, t, off // D:(off + cw) // D, :D]
                src = mm[:, :cw].rearrange("p (h d) -> p h d", d=D)
                if vb_nz:
                    nc.vector.tensor_add(
                        dst, src,
                        vb_b[:, off:off + cw].rearrange("p (h d) -> p h d", d=D))
                else:
                    nc.vector.tensor_copy(out=dst, in_=src)

        # resident MLP weights: queue the DMAs here so they land during
        # attention (after the first head pairs' rpb tiles)
        f1sb = sb.tile([P, KM, KC, P], fp8, tag="f1sb", bufs=1)
        f2sb = sb.tile([P, KC, KM, P], fp8, tag="f2sb", bufs=1)
        ausb = sb.tile([P, KC, KR, P], fp8, tag="ausb", bufs=1)

        # ---------------- attention ----------------
        ident_sb = sb.tile([P, P], fp8, tag="ident", bufs=1)
        nc.sync.dma_start(out=ident_sb, in_=id_d[:])

        def evac_head(o_ps, hp, hh):
            # ACT evacuates the accumulators (frees PSUM fast); colsum row
            # scatters across 128 lanes for the reciprocal, gathers back,
            # K=1 PE broadcast, one bf16 2x DVE multiply.
            ou = sb.tile([D + 1, N], bf16, tag="ou", bufs=2)
            for nk in range(2):
                nc.scalar.activation(out=ou[:, NSL[nk]], in_=o_ps[nk],
                                     func=AF.Copy)
            cs = sb.tile([P, NT], bf16, tag="cs", bufs=2)
            nc.sync.dma_start(out=cs, in_=ou[D:D + 1, :])
            rcp = sb.tile([P, NT], bf16, tag="cs", bufs=2)
            with nc.allow_low_precision(reason="bf16 softmax denom"):
                # 8/colsum (the OSC/WS fold rides the reciprocal input: the
                # scatter wrote colsum; scale via tensor_scalar on the way)
                nc.vector.tensor_scalar_mul(rcp, cs, float(WS / OSC))
                nc.vector.reciprocal(rcp, rcp)
            scr = dram.tile([1, N], bf16, tag="evscr", bufs=4)
            nc.sync.dma_start(out=scr, in_=rcp)
            rbs = sb.tile([P, N], bf16, tag="rbs", bufs=2)
            nc.sync.dma_start(out=rbs, in_=scr.to_broadcast(rbs.shape))
            if hh == 0:
                nc.vector.tensor_mul(oT[0:D, hp, :], ou[0:D, :], rbs[0:D, :])
            else:
                ot_tmp = sb.tile([D, N], fp8, tag="ott", bufs=2)
                nc.vector.tensor_mul(ot_tmp, ou[0:D, :], rbs[0:D, :])
                nc.sync.dma_start(out=oT[D:P, hp, :], in_=ot_tmp)

        for hp in range(H // 2):
            qh = [qkT[hh * D:(hh + 1) * D, hp, :] for hh in range(2)]
            kh = [qkT[hh * D:(hh + 1) * D, 6 + hp, :] for hh in range(2)]
            o_ps = [[pp.tile([D + 1, HALF], f32, tag="acc", bufs=4,
                             name=f"o_ps{hh}{nk}") for nk in range(2)]
                    for hh in range(2)]
            if hp == 2:  # let early rpb tiles through the queue first
                nc.sync.dma_start(out=f1sb, in_=f1_d[:])
            if hp == 3:
                nc.sync.dma_start(out=f2sb, in_=f2_d[:])
                nc.sync.dma_start(out=ausb, in_=au_d[:])
            def emit_av(pts, mt):
                for hh in range(2):
                    for nk in range(2):
                        nc.tensor.matmul(
                            o_ps[hh][nk][:, :],
                            lhsT=vaug[:, mt, 2 * hp + hh, :],
                            rhs=pts[hh][:, NSL[nk]].bitcast(bf16),
                            start=(mt == 0), stop=(mt == NT - 1))

            pend = None  # software-pipelined: AV(mt-1) issues while the
            for mt in range(NT):  # DVE/ACT produce pts(mt)
                # hh=1 tiles mostly take the "ACT path": rpb is accumulated
                # into the score PSUM by an fp8 identity matmul, and the
                # +8192/int16 conversion runs on the (otherwise idle)
                # scalar engine; hh=0 (and the rest) use the DVE
                # scalar_tensor_tensor with int8 rpb. This splits the
                # unavoidable fp32-PSUM evacuation across both engines.
                act_path = [False, mt % 4 != 3]
                sts, rps = [], []
                for hh in range(2):
                    h = 2 * hp + hh
                    msl = slice(mt * P, (mt + 1) * P)
                    if act_path[hh]:
                        rp = sb.tile([P, N], fp8, tag="rpf", bufs=4)
                        nc.sync.dma_start(out=rp, in_=rpbf_d[h, msl, :])
                    else:
                        rp = sb.tile([P, N], i8, tag="rpb", bufs=6)
                        nc.sync.dma_start(out=rp, in_=rpb8_d[h, msl, :])
                    rps.append(rp)
                    sts.append(pp.tile([P, N], f32, tag="st", bufs=2, name="st"))
                for nk in range(2):  # hh-pairs issue adjacently -> row-group
                    for hh in range(2):  # concurrency on the PE
                        nc.tensor.matmul(sts[hh][:, NSL[nk]],
                                         lhsT=kh[hh][:, mt * P:(mt + 1) * P],
                                         rhs=qh[hh][:, NSL[nk]],
                                         start=True, stop=not act_path[hh])
                    for hh in range(2):
                        if act_path[hh]:
                            nc.tensor.matmul(sts[hh][:, NSL[nk]],
                                             lhsT=ident_sb,
                                             rhs=rps[hh][:, NSL[nk]],
                                             start=False, stop=True)
                if pend is not None:
                    emit_av(*pend)
                pts = []
                for hh in range(2):
                    pt = sb.tile([P, N], i16, tag="pt", bufs=4)
                    pts.append(pt)
                    # int16(st + 8192 + rpb) -- bit-pattern is bf16 exp(s+r)
                    if act_path[hh]:
                        nc.scalar.activation(out=pt, in_=sts[hh],
                                             func=AF.Copy, bias=IBIAS)
                    else:
                        nc.vector.scalar_tensor_tensor(
                            out=pt, in0=sts[hh], scalar=IBIAS, in1=rps[hh],
                            op0=OP.add, op1=OP.add)
                    if has_mask:
                        nc.vector.tensor_scalar_mul(pt, pt, maskb[:, mt:mt + 1])
                pend = (pts, mt)
            emit_av(*pend)
            evac_head(o_ps[0], hp, 0)
            evac_head(o_ps[1], hp, 1)

        # ---------------- proj + residual 1 + LN2 stats ----------------
        stt2s = pp.tile([1, N], f32, tag="st", bufs=2, name="ln2_s")
        stt2q = pp.tile([1, N], f32, tag="st", bufs=2, name="ln2_q")
        xbs2 = []
        for mt in range(KC):
            wt = sb.tile([P, KC, P], fp8, tag="w6", bufs=3)
            nc.sync.dma_start(out=wt, in_=pw_d[mt])
            for nk in range(2):
                mm = pp.tile([P, HALF], f32, tag="acc", bufs=4)
                dr_mms(mm, wt, oT, NSL[nk])
                if pb_nz:
                    nc.vector.tensor_scalar_add(mm, mm, vec(V_PB)[:, mt:mt + 1])
                nc.vector.scalar_tensor_tensor(
                    out=xres[:, mt, NSL[nk]], in0=mm,
                    scalar=vec(V_G1)[:, mt:mt + 1],
                    in1=xres[:, mt, NSL[nk]], op0=OP.mult, op1=OP.add)
            if mt >= 2:  # stats lag the proj chunks so the PE never waits
                ln_stats_ch(stt2s, stt2q, mt - 2, xbs2, xres)
        for mt in (KC - 2, KC - 1):
            ln_stats_ch(stt2s, stt2q, mt, xbs2, xres)
        ln_finish(stt2s, stt2q, xbs2, xn2T, V_L2G, V_L2B, ln2_triv)

        # ---------------- adapter down (relu via ACT, 1/32 descale) -----
        for mt in range(KR):
            wt = sb.tile([P, KC, P], fp8, tag="w6", bufs=3)
            nc.sync.dma_start(out=wt, in_=ad_d[mt])
            for nk in range(2):
                mm = pp.tile([P, HALF], f32, tag="acc", bufs=4)
                dr_mms(mm, wt, xn2T, NSL[nk])
                nc.scalar.activation(
                    out=a1T[:, mt, NSL[nk]], in_=mm, func=AF.Relu,
                    scale=1.0 / WS,
                    bias=(adb[:, mt:mt + 1] if adb_nz else zero_col))

        # ---------------- MLP (fp8 DR, resident weights) ----------------
        for nk in range(2):
            h1 = sb.tile([P, KM, HALF], fp8, tag="h1", bufs=1)
            for mt in range(KM):
                mm = pp.tile([P, HALF], f32, tag="acc", bufs=4)
                dr_mms(mm, f1sb[:, mt], xn2T, NSL[nk])
                nc.scalar.activation(
                    out=h1[:, mt], in_=mm, func=AF.Gelu, scale=1.0 / WS,
                    bias=(f1b[:, mt:mt + 1] if f1b_nz else zero_col))
            for mt in range(KC):
                mm = pp.tile([P, HALF], f32, tag="acc", bufs=4)
                for j in range(KM // 2):
                    nc.tensor.matmul(mm, lhsT=f2sb[:, mt, 2 * j:2 * j + 2, :],
                                     rhs=h1[:, 2 * j:2 * j + 2, :],
                                     start=(j == 0), stop=False,
                                     perf_mode=PM.DoubleRow)
                nc.tensor.matmul(mm, lhsT=ausb[:, mt],
                                 rhs=a1T[:, 0:KR, NSL[nk]],
                                 start=False, stop=True,
                                 perf_mode=PM.DoubleRow)
                if fb_nz:
                    nc.vector.tensor_scalar_add(mm, mm, vec(V_FB)[:, mt:mt + 1])
                nc.vector.scalar_tensor_tensor(
                    out=xres[:, mt, NSL[nk]], in0=mm,
                    scalar=vec(V_G2)[:, mt:mt + 1],
                    in1=xres[:, mt, NSL[nk]], op0=OP.mult, op1=OP.add)
                nc.sync.dma_start(out=out_d[:, mt, NSL[nk]],
                                  in_=xres[:, mt, NSL[nk]])

    if not nc.is_finalized():
        nc.finalize()
    return nc


def _pack_w6(wT, km, kk):
    """[K, M] (K=contraction, M=out) -> [M//128, 128, K//128, 128] fp8 tiles
    laid out so each DMA partition read is contiguous."""
    K, M = wT.shape
    assert K == kk * P and M == km * P
    a = wT.reshape(kk, P, km, P)          # [ks, p, mt, col]
    return np.ascontiguousarray(a.transpose(2, 1, 0, 3)).astype(FP8)


def _pack_res(wT, km, kk):
    """[K, M] -> [128, M//128, K//128, 128] partition-major for one-shot
    SBUF-resident DMA (contiguous per-partition reads)."""
    K, M = wT.shape
    assert K == kk * P and M == km * P
    a = wT.reshape(kk, P, km, P)          # [ks, p, mt, col]
    return np.ascontiguousarray(a.transpose(1, 2, 0, 3)).astype(FP8)


def _stripe(v, k):
    """[k*128] -> [128, k] with v[ks*128+p] at [p, ks]."""
    return np.ascontiguousarray(v.reshape(k, P).T).astype(np.float32)


def prepare_core_inputs(x, mask, rpb, ln1_g, ln1_b, qkv_w, q_bias, v_bias,
                        proj_w, proj_b, gamma1, ln2_g, ln2_b, fc1_w, fc1_b,
                        fc2_w, fc2_b, ad_dw, ad_db, ad_uw, ad_ub, gamma2):
    """Host-side layout prep. Returns (per_core_maps, flags)."""
    f32 = np.float32

    qkv_w = np.asarray(qkv_w, f32)
    wq = qkv_w[:C] * ALPHA            # q-side carries alpha (incl. D^-.5*K2)
    wk = qkv_w[C:2 * C] * BETA
    wv = qkv_w[2 * C:] * WS
    wqkT = np.concatenate([wq, wk], 0).T          # [C, 1536]
    wqk = _pack_w6(wqkT, 12, KC)
    wv_packed = np.ascontiguousarray(
        wv.T.reshape(KC, P, C).transpose(1, 0, 2)).astype(FP8)

    projw = _pack_w6(np.asarray(proj_w, f32).T * WS, KC, KC)
    fc1w = _pack_res(np.asarray(fc1_w, f32).T * WS, KM, KC)
    fc2w = _pack_res(np.asarray(fc2_w, f32).T * WS, KC, KM)
    adw = _pack_w6(np.asarray(ad_dw, f32).T * WS, KR, KC)
    auw = _pack_res(np.asarray(ad_uw, f32).T * WS, KC, KR)

    # rpb as exponent steps K2*rpb in two formats: int8 for the DVE
    # schraudolph path, fp8 for the identity-matmul (ACT) path
    rpbk = K2 * np.ascontiguousarray(np.asarray(rpb, f32).transpose(0, 2, 1))
    rpb8 = np.clip(np.round(rpbk), -127, 127).astype(np.int8)
    rpbf = rpbk.astype(FP8)
    ident = np.eye(P).astype(FP8)

    q_bias_s = np.asarray(q_bias, f32) * ALPHA
    fb = (np.asarray(fc2_b, f32) + np.asarray(ad_ub, f32)) * WS

    vecs = np.stack([
        _stripe(np.asarray(gamma1, f32) / (OSC * WS), KC),
        _stripe(np.asarray(gamma2, f32) / WS, KC),
        _stripe(q_bias_s, KC),
        _stripe(np.asarray(proj_b, f32) * OSC * WS, KC),
        _stripe(fb, KC),
        _stripe(np.asarray(ln1_g, f32), KC),
        _stripe(np.asarray(ln1_b, f32), KC),
        _stripe(np.asarray(ln2_g, f32), KC),
        _stripe(np.asarray(ln2_b, f32), KC),
    ], 0)  # [NVEC, 128, KC]

    f1b = _stripe(np.asarray(fc1_b, f32), KM)
    adb = _stripe(np.asarray(ad_db, f32), KR)
    vb = (np.asarray(v_bias, f32) * WS).reshape(1, C).astype(f32)

    mask = np.asarray(mask)
    has_mask = not bool(mask.all())

    flags = (
        has_mask,
        bool(np.any(q_bias_s)),
        bool(np.any(v_bias)),
        bool(np.any(proj_b)),
        bool(np.any(fc1_b)),
        bool(np.any(fb)),
        bool(np.any(ad_db)),
        bool(np.all(ln1_g == 1.0) and not np.any(ln1_b)),
        bool(np.all(ln2_g == 1.0) and not np.any(ln2_b)),
    )

    shared = {
        "rpb8": rpb8, "rpbf": rpbf, "ident": ident,
        "wqk": wqk, "wv": wv_packed, "projw": projw,
        "fc1w": fc1w, "fc2w": fc2w, "adw": adw, "auw": auw,
        "vecs": vecs, "fc1b": f1b, "adb": adb, "vbias": vb,
    }

    x = np.asarray(x, f32)
    per_core = []
    for b in range(B):
        xT = np.ascontiguousarray(
            x[b].T.reshape(KC, P, N).transpose(1, 0, 2)).astype(f32)
        if has_mask:
            mb = np.where(mask[b], 1.0, 0.0).astype(f32)    # [N] over keys m
            mb = np.ascontiguousarray(mb.reshape(NT, P).T)  # [128, NT]
        else:
            mb = np.zeros((P, NT), f32)
        m = dict(shared)
        m["xT"] = xT
        m["maskb"] = mb
        per_core.append(m)
    return per_core, flags


def _ensure_ntff_hook():
    """The agent image lacks ``antenv.axon_hooks``; provide it and register
    the ctypes NTFF profile hook so trace=True works under axon."""
    import types
    try:
        from antenv.axon_hooks import get_axon_ntff_profile_hook  # noqa: F401
        return
    except ImportError:
        pass
    import antenv
    mod = types.ModuleType("antenv.axon_hooks")
    _h = {"hook": None}
    mod.set_axon_ntff_profile_hook = lambda h: _h.__setitem__("hook", h)
    mod.get_axon_ntff_profile_hook = lambda: _h["hook"]
    sys.modules["antenv.axon_hooks"] = mod
    antenv.axon_hooks = mod
    try:
        from trn_agent_boot.trn_boot import _ntff_profile_via_ctypes
        hook = _ntff_profile_via_ctypes("/opt/axon/libaxon_pjrt.so")
        if hook is not None:
            mod.set_axon_ntff_profile_hook(hook)
    except Exception as e:  # profiling degrades, run still works
        print("ntff hook setup failed:", e)


def run_sharded(inputs, trace=False, trace_kwargs=None):
    """Compile (cached) + run on 8 cores. Returns (out [B,N,C] f32, results)."""
    from concourse.bass_utils import run_bass_kernel_spmd
    if trace:
        _ensure_ntff_hook()

    per_core, flags = prepare_core_inputs(**inputs)
    if flags not in _PROG_CACHE:
        _PROG_CACHE[flags] = _build(flags)
    nc = _PROG_CACHE[flags]

    kw = {}
    if trace:
        kw["trace"] = True
        kw["trace_cores"] = [0]
        if trace_kwargs:
            kw["trace_kwargs"] = trace_kwargs
    res = run_bass_kernel_spmd(nc, per_core, core_ids=list(range(B)), **kw)

    out = np.empty((B, N, C), np.float32)
    for b in range(B):
        oT = res.results[b]["outT"]          # [128, KC, N]
        out[b] = oT.transpose(1, 0, 2).reshape(C, N).T
    return out, res


def kernel(**inputs):
    out, _ = run_sharded(inputs, trace=False)
    return out


# revision 42
# speedup vs baseline: 1.5407x; 1.0084x over previous
"""Trainium2 Bass kernel for nn_Block_89361089561275 (dense transformer block).

Sharding: data-parallel over batch B=8 -> one batch element per NeuronCore.
No collectives. Feature-transposed layout (features on SBUF partitions,
tokens on the free dim) throughout.

Key speed tricks over the bf16 baseline:
  * All big matmuls (qkv, v, proj, fc1, fc2, adapters) run as fp8e4m3
    DoubleRow matmuls: lhsT [128,2,M] + rhs [128,2,N] contract 256 K per
    instruction (~2x tensor-engine throughput). Weights are host-scaled
    (x32-ish) into fp8's sweet spot; inverse scales fold into downstream
    per-feature vectors / activation `scale=` operands.
  * Softmax exp is a Schraudolph bit-trick fused into the mandatory
    PSUM-evacuation op: the q/k weight scales carry alpha*beta =
    128*log2(e)*D^-0.5 so the score PSUM is already K2*s; one DVE
    scalar_tensor_tensor computes int16(round(st + 8192 + rpb_i8)) whose
    bf16 bit-pattern IS exp(s+r) up to a global power-of-2 scale that
    cancels in softmax. rpb ships as int8 = round(K2*rpb) (+8192 via the
    freed scalar slot). ScalarE does no exp at all.
  * The softmax denominator falls out of the AV matmul via a ones-column
    appended to v (M=65 accumulators, row 64 = colsum).
  * LN mean/var still come from ones-matmuls, but the finish chain runs on
    [1,N] rows and broadcasts rstd/-mean*rstd back across partitions with a
    K=1 ones matmul (no DRAM bounce round-trips).
  * fc1/fc2 weights are SBUF-resident (loaded once, streamed during
    attention), gelu/relu descale by 1/32 via the activation scale operand.
"""

import sys

for _p in ("/opt/trn_rl_repo",):
    if _p not in sys.path:
        sys.path.insert(0, _p)

import numpy as np
import ml_dtypes

BF16 = ml_dtypes.bfloat16
FP8 = ml_dtypes.float8_e4m3

B, N, C, H = 8, 1024, 768, 12
D = C // H            # 64
MLP = 4 * C           # 3072
RED = C // 3          # 256
EPS = 1e-5
P = 128
KC = C // P           # 6   c-chunks
KM = MLP // P         # 24  mlp-chunks
KR = RED // P         # 2   adapter chunks
NT = N // P           # 8   token tiles
HALF = 512
NSL = (slice(0, HALF), slice(HALF, N))

K2 = 128.0 / float(np.log(2.0))   # exponent-steps-per-unit-score for bf16
K8 = 8.0 / float(np.log(2.0))     # same for fp8e4m3 (3 mantissa bits)
IBIAS = 64.0                      # int8 exponent offset (global 2^-7 scale)
SCALE = D ** -0.5
AB = K2 * SCALE                   # alpha*beta for the q/k weight pair
ALPHA = float(np.sqrt(AB))        # q-side host scale
BETA = float(np.sqrt(AB))         # k-side host scale
WS = 32.0                         # generic fp8 weight scale
OSC = 256.0                       # oT = OSC * o_true (8.0 folded in evac)

_PROG_CACHE: dict = {}

# indices into the packed [n, 128, KC] per-feature vector table
V_G1, V_G2, V_QB, V_PB, V_FB, V_L1G, V_L1B, V_L2G, V_L2B = range(9)
NVEC = 9


def _build(flags):
    """Build the single-core Bass program. flags is a tuple of bools:
    (has_mask, qb_nz, vb_nz, pb_nz, f1b_nz, fb_nz, adb_nz,
     ln1_triv, ln2_triv)
    """
    (has_mask, qb_nz, vb_nz, pb_nz, f1b_nz, fb_nz, adb_nz,
     ln1_triv, ln2_triv) = flags

    import concourse.tile as tile
    from concourse import bacc, mybir
    from contextlib import ExitStack

    f32 = mybir.dt.float32
    bf16 = mybir.dt.bfloat16
    fp8 = mybir.dt.float8e4
    i16 = mybir.dt.int16
    i8 = mybir.dt.int8
    AF = mybir.ActivationFunctionType
    OP = mybir.AluOpType
    PM = mybir.MatmulPerfMode

    nc = bacc.Bacc("TRN2")

    # ---- external I/O ----
    x_d = nc.declare_dram_parameter("xT", [P, KC, N], f32, isOutput=False)
    rpb8_d = nc.declare_dram_parameter("rpb8", [H, N, N], i8, isOutput=False)
    rpbf_d = nc.declare_dram_parameter("rpbf", [H, N, N], fp8, isOutput=False)
    id_d = nc.declare_dram_parameter("ident", [P, P], fp8, isOutput=False)
    wqk_d = nc.declare_dram_parameter("wqk", [12, P, KC, P], fp8, isOutput=False)
    wv_d = nc.declare_dram_parameter("wv", [P, KC, C], fp8, isOutput=False)
    pw_d = nc.declare_dram_parameter("projw", [KC, P, KC, P], fp8, isOutput=False)
    f1_d = nc.declare_dram_parameter("fc1w", [P, KM, KC, P], fp8, isOutput=False)
    f2_d = nc.declare_dram_parameter("fc2w", [P, KC, KM, P], fp8, isOutput=False)
    ad_d = nc.declare_dram_parameter("adw", [KR, P, KC, P], fp8, isOutput=False)
    au_d = nc.declare_dram_parameter("auw", [P, KC, KR, P], fp8, isOutput=False)
    vec_d = nc.declare_dram_parameter("vecs", [NVEC, P, KC], f32, isOutput=False)
    f1b_d = nc.declare_dram_parameter("fc1b", [P, KM], f32, isOutput=False)
    adb_d = nc.declare_dram_parameter("adb", [P, KR], f32, isOutput=False)
    vb_d = nc.declare_dram_parameter("vbias", [1, C], f32, isOutput=False)
    mb_d = nc.declare_dram_parameter("maskb", [P, NT], f32, isOutput=False)
    out_d = nc.declare_dram_parameter("outT", [P, KC, N], f32, isOutput=True)

    with tile.TileContext(nc) as tc, ExitStack() as ctx:
        sb = ctx.enter_context(tc.tile_pool(name="sb", bufs=1))
        pp = ctx.enter_context(tc.tile_pool(name="pp", bufs=1, space="PSUM"))
        dram = ctx.enter_context(tc.tile_pool(name="dram", bufs=2, space="DRAM"))

        # ---- persistent tiles ----
        VP = 80  # vaug row pitch: D+1 padded so the DoubleRow mt-pair
        #          stride (H*VP bytes) stays 16B-aligned
        xres = sb.tile([P, KC, N], f32, tag="xres", bufs=1)
        qkT = sb.tile([P, 12, N], bf16, tag="qkT", bufs=1)
        vaug = sb.tile([P, NT, H, VP], fp8, tag="vaug", bufs=1)
        oT = sb.tile([P, KC, N], fp8, tag="oT", bufs=1)
        xnT = sb.tile([P, KC, N], fp8, tag="xnT", bufs=1)
        xn2T = sb.tile([P, KC, N], fp8, tag="xn2T", bufs=1)
        a1T = sb.tile([P, KR, N], fp8, tag="a1", bufs=1)

        for ch in range(KC):  # per-chunk loads so LN1 stats start early
            nc.sync.dma_start(out=xres[:, ch], in_=x_d[:, ch])

        ones_bf = sb.tile([P, 1], bf16, tag="ones", bufs=1)
        nc.vector.memset(ones_bf, 1.0)
        ones_row = sb.tile([1, P], bf16, tag="onesr", bufs=1)
        nc.vector.memset(ones_row, 1.0)
        nc.vector.memset(vaug[:, :, :, D:D + 1], 1.0)

        zero_col = sb.tile([P, 1], f32, tag="zcol", bufs=1)
        nc.vector.memset(zero_col, 0.0)
        eps_col = sb.tile([P, 1], f32, tag="ecol", bufs=1)
        nc.vector.memset(eps_col, float(EPS))

        vecs = sb.tile([P, NVEC, KC], f32, tag="vecs", bufs=1)
        nc.sync.dma_start(out=vecs, in_=vec_d[:].rearrange("v p k -> p v k"))

        def vec(i):
            return vecs[:, i]  # [128, KC]

        if f1b_nz:
            f1b = sb.tile([P, KM], f32, tag="f1b", bufs=1)
            nc.sync.dma_start(out=f1b, in_=f1b_d[:])
        if adb_nz:
            adb = sb.tile([P, KR], f32, tag="adb", bufs=1)
            nc.sync.dma_start(out=adb, in_=adb_d[:])
        if vb_nz:
            vb1 = sb.tile([1, C], f32, tag="vb1", bufs=1)
            nc.sync.dma_start(out=vb1, in_=vb_d[:])
            vb_b = sb.tile([P, C], f32, tag="vb_b", bufs=1)
            scratch = dram.tile([1, C], f32, tag="bscratch", bufs=2)
            nc.sync.dma_start(out=scratch, in_=vb1)
            nc.sync.dma_start(out=vb_b, in_=scratch.to_broadcast(vb_b.shape))
        if has_mask:
            maskb = sb.tile([P, NT], f32, tag="maskb", bufs=1)
            nc.sync.dma_start(out=maskb, in_=mb_d[:])

        # ---------------- layernorm (feature-transposed) ----------------
        # stats via ones-matmuls into two base-0 psum rows (sum, sum-sq) --
        # engine ops need all operands on the same partitions.
        def ln_stats_ch(stt_s, stt_q, ch, xbs, src):
            xb = sb.tile([P, N], bf16, tag="xb", bufs=KC)
            xbs.append(xb)
            nc.vector.tensor_copy(out=xb, in_=src[:, ch])
            x2 = sb.tile([P, N], bf16, tag="x2", bufs=2)
            nc.vector.tensor_mul(x2, xb, xb)
            for nk in range(2):
                nc.tensor.matmul(stt_s[0:1, NSL[nk]], lhsT=ones_bf,
                                 rhs=xb[:, NSL[nk]],
                                 start=(ch == 0), stop=(ch == KC - 1))
                nc.tensor.matmul(stt_q[0:1, NSL[nk]], lhsT=ones_bf,
                                 rhs=x2[:, NSL[nk]],
                                 start=(ch == 0), stop=(ch == KC - 1))

        def ln_finish(stt_s, stt_q, xbs, dst, g_i, b_i, triv):
            # evacuate sum rows via ACT (with the 1/C fold), scatter the
            # per-token stats across 128 lanes via DMA so sqrt/reciprocal
            # run wide, gather back, K=1 PE broadcast.
            mq = sb.tile([1, 2, N], f32, tag="lnrow", bufs=2)
            nc.scalar.activation(out=mq[0:1, 0], in_=stt_s[0:1, :],
                                 func=AF.Copy, scale=1.0 / C)
            nc.scalar.activation(out=mq[0:1, 1], in_=stt_q[0:1, :],
                                 func=AF.Copy, scale=1.0 / C)
            s128 = sb.tile([P, 2, NT], f32, tag="s128", bufs=2)
            nc.sync.dma_start(out=s128[:, 0], in_=mq[0:1, 0])
            nc.sync.dma_start(out=s128[:, 1], in_=mq[0:1, 1])
            t128 = sb.tile([P, NT], f32, tag="t128", bufs=4)
            nc.vector.tensor_mul(t128, s128[:, 0], s128[:, 0])
            v128 = sb.tile([P, NT], f32, tag="t128", bufs=4)
            nc.vector.tensor_sub(v128, s128[:, 1], t128)
            sd = sb.tile([P, NT], f32, tag="t128", bufs=4)
            nc.scalar.activation(out=sd, in_=v128, func=AF.Sqrt,
                                 bias=eps_col)
            r2 = sb.tile([P, 2, NT], bf16, tag="r128", bufs=2)
            with nc.allow_low_precision(reason="bf16 rstd broadcast"):
                nc.vector.reciprocal(r2[:, 0], sd)             # rstd
            nc.vector.scalar_tensor_tensor(out=r2[:, 1], in0=s128[:, 0],
                                           scalar=-1.0, in1=r2[:, 0],
                                           op0=OP.mult, op1=OP.mult)
            # gather to one SBUF row, broadcast across partitions with K=1
            # matmuls (PSUM is free during the LN chains), evacuate via ACT
            rn = sb.tile([1, 2, N], bf16, tag="lnrn", bufs=1)
            nc.sync.dma_start(out=rn[0:1, 0], in_=r2[:, 0])
            nc.sync.dma_start(out=rn[0:1, 1], in_=r2[:, 1])
            ab = sb.tile([P, 2, N], bf16, tag="lnab", bufs=1)
            for i in range(2):
                rb = pp.tile([P, N], f32, tag="st", bufs=2, name="lnrb")
                for nk in range(2):
                    nc.tensor.matmul(rb[:, NSL[nk]], lhsT=ones_row,
                                     rhs=rn[0:1, i, NSL[nk]],
                                     start=True, stop=True)
                nc.scalar.activation(out=ab[:, i], in_=rb, func=AF.Copy)
            # apply in nk-halves, nk0 for every chunk first, so consumers
            # of the first half can start ~5us earlier
            for nk in range(2):
                for ch in range(KC):
                    t1 = sb.tile([P, HALF], bf16, tag="x2h", bufs=2)
                    nc.vector.tensor_mul(t1, xbs[ch][:, NSL[nk]],
                                         ab[:, 0, NSL[nk]])
                    if triv:
                        nc.vector.tensor_add(dst[:, ch, NSL[nk]], t1,
                                             ab[:, 1, NSL[nk]])
                    else:
                        nc.vector.tensor_add(t1, t1, ab[:, 1, NSL[nk]])
                        nc.vector.tensor_scalar(
                            out=dst[:, ch, NSL[nk]], in0=t1,
                            scalar1=vec(g_i)[:, ch:ch + 1],
                            scalar2=vec(b_i)[:, ch:ch + 1],
                            op0=OP.mult, op1=OP.add)

        # ---------------- LN1 -> xnT (fp8) ----------------
        stt1s = pp.tile([1, N], f32, tag="st", bufs=2, name="ln_s")
        stt1q = pp.tile([1, N], f32, tag="st", bufs=2, name="ln_q")
        xbs1 = []
        for ch in range(KC):
            ln_stats_ch(stt1s, stt1q, ch, xbs1, xres)
        ln_finish(stt1s, stt1q, xbs1, xnT, V_L1G, V_L1B, ln1_triv)

        # ---------------- QKV (fp8 DoubleRow) ----------------
        def dr_mms(acc, wt, rhs_t, nsl, kd=KC):
            for j in range(kd // 2):
                nc.tensor.matmul(acc, lhsT=wt[:, 2 * j:2 * j + 2, :],
                                 rhs=rhs_t[:, 2 * j:2 * j + 2, nsl],
                                 start=(j == 0), stop=(j == kd // 2 - 1),
                                 perf_mode=PM.DoubleRow)

        # per q/k block pair, nk0 for both blocks first -- the LN apply
        # produces the nk0 halves of xnT first
        for bq in range(6):
            wts = []
            for blk in (bq, 6 + bq):
                wt = sb.tile([P, KC, P], fp8, tag="w6", bufs=4)
                nc.sync.dma_start(out=wt, in_=wqk_d[blk])
                wts.append(wt)
            for nk in range(2):
                for blk, wt in zip((bq, 6 + bq), wts):
                    mm = pp.tile([P, HALF], f32, tag="acc", bufs=4)
                    dr_mms(mm, wt, xnT, NSL[nk])
                    dst = qkT[:, blk, NSL[nk]]
                    if blk < 6 and qb_nz:
                        nc.vector.tensor_scalar_add(dst, mm,
                                                    vec(V_QB)[:, blk:blk + 1])
                    else:
                        nc.scalar.copy(out=dst, in_=mm)

        wv_sb = sb.tile([P, KC, C], fp8, tag="h1", bufs=1)
        nc.sync.dma_start(out=wv_sb, in_=wv_d[:])
        for t in range(NT):
            for off, cw in ((0, HALF), (HALF, C - HALF)):
                mm = pp.tile([P, HALF], f32, tag="acc", bufs=4)
                for j in range(KC // 2):
                    nc.tensor.matmul(
                        mm[:, :cw], lhsT=xnT[:, 2 * j:2 * j + 2, t * P:(t + 1) * P],
                        rhs=wv_sb[:, 2 * j:2 * j + 2, off:off + cw],
                        start=(j == 0), stop=(j == KC // 2 - 1),
                        perf_mode=PM.DoubleRow)
                dst = vaug[:, t, off // D:(off + cw) // D, :D]
                src = mm[:, :cw].rearrange("p (h d) -> p h d", d=D)
                if vb_nz:
                    nc.vector.tensor_add(
                        dst, src,
                        vb_b[:, off:off + cw].rearrange("p (h d) -> p h d", d=D))
                else:
                    nc.vector.tensor_copy(out=dst, in_=src)

        # resident MLP weights: queue the DMAs here so they land during
        # attention (after the first head pairs' rpb tiles)
        f1sb = sb.tile([P, KM, KC, P], fp8, tag="f1sb", bufs=1)
        f2sb = sb.tile([P, KC, KM, P], fp8, tag="f2sb", bufs=1)
        ausb = sb.tile([P, KC, KR, P], fp8, tag="ausb", bufs=1)

        # ---------------- attention ----------------
        ident_sb = sb.tile([P, P], fp8, tag="ident", bufs=1)
        nc.sync.dma_start(out=ident_sb, in_=id_d[:])

        def evac_head(o_ps, hp, hh):
            # ACT evacuates the accumulators (frees PSUM fast); colsum row
            # scatters across 128 lanes for the reciprocal, gathers back,
            # K=1 PE broadcast, one bf16 2x DVE multiply.
            ou = sb.tile([D + 1, N], bf16, tag="ou", bufs=2)
            for nk in range(2):
                nc.scalar.activation(out=ou[:, NSL[nk]], in_=o_ps[nk],
                                     func=AF.Copy)
            cs = sb.tile([P, NT], bf16, tag="cs", bufs=2)
            nc.sync.dma_start(out=cs, in_=ou[D:D + 1, :])
            rcp = sb.tile([P, NT], bf16, tag="cs", bufs=2)
            with nc.allow_low_precision(reason="bf16 softmax denom"):
                # 8/colsum (the OSC/WS fold rides the reciprocal input: the
                # scatter wrote colsum; scale via tensor_scalar on the way)
                nc.vector.tensor_scalar_mul(rcp, cs, float(WS / OSC))
                nc.vector.reciprocal(rcp, rcp)
            scr = dram.tile([1, N], bf16, tag="evscr", bufs=4)
            nc.sync.dma_start(out=scr, in_=rcp)
            rbs = sb.tile([P, N], bf16, tag="rbs", bufs=2)
            nc.sync.dma_start(out=rbs, in_=scr.to_broadcast(rbs.shape))
            if hh == 0:
                nc.vector.tensor_mul(oT[0:D, hp, :], ou[0:D, :], rbs[0:D, :])
            else:
                ot_tmp = sb.tile([D, N], fp8, tag="ott", bufs=2)
                nc.vector.tensor_mul(ot_tmp, ou[0:D, :], rbs[0:D, :])
                nc.sync.dma_start(out=oT[D:P, hp, :], in_=ot_tmp)

        for hp in range(H // 2):
            qh = [qkT[hh * D:(hh + 1) * D, hp, :] for hh in range(2)]
            kh = [qkT[hh * D:(hh + 1) * D, 6 + hp, :] for hh in range(2)]
            o_ps = [[pp.tile([D + 1, HALF], f32, tag="acc", bufs=4,
                             name=f"o_ps{hh}{nk}") for nk in range(2)]
                    for hh in range(2)]
            if hp == 2:  # let early rpb tiles through the queue first
                nc.sync.dma_start(out=f1sb, in_=f1_d[:])
            if hp == 3:
                nc.sync.dma_start(out=f2sb, in_=f2_d[:])
                nc.sync.dma_start(out=ausb, in_=au_d[:])
            if hp == 4:  # prefetch proj weights so proj starts instantly
                pwts = []
                for mt in range(KC):
                    wt = sb.tile([P, KC, P], fp8, tag="wp", bufs=KC)
                    nc.sync.dma_start(out=wt, in_=pw_d[mt])
                    pwts.append(wt)
            def emit_av(ptp, pi):
                # fp8 DoubleRow: one MM contracts an mt-pair (256 tokens)
                for hh in range(2):
                    for nk in range(2):
                        nc.tensor.matmul(
                            o_ps[hh][nk][:, :],
                            lhsT=vaug[:, 2 * pi:2 * pi + 2, 2 * hp + hh, :D + 1],
                            rhs=ptp[hh][:, :, NSL[nk]].bitcast(fp8),
                            start=(pi == 0), stop=(pi == NT // 2 - 1),
                            perf_mode=PM.DoubleRow)

            pend = None  # software-pipelined: AV(pair-1) issues while the
            ptp = None
            for mt in range(NT):  # DVE/ACT produce pts(mt)
                # hh=1 tiles mostly take the "ACT path": rpb is accumulated
                # into the score PSUM by an fp8 identity matmul, and the
                # +8192/int16 conversion runs on the (otherwise idle)
                # scalar engine; hh=0 (and the rest) use the DVE
                # scalar_tensor_tensor with int8 rpb. This splits the
                # unavoidable fp32-PSUM evacuation across both engines.
                act_path = [False, mt % 4 != 3]
                sts, rps = [], []
                for hh in range(2):
                    h = 2 * hp + hh
                    msl = slice(mt * P, (mt + 1) * P)
                    if act_path[hh]:
                        rp = sb.tile([P, N], fp8, tag="rpf", bufs=4)
                        nc.sync.dma_start(out=rp, in_=rpbf_d[h, msl, :])
                    else:
                        rp = sb.tile([P, N], i8, tag="rpb", bufs=6)
                        nc.sync.dma_start(out=rp, in_=rpb8_d[h, msl, :])
                    rps.append(rp)
                    sts.append(pp.tile([P, N], f32, tag="st", bufs=2, name="st"))
                for nk in range(2):  # hh-pairs issue adjacently -> row-group
                    for hh in range(2):  # concurrency on the PE
                        nc.tensor.matmul(sts[hh][:, NSL[nk]],
                                         lhsT=kh[hh][:, mt * P:(mt + 1) * P],
                                         rhs=qh[hh][:, NSL[nk]],
                                         start=True, stop=not act_path[hh])
                    for hh in range(2):
                        if act_path[hh]:
                            nc.tensor.matmul(sts[hh][:, NSL[nk]],
                                             lhsT=ident_sb,
                                             rhs=rps[hh][:, NSL[nk]],
                                             start=False, stop=True)
                if mt % 2 == 0:
                    if pend is not None:
                        emit_av(*pend)
                    ptp = [sb.tile([P, 2, N], i8, tag="pt", bufs=4,
                                   name=f"ptp{_hh}")
                           for _hh in range(2)]
                for hh in range(2):
                    dst8 = ptp[hh][:, mt % 2]
                    # int8(st/16 + 64 + K8*rpb): bit-pattern is the fp8e4m3
                    # encoding of exp(s+r) up to a global power-of-2 scale
                    if act_path[hh]:
                        nc.scalar.activation(out=dst8, in_=sts[hh],
                                             func=AF.Copy, bias=IBIAS,
                                             scale=1.0 / 16.0)
                    else:
                        nc.vector.scalar_tensor_tensor(
                            out=dst8, in0=sts[hh], scalar=1.0 / 16.0,
                            in1=rps[hh], op0=OP.mult, op1=OP.add)
                    if has_mask:
                        nc.vector.tensor_scalar_mul(dst8, dst8,
                                                    maskb[:, mt:mt + 1])
                if mt % 2 == 1:
                    pend = (ptp, mt // 2)
            emit_av(*pend)
            evac_head(o_ps[0], hp, 0)
            evac_head(o_ps[1], hp, 1)

        # ---------------- proj + residual 1 + LN2 stats ----------------
        stt2s = pp.tile([1, N], f32, tag="st", bufs=2, name="ln2_s")
        stt2q = pp.tile([1, N], f32, tag="st", bufs=2, name="ln2_q")
        xbs2 = []
        for mt in range(KC):
            for nk in range(2):
                mm = pp.tile([P, HALF], f32, tag="acc", bufs=4)
                dr_mms(mm, pwts[mt], oT, NSL[nk])
                if pb_nz:
                    nc.vector.tensor_scalar_add(mm, mm, vec(V_PB)[:, mt:mt + 1])
                nc.vector.scalar_tensor_tensor(
                    out=xres[:, mt, NSL[nk]], in0=mm,
                    scalar=vec(V_G1)[:, mt:mt + 1],
                    in1=xres[:, mt, NSL[nk]], op0=OP.mult, op1=OP.add)
            if mt >= 1:  # stats lag the proj chunks so the PE never waits
                ln_stats_ch(stt2s, stt2q, mt - 1, xbs2, xres)
        ln_stats_ch(stt2s, stt2q, KC - 1, xbs2, xres)
        ln_finish(stt2s, stt2q, xbs2, xn2T, V_L2G, V_L2B, ln2_triv)

        # ---------------- adapter down (relu via ACT, 1/32 descale) -----
        for mt in range(KR):
            wt = sb.tile([P, KC, P], fp8, tag="w6", bufs=4)
            nc.sync.dma_start(out=wt, in_=ad_d[mt])
            for nk in range(2):
                mm = pp.tile([P, HALF], f32, tag="acc", bufs=4)
                dr_mms(mm, wt, xn2T, NSL[nk])
                nc.scalar.activation(
                    out=a1T[:, mt, NSL[nk]], in_=mm, func=AF.Relu,
                    scale=1.0 / WS,
                    bias=(adb[:, mt:mt + 1] if adb_nz else zero_col))

        # ---------------- MLP (fp8 DR, resident weights) ----------------
        for nk in range(2):
            h1 = sb.tile([P, KM, HALF], fp8, tag="h1", bufs=1)
            for mt in range(KM):
                mm = pp.tile([P, HALF], f32, tag="acc", bufs=4)
                dr_mms(mm, f1sb[:, mt], xn2T, NSL[nk])
                nc.scalar.activation(
                    out=h1[:, mt], in_=mm, func=AF.Gelu, scale=1.0 / WS,
                    bias=(f1b[:, mt:mt + 1] if f1b_nz else zero_col))
            for mt in range(KC):
                mm = pp.tile([P, HALF], f32, tag="acc", bufs=4)
                for j in range(KM // 2):
                    nc.tensor.matmul(mm, lhsT=f2sb[:, mt, 2 * j:2 * j + 2, :],
                                     rhs=h1[:, 2 * j:2 * j + 2, :],
                                     start=(j == 0), stop=False,
                                     perf_mode=PM.DoubleRow)
                nc.tensor.matmul(mm, lhsT=ausb[:, mt],
                                 rhs=a1T[:, 0:KR, NSL[nk]],
                                 start=False, stop=True,
                                 perf_mode=PM.DoubleRow)
                if fb_nz:
                    nc.vector.tensor_scalar_add(mm, mm, vec(V_FB)[:, mt:mt + 1])
                nc.vector.scalar_tensor_tensor(
                    out=xres[:, mt, NSL[nk]], in0=mm,
                    scalar=vec(V_G2)[:, mt:mt + 1],
                    in1=xres[:, mt, NSL[nk]], op0=OP.mult, op1=OP.add)
                nc.sync.dma_start(out=out_d[:, mt, NSL[nk]],
                                  in_=xres[:, mt, NSL[nk]])

    if not nc.is_finalized():
        nc.finalize()
    return nc


def _pack_w6(wT, km, kk):
    """[K, M] (K=contraction, M=out) -> [M//128, 128, K//128, 128] fp8 tiles
    laid out so each DMA partition read is contiguous."""
    K, M = wT.shape
    assert K == kk * P and M == km * P
    a = wT.reshape(kk, P, km, P)          # [ks, p, mt, col]
    return np.ascontiguousarray(a.transpose(2, 1, 0, 3)).astype(FP8)


def _pack_res(wT, km, kk):
    """[K, M] -> [128, M//128, K//128, 128] partition-major for one-shot
    SBUF-resident DMA (contiguous per-partition reads)."""
    K, M = wT.shape
    assert K == kk * P and M == km * P
    a = wT.reshape(kk, P, km, P)          # [ks, p, mt, col]
    return np.ascontiguousarray(a.transpose(1, 2, 0, 3)).astype(FP8)


def _stripe(v, k):
    """[k*128] -> [128, k] with v[ks*128+p] at [p, ks]."""
    return np.ascontiguousarray(v.reshape(k, P).T).astype(np.float32)


def prepare_core_inputs(x, mask, rpb, ln1_g, ln1_b, qkv_w, q_bias, v_bias,
                        proj_w, proj_b, gamma1, ln2_g, ln2_b, fc1_w, fc1_b,
                        fc2_w, fc2_b, ad_dw, ad_db, ad_uw, ad_ub, gamma2):
    """Host-side layout prep. Returns (per_core_maps, flags)."""
    f32 = np.float32

    qkv_w = np.asarray(qkv_w, f32)
    wq = qkv_w[:C] * ALPHA            # q-side carries alpha (incl. D^-.5*K2)
    wk = qkv_w[C:2 * C] * BETA
    wv = qkv_w[2 * C:] * WS
    wqkT = np.concatenate([wq, wk], 0).T          # [C, 1536]
    wqk = _pack_w6(wqkT, 12, KC)
    wv_packed = np.ascontiguousarray(
        wv.T.reshape(KC, P, C).transpose(1, 0, 2)).astype(FP8)

    projw = _pack_w6(np.asarray(proj_w, f32).T * WS, KC, KC)
    fc1w = _pack_res(np.asarray(fc1_w, f32).T * WS, KM, KC)
    fc2w = _pack_res(np.asarray(fc2_w, f32).T * WS, KC, KM)
    adw = _pack_w6(np.asarray(ad_dw, f32).T * WS, KR, KC)
    auw = _pack_res(np.asarray(ad_uw, f32).T * WS, KC, KR)

    # rpb as exponent steps: int8 round(K8*rpb)+64 for the DVE schraudolph
    # path (the +64 fp8-exponent bias rides along), fp8 K2*rpb for the
    # identity-matmul (ACT) path where the bias comes from the ACT op
    rpbT = np.ascontiguousarray(np.asarray(rpb, f32).transpose(0, 2, 1))
    rpb8 = np.clip(np.round(K8 * rpbT) + 64.0, -127, 127).astype(np.int8)
    rpbf = (K2 * rpbT).astype(FP8)
    ident = np.eye(P).astype(FP8)

    q_bias_s = np.asarray(q_bias, f32) * ALPHA
    fb = (np.asarray(fc2_b, f32) + np.asarray(ad_ub, f32)) * WS

    vecs = np.stack([
        _stripe(np.asarray(gamma1, f32) / (OSC * WS), KC),
        _stripe(np.asarray(gamma2, f32) / WS, KC),
        _stripe(q_bias_s, KC),
        _stripe(np.asarray(proj_b, f32) * OSC * WS, KC),
        _stripe(fb, KC),
        _stripe(np.asarray(ln1_g, f32), KC),
        _stripe(np.asarray(ln1_b, f32), KC),
        _stripe(np.asarray(ln2_g, f32), KC),
        _stripe(np.asarray(ln2_b, f32), KC),
    ], 0)  # [NVEC, 128, KC]

    f1b = _stripe(np.asarray(fc1_b, f32), KM)
    adb = _stripe(np.asarray(ad_db, f32), KR)
    vb = (np.asarray(v_bias, f32) * WS).reshape(1, C).astype(f32)

    mask = np.asarray(mask)
    has_mask = not bool(mask.all())

    flags = (
        has_mask,
        bool(np.any(q_bias_s)),
        bool(np.any(v_bias)),
        bool(np.any(proj_b)),
        bool(np.any(fc1_b)),
        bool(np.any(fb)),
        bool(np.any(ad_db)),
        bool(np.all(ln1_g == 1.0) and not np.any(ln1_b)),
        bool(np.all(ln2_g == 1.0) and not np.any(ln2_b)),
    )

    shared = {
        "rpb8": rpb8, "rpbf": rpbf, "ident": ident,
        "wqk": wqk, "wv": wv_packed, "projw": projw,
        "fc1w": fc1w, "fc2w": fc2w, "adw": adw, "auw": auw,
        "vecs": vecs, "fc1b": f1b, "adb": adb, "vbias": vb,
    }

    x = np.asarray(x, f32)
    per_core = []
    for b in range(B):
        xT = np.ascontiguousarray(
            x[b].T.reshape(KC, P, N).transpose(1, 0, 2)).astype(f32)
        if has_mask:
            mb = np.where(mask[b], 1.0, 0.0).astype(f32)    # [N] over keys m
            mb = np.ascontiguousarray(mb.reshape(NT, P).T)  # [128, NT]
        else:
            mb = np.zeros((P, NT), f32)
        m = dict(shared)
        m["xT"] = xT
        m["maskb"] = mb
        per_core.append(m)
    return per_core, flags


def _ensure_ntff_hook():
    """The agent image lacks ``antenv.axon_hooks``; provide it and register
    the ctypes NTFF profile hook so trace=True works under axon."""
    import types
    try:
        from antenv.axon_hooks import get_axon_ntff_profile_hook  # noqa: F401
        return
    except ImportError:
        pass
    import antenv
    mod = types.ModuleType("antenv.axon_hooks")
    _h = {"hook": None}
    mod.set_axon_ntff_profile_hook = lambda h: _h.__setitem__("hook", h)
    mod.get_axon_ntff_profile_hook = lambda: _h["hook"]
    sys.modules["antenv.axon_hooks"] = mod
    antenv.axon_hooks = mod
    try:
        from trn_agent_boot.trn_boot import _ntff_profile_via_ctypes
        hook = _ntff_profile_via_ctypes("/opt/axon/libaxon_pjrt.so")
        if hook is not None:
            mod.set_axon_ntff_profile_hook(hook)
    except Exception as e:  # profiling degrades, run still works
        print("ntff hook setup failed:", e)


def run_sharded(inputs, trace=False, trace_kwargs=None):
    """Compile (cached) + run on 8 cores. Returns (out [B,N,C] f32, results)."""
    from concourse.bass_utils import run_bass_kernel_spmd
    if trace:
        _ensure_ntff_hook()

    per_core, flags = prepare_core_inputs(**inputs)
    if flags not in _PROG_CACHE:
        _PROG_CACHE[flags] = _build(flags)
    nc = _PROG_CACHE[flags]

    kw = {}
    if trace:
        kw["trace"] = True
        kw["trace_cores"] = [0]
        if trace_kwargs:
            kw["trace_kwargs"] = trace_kwargs
    res = run_bass_kernel_spmd(nc, per_core, core_ids=list(range(B)), **kw)

    out = np.empty((B, N, C), np.float32)
    for b in range(B):
        oT = res.results[b]["outT"]          # [128, KC, N]
        out[b] = oT.transpose(1, 0, 2).reshape(C, N).T
    return out, res


def kernel(**inputs):
    out, _ = run_sharded(inputs, trace=False)
    return out


# revision 54
# speedup vs baseline: 1.6656x; 1.0811x over previous
"""Trainium2 Bass kernel for nn_Block_89361089561275 (dense transformer block).

Sharding: data-parallel over batch B=8 -> one batch element per NeuronCore.
No collectives. Feature-transposed layout (features on SBUF partitions,
tokens on the free dim) throughout.

Key speed tricks over the bf16 baseline:
  * All big matmuls (qkv, v, proj, fc1, fc2, adapters) run as fp8e4m3
    DoubleRow matmuls: lhsT [128,2,M] + rhs [128,2,N] contract 256 K per
    instruction (~2x tensor-engine throughput). Weights are host-scaled
    (x32-ish) into fp8's sweet spot; inverse scales fold into downstream
    per-feature vectors / activation `scale=` operands.
  * Softmax exp is a Schraudolph bit-trick fused into the mandatory
    PSUM-evacuation op: the q/k weight scales carry alpha*beta =
    128*log2(e)*D^-0.5 so the score PSUM is already K2*s; one DVE
    scalar_tensor_tensor computes int16(round(st + 8192 + rpb_i8)) whose
    bf16 bit-pattern IS exp(s+r) up to a global power-of-2 scale that
    cancels in softmax. rpb ships as int8 = round(K2*rpb) (+8192 via the
    freed scalar slot). ScalarE does no exp at all.
  * The softmax denominator falls out of the AV matmul via a ones-column
    appended to v (M=65 accumulators, row 64 = colsum).
  * LN mean/var still come from ones-matmuls, but the finish chain runs on
    [1,N] rows and broadcasts rstd/-mean*rstd back across partitions with a
    K=1 ones matmul (no DRAM bounce round-trips).
  * fc1/fc2 weights are SBUF-resident (loaded once, streamed during
    attention), gelu/relu descale by 1/32 via the activation scale operand.
"""

import sys

for _p in ("/opt/trn_rl_repo",):
    if _p not in sys.path:
        sys.path.insert(0, _p)

import numpy as np
import ml_dtypes

BF16 = ml_dtypes.bfloat16
FP8 = ml_dtypes.float8_e4m3

B, N, C, H = 8, 1024, 768, 12
D = C // H            # 64
MLP = 4 * C           # 3072
RED = C // 3          # 256
EPS = 1e-5
P = 128
KC = C // P           # 6   c-chunks
KM = MLP // P         # 24  mlp-chunks
KR = RED // P         # 2   adapter chunks
NT = N // P           # 8   token tiles
HALF = 512
NSL = (slice(0, HALF), slice(HALF, N))

K2 = 128.0 / float(np.log(2.0))   # exponent-steps-per-unit-score for bf16
K8 = 8.0 / float(np.log(2.0))     # same for fp8e4m3 (3 mantissa bits)
IBIAS = 64.0                      # int8 exponent offset (global 2^-7 scale)
SCALE = D ** -0.5
AB = K2 * SCALE                   # alpha*beta for the q/k weight pair
ALPHA = float(np.sqrt(AB))        # q-side host scale
BETA = float(np.sqrt(AB))         # k-side host scale
WS = 32.0                         # generic fp8 weight scale
OSC = 256.0                       # oT = OSC * o_true (8.0 folded in evac)

_PROG_CACHE: dict = {}

# indices into the packed [n, 128, KC] per-feature vector table
V_G1, V_G2, V_QB, V_PB, V_FB, V_L1G, V_L1B, V_L2G, V_L2B = range(9)
NVEC = 9


def _build(flags):
    """Build the single-core Bass program. flags is a tuple of bools:
    (has_mask, qb_nz, vb_nz, pb_nz, f1b_nz, fb_nz, adb_nz,
     ln1_triv, ln2_triv)
    """
    (has_mask, qb_nz, vb_nz, pb_nz, f1b_nz, fb_nz, adb_nz,
     ln1_triv, ln2_triv) = flags

    import concourse.tile as tile
    from concourse import bacc, mybir
    from contextlib import ExitStack

    f32 = mybir.dt.float32
    bf16 = mybir.dt.bfloat16
    fp8 = mybir.dt.float8e4
    i16 = mybir.dt.int16
    i8 = mybir.dt.int8
    AF = mybir.ActivationFunctionType
    OP = mybir.AluOpType
    PM = mybir.MatmulPerfMode

    nc = bacc.Bacc("TRN2")

    # ---- external I/O ----
    x_d = nc.declare_dram_parameter("xT", [P, KC, N], f32, isOutput=False)
    rpb8_d = nc.declare_dram_parameter("rpb8", [H, N, N], i8, isOutput=False)
    rpbf_d = nc.declare_dram_parameter("rpbf", [H, N, N], fp8, isOutput=False)
    id_d = nc.declare_dram_parameter("ident", [P, P], fp8, isOutput=False)
    wqk_d = nc.declare_dram_parameter("wqk", [12, P, KC, P], fp8, isOutput=False)
    wv_d = nc.declare_dram_parameter("wv", [P, KC, C], fp8, isOutput=False)
    pw_d = nc.declare_dram_parameter("projw", [KC, P, KC, P], fp8, isOutput=False)
    f1_d = nc.declare_dram_parameter("fc1w", [P, KM, KC, P], fp8, isOutput=False)
    f2_d = nc.declare_dram_parameter("fc2w", [P, KC, KM, P], fp8, isOutput=False)
    ad_d = nc.declare_dram_parameter("adw", [KR, P, KC, P], fp8, isOutput=False)
    au_d = nc.declare_dram_parameter("auw", [P, KC, KR, P], fp8, isOutput=False)
    vec_d = nc.declare_dram_parameter("vecs", [NVEC, P, KC], f32, isOutput=False)
    f1b_d = nc.declare_dram_parameter("fc1b", [P, KM], f32, isOutput=False)
    adb_d = nc.declare_dram_parameter("adb", [P, KR], f32, isOutput=False)
    vb_d = nc.declare_dram_parameter("vbias", [1, C], f32, isOutput=False)
    mb_d = nc.declare_dram_parameter("maskb", [P, NT], f32, isOutput=False)
    out_d = nc.declare_dram_parameter("outT", [P, KC, N], f32, isOutput=True)

    with tile.TileContext(nc) as tc, ExitStack() as ctx:
        sb = ctx.enter_context(tc.tile_pool(name="sb", bufs=1))
        pp = ctx.enter_context(tc.tile_pool(name="pp", bufs=1, space="PSUM"))
        dram = ctx.enter_context(tc.tile_pool(name="dram", bufs=2, space="DRAM"))

        # ---- persistent tiles ----
        VP = 80  # vaug row pitch: D+1 padded so the DoubleRow mt-pair
        #          stride (H*VP bytes) stays 16B-aligned
        xres = sb.tile([P, KC, N], f32, tag="xres", bufs=1)
        qkT = sb.tile([P, 12, N], bf16, tag="qkT", bufs=1)
        vaug = sb.tile([P, NT, H, VP], fp8, tag="vaug", bufs=1)
        oT = sb.tile([P, KC, N], fp8, tag="oT", bufs=1)
        xnT = sb.tile([P, KC, N], fp8, tag="xnT", bufs=1)
        xn2T = sb.tile([P, KC, N], fp8, tag="xn2T", bufs=1)
        a1T = sb.tile([P, KR, N], fp8, tag="a1", bufs=1)

        for ch in range(KC):  # per-chunk loads so LN1 stats start early
            nc.sync.dma_start(out=xres[:, ch], in_=x_d[:, ch])

        ones_bf = sb.tile([P, 1], bf16, tag="ones", bufs=1)
        nc.vector.memset(ones_bf, 1.0)
        ones_row = sb.tile([1, P], bf16, tag="onesr", bufs=1)
        nc.vector.memset(ones_row, 1.0)
        # one contiguous memset initializes every byte (ones column + pad);
        # v evacuations overwrite the [0:D] slices later
        nc.vector.memset(vaug.rearrange("p t h c -> p (t h c)"), 1.0)

        zero_col = sb.tile([P, 1], f32, tag="zcol", bufs=1)
        nc.vector.memset(zero_col, 0.0)
        eps_col = sb.tile([P, 1], f32, tag="ecol", bufs=1)
        nc.vector.memset(eps_col, float(EPS))

        vecs = sb.tile([P, NVEC, KC], f32, tag="vecs", bufs=1)
        nc.sync.dma_start(out=vecs, in_=vec_d[:].rearrange("v p k -> p v k"))

        def vec(i):
            return vecs[:, i]  # [128, KC]

        if f1b_nz:
            f1b = sb.tile([P, KM], f32, tag="f1b", bufs=1)
            nc.sync.dma_start(out=f1b, in_=f1b_d[:])
        if adb_nz:
            adb = sb.tile([P, KR], f32, tag="adb", bufs=1)
            nc.sync.dma_start(out=adb, in_=adb_d[:])
        if vb_nz:
            vb1 = sb.tile([1, C], f32, tag="vb1", bufs=1)
            nc.sync.dma_start(out=vb1, in_=vb_d[:])
            vb_b = sb.tile([P, C], f32, tag="vb_b", bufs=1)
            scratch = dram.tile([1, C], f32, tag="bscratch", bufs=2)
            nc.sync.dma_start(out=scratch, in_=vb1)
            nc.sync.dma_start(out=vb_b, in_=scratch.to_broadcast(vb_b.shape))
        if has_mask:
            maskb = sb.tile([P, NT], f32, tag="maskb", bufs=1)
            nc.sync.dma_start(out=maskb, in_=mb_d[:])

        # ---------------- layernorm (feature-transposed) ----------------
        # stats via ones-matmuls into base-0 psum rows (sum, sum-sq), one
        # [1,512] tile per (stat, nk) so they fit the shared "st" slots --
        # engine ops need all operands on the same partitions.
        def ln_stats_alloc(pfx):
            return [pp.tile([1, HALF], f32, tag="st", bufs=4,
                            name=f"{pfx}{i}") for i in range(4)]

        def ln_stats_ch(stq, ch, xbs, src):
            xb = sb.tile([P, N], bf16, tag="xb", bufs=KC)
            xbs.append(xb)
            nc.vector.tensor_copy(out=xb, in_=src[:, ch])
            x2 = sb.tile([P, N], bf16, tag="x2", bufs=2)
            nc.vector.tensor_mul(x2, xb, xb)
            for nk in range(2):
                nc.tensor.matmul(stq[nk][0:1, :], lhsT=ones_bf,
                                 rhs=xb[:, NSL[nk]],
                                 start=(ch == 0), stop=(ch == KC - 1))
                nc.tensor.matmul(stq[2 + nk][0:1, :], lhsT=ones_bf,
                                 rhs=x2[:, NSL[nk]],
                                 start=(ch == 0), stop=(ch == KC - 1))

        def ln_finish(stq, xbs, dst, g_i, b_i, triv):
            # evacuate sum rows via ACT (with the 1/C fold), scatter the
            # per-token stats across 128 lanes via DMA so sqrt/reciprocal
            # run wide, gather back, K=1 PE broadcast.
            mq = sb.tile([1, 2, N], f32, tag="lnrow", bufs=1)
            for nk in range(2):
                nc.scalar.activation(out=mq[0:1, 0, NSL[nk]], in_=stq[nk],
                                     func=AF.Copy, scale=1.0 / C)
                nc.scalar.activation(out=mq[0:1, 1, NSL[nk]], in_=stq[2 + nk],
                                     func=AF.Copy, scale=1.0 / C)
            s128 = sb.tile([P, 2, NT], f32, tag="s128", bufs=2)
            nc.sync.dma_start(out=s128[:, 0], in_=mq[0:1, 0])
            nc.sync.dma_start(out=s128[:, 1], in_=mq[0:1, 1])
            t128 = sb.tile([P, NT], f32, tag="t128", bufs=4)
            nc.vector.tensor_mul(t128, s128[:, 0], s128[:, 0])
            v128 = sb.tile([P, NT], f32, tag="t128", bufs=4)
            nc.vector.tensor_sub(v128, s128[:, 1], t128)
            sd = sb.tile([P, NT], f32, tag="t128", bufs=4)
            nc.scalar.activation(out=sd, in_=v128, func=AF.Sqrt,
                                 bias=eps_col)
            r2 = sb.tile([P, 2, NT], bf16, tag="r128", bufs=2)
            with nc.allow_low_precision(reason="bf16 rstd broadcast"):
                nc.vector.reciprocal(r2[:, 0], sd)             # rstd
            nc.vector.scalar_tensor_tensor(out=r2[:, 1], in0=s128[:, 0],
                                           scalar=-1.0, in1=r2[:, 0],
                                           op0=OP.mult, op1=OP.mult)
            # gather to one SBUF row, broadcast across partitions with K=1
            # matmuls (PSUM is free during the LN chains), evacuate via ACT
            rn = sb.tile([1, 2, N], bf16, tag="lnrn", bufs=1)
            nc.sync.dma_start(out=rn[0:1, 0], in_=r2[:, 0])
            nc.sync.dma_start(out=rn[0:1, 1], in_=r2[:, 1])
            ab = sb.tile([P, 2, N], bf16, tag="lnab", bufs=1)
            for i in range(2):
                for nk in range(2):
                    rb = pp.tile([P, HALF], f32, tag="st", bufs=4,
                                 name="lnrb")
                    nc.tensor.matmul(rb, lhsT=ones_row,
                                     rhs=rn[0:1, i, NSL[nk]],
                                     start=True, stop=True)
                    nc.scalar.activation(out=ab[:, i, NSL[nk]], in_=rb,
                                         func=AF.Copy)
            # apply in nk-halves, nk0 for every chunk first, so consumers
            # of the first half can start ~5us earlier
            for nk in range(2):
                for ch in range(KC):
                    t1 = sb.tile([P, HALF], bf16, tag="x2h", bufs=2)
                    nc.vector.tensor_mul(t1, xbs[ch][:, NSL[nk]],
                                         ab[:, 0, NSL[nk]])
                    if triv:
                        nc.vector.tensor_add(dst[:, ch, NSL[nk]], t1,
                                             ab[:, 1, NSL[nk]])
                    else:
                        nc.vector.tensor_add(t1, t1, ab[:, 1, NSL[nk]])
                        nc.vector.tensor_scalar(
                            out=dst[:, ch, NSL[nk]], in0=t1,
                            scalar1=vec(g_i)[:, ch:ch + 1],
                            scalar2=vec(b_i)[:, ch:ch + 1],
                            op0=OP.mult, op1=OP.add)

        # ---------------- LN1 -> xnT (fp8) ----------------
        stq1 = ln_stats_alloc("ln1")
        xbs1 = []
        for ch in range(KC):
            ln_stats_ch(stq1, ch, xbs1, xres)
        ln_finish(stq1, xbs1, xnT, V_L1G, V_L1B, ln1_triv)

        # ---------------- QKV (fp8 DoubleRow) ----------------
        def dr_mms(acc, wt, rhs_t, nsl, kd=KC):
            for j in range(kd // 2):
                nc.tensor.matmul(acc, lhsT=wt[:, 2 * j:2 * j + 2, :],
                                 rhs=rhs_t[:, 2 * j:2 * j + 2, nsl],
                                 start=(j == 0), stop=(j == kd // 2 - 1),
                                 perf_mode=PM.DoubleRow)

        # qkv work is emitted as "fill" groups, mostly interleaved into the
        # attention stream so the PE always has runnable instructions while
        # the DVE/ACT produce softmax terms (keeps the HAM clock warm).
        # Accumulators come from the shared 1-bank "st" slots.
        wv_sb = sb.tile([P, KC, C], fp8, tag="h1", bufs=1)
        nc.sync.dma_start(out=wv_sb, in_=wv_d[:])
        wqk_sb = sb.tile([P, 12, KC, P], fp8, tag="wqk", bufs=1)
        nc.sync.dma_start(out=wqk_sb, in_=wqk_d[:])

        def qk_group(blk, nk):
            mm = pp.tile([P, HALF], f32, tag="st", bufs=4, name="qkacc")
            dr_mms(mm, wqk_sb[:, blk], xnT, NSL[nk])
            dst = qkT[:, blk, NSL[nk]]
            if blk < 6 and qb_nz:
                nc.vector.tensor_scalar_add(dst, mm, vec(V_QB)[:, blk:blk + 1])
            else:
                nc.scalar.copy(out=dst, in_=mm)

        def v_group(t, off):
            cw = HALF if off == 0 else C - HALF
            mm = pp.tile([P, HALF], f32, tag="st", bufs=4, name="vacc")
            for j in range(KC // 2):
                nc.tensor.matmul(
                    mm[:, :cw], lhsT=xnT[:, 2 * j:2 * j + 2, t * P:(t + 1) * P],
                    rhs=wv_sb[:, 2 * j:2 * j + 2, off:off + cw],
                    start=(j == 0), stop=(j == KC // 2 - 1),
                    perf_mode=PM.DoubleRow)
            dst = vaug[:, t, off // D:(off + cw) // D, :D]
            src = mm[:, :cw].rearrange("p (h d) -> p h d", d=D)
            if vb_nz:
                nc.vector.tensor_add(
                    dst, src,
                    vb_b[:, off:off + cw].rearrange("p (h d) -> p h d", d=D))
            else:
                nc.vector.tensor_copy(out=dst, in_=src)

        # pre-attention: q/k blocks 0+6 and all v tiles (the strided vaug
        # writes must stay in simple program order before the AV reads --
        # Tile's range tracker misses the cross-engine dep otherwise)
        for blk in (0, 6):
            for nk in range(2):
                qk_group(blk, nk)
        for t in range(NT):
            v_group(t, 0)
            v_group(t, 1)

        # fill queue of remaining q/k blocks for the attention loop: one
        # group per mt keeps the PE fed while softmax terms are produced
        fill: list = []
        for bq in (1, 2, 3, 4, 5):
            for blk in (bq, 6 + bq):
                for nk in range(2):
                    fill.append(lambda b=blk, n=nk: qk_group(b, n))

        # resident MLP weights: queue the DMAs here so they land during
        # attention (after the first head pairs' rpb tiles)
        f1sb = sb.tile([P, KM, KC, P], fp8, tag="f1sb", bufs=1)
        f2sb = sb.tile([P, KC, KM, P], fp8, tag="f2sb", bufs=1)
        ausb = sb.tile([P, KC, KR, P], fp8, tag="ausb", bufs=1)

        # ---------------- attention ----------------
        ident_sb = sb.tile([P, P], fp8, tag="ident", bufs=1)
        nc.sync.dma_start(out=ident_sb, in_=id_d[:])

        def evac_head(o_ps, hp, hh):
            # ACT evacuates the accumulators (frees PSUM fast); colsum row
            # scatters across 128 lanes for the reciprocal, gathers back,
            # K=1 PE broadcast, one bf16 2x DVE multiply.
            ou = sb.tile([D + 1, N], bf16, tag="ou", bufs=2)
            for nk in range(2):
                nc.scalar.activation(out=ou[:, NSL[nk]], in_=o_ps[nk],
                                     func=AF.Copy)
            cs = sb.tile([P, NT], bf16, tag="cs", bufs=2)
            nc.sync.dma_start(out=cs, in_=ou[D:D + 1, :])
            rcp = sb.tile([P, NT], bf16, tag="cs", bufs=2)
            with nc.allow_low_precision(reason="bf16 softmax denom"):
                # 8/colsum (the OSC/WS fold rides the reciprocal input: the
                # scatter wrote colsum; scale via tensor_scalar on the way)
                nc.vector.tensor_scalar_mul(rcp, cs, float(WS / OSC))
                nc.vector.reciprocal(rcp, rcp)
            scr = dram.tile([1, N], bf16, tag="evscr", bufs=4)
            nc.sync.dma_start(out=scr, in_=rcp)
            rbs = sb.tile([P, N], bf16, tag="rbs", bufs=2)
            nc.sync.dma_start(out=rbs, in_=scr.to_broadcast(rbs.shape))
            if hh == 0:
                nc.vector.tensor_mul(oT[0:D, hp, :], ou[0:D, :], rbs[0:D, :])
            else:
                ot_tmp = sb.tile([D, N], fp8, tag="ott", bufs=2)
                nc.vector.tensor_mul(ot_tmp, ou[0:D, :], rbs[0:D, :])
                nc.sync.dma_start(out=oT[D:P, hp, :], in_=ot_tmp)

        for hp in range(H // 2):
            qh = [qkT[hh * D:(hh + 1) * D, hp, :] for hh in range(2)]
            kh = [qkT[hh * D:(hh + 1) * D, 6 + hp, :] for hh in range(2)]
            o_ps = [[pp.tile([D + 1, HALF], f32, tag="acc", bufs=4,
                             name=f"o_ps{hh}{nk}") for nk in range(2)]
                    for hh in range(2)]
            if hp == 2:  # let early rpb tiles through the queue first
                nc.sync.dma_start(out=f1sb, in_=f1_d[:])
            if hp == 3:
                nc.sync.dma_start(out=f2sb, in_=f2_d[:])
                nc.sync.dma_start(out=ausb, in_=au_d[:])
            if hp == 4:  # prefetch proj weights so proj starts instantly
                pwts = []
                for mt in range(KC):
                    wt = sb.tile([P, KC, P], fp8, tag="wp", bufs=KC)
                    nc.sync.dma_start(out=wt, in_=pw_d[mt])
                    pwts.append(wt)
            def emit_av(ptp, pi):
                # fp8 DoubleRow: one MM contracts an mt-pair (256 tokens)
                for hh in range(2):
                    for nk in range(2):
                        nc.tensor.matmul(
                            o_ps[hh][nk][:, :],
                            lhsT=vaug[:, 2 * pi:2 * pi + 2, 2 * hp + hh, :D + 1],
                            rhs=ptp[hh][:, :, NSL[nk]].bitcast(fp8),
                            start=(pi == 0), stop=(pi == NT // 2 - 1),
                            perf_mode=PM.DoubleRow)

            pend = None  # software-pipelined: AV(pair-1) issues while the
            ptp = None
            for mt in range(NT):  # DVE/ACT produce pts(mt)
                # hh=1 tiles mostly take the "ACT path": rpb is accumulated
                # into the score PSUM by an fp8 identity matmul, and the
                # scale/+64/int8 conversion runs on the scalar engine; the
                # rest use the DVE scalar_tensor_tensor with int8 rpb. This
                # splits the unavoidable fp32-PSUM evacuation across both
                # engines. Scores live in per-(hh,nk) 1-bank quarters so
                # the pipeline is 4 deep.
                act_path = [False, mt % 4 != 3]
                stq, rps = [[None, None], [None, None]], []
                for hh in range(2):
                    h = 2 * hp + hh
                    msl = slice(mt * P, (mt + 1) * P)
                    if act_path[hh]:
                        rp = sb.tile([P, N], fp8, tag="rpf", bufs=4)
                        nc.sync.dma_start(out=rp, in_=rpbf_d[h, msl, :])
                    else:
                        rp = sb.tile([P, N], i8, tag="rpb", bufs=5)
                        nc.sync.dma_start(out=rp, in_=rpb8_d[h, msl, :])
                    rps.append(rp)
                if mt % 2 == 0:
                    if pend is not None:
                        emit_av(*pend)
                    ptp = [sb.tile([P, 2, N], i8, tag="pt", bufs=4,
                                   name=f"ptp{_hh}")
                           for _hh in range(2)]
                for nk in range(2):  # hh-pairs issue adjacently -> row-group
                    for hh in range(2):  # concurrency on the PE
                        stq[hh][nk] = pp.tile([P, HALF], f32, tag="st",
                                              bufs=4, name="stq")
                        nc.tensor.matmul(stq[hh][nk],
                                         lhsT=kh[hh][:, mt * P:(mt + 1) * P],
                                         rhs=qh[hh][:, NSL[nk]],
                                         start=True, stop=not act_path[hh])
                    for hh in range(2):
                        if act_path[hh]:
                            nc.tensor.matmul(stq[hh][nk], lhsT=ident_sb,
                                             rhs=rps[hh][:, NSL[nk]],
                                             start=False, stop=True)
                    for hh in range(2):
                        dst8 = ptp[hh][:, mt % 2, NSL[nk]]
                        # int8(st/16 + 64 + K8*rpb): bit-pattern is the
                        # fp8e4m3 encoding of exp(s+r) up to a global
                        # power-of-2 scale that cancels in softmax
                        if act_path[hh]:
                            nc.scalar.activation(out=dst8, in_=stq[hh][nk],
                                                 func=AF.Copy, bias=IBIAS,
                                                 scale=1.0 / 16.0)
                        else:
                            nc.vector.scalar_tensor_tensor(
                                out=dst8, in0=stq[hh][nk], scalar=1.0 / 16.0,
                                in1=rps[hh][:, NSL[nk]],
                                op0=OP.mult, op1=OP.add)
                        if has_mask:
                            nc.vector.tensor_scalar_mul(dst8, dst8,
                                                        maskb[:, mt:mt + 1])
                if fill:  # keep the PE fed while softmax terms are produced
                    fill.pop(0)()
                if mt % 2 == 1:
                    pend = (ptp, mt // 2)
            emit_av(*pend)
            evac_head(o_ps[0], hp, 0)
            evac_head(o_ps[1], hp, 1)

        # ---------------- proj + residual 1 + LN2 stats ----------------
        stq2 = ln_stats_alloc("ln2")
        xbs2 = []
        for mt in range(KC):
            for nk in range(2):
                mm = pp.tile([P, HALF], f32, tag="acc", bufs=4)
                dr_mms(mm, pwts[mt], oT, NSL[nk])
                if pb_nz:
                    nc.vector.tensor_scalar_add(mm, mm, vec(V_PB)[:, mt:mt + 1])
                nc.vector.scalar_tensor_tensor(
                    out=xres[:, mt, NSL[nk]], in0=mm,
                    scalar=vec(V_G1)[:, mt:mt + 1],
                    in1=xres[:, mt, NSL[nk]], op0=OP.mult, op1=OP.add)
            if mt >= 1:  # stats lag the proj chunks so the PE never waits
                ln_stats_ch(stq2, mt - 1, xbs2, xres)
        ln_stats_ch(stq2, KC - 1, xbs2, xres)
        ln_finish(stq2, xbs2, xn2T, V_L2G, V_L2B, ln2_triv)

        # ---------------- adapter down (relu via ACT, 1/32 descale) -----
        for mt in range(KR):
            wt = sb.tile([P, KC, P], fp8, tag="w6", bufs=2)
            nc.sync.dma_start(out=wt, in_=ad_d[mt])
            for nk in range(2):
                mm = pp.tile([P, HALF], f32, tag="acc", bufs=4)
                dr_mms(mm, wt, xn2T, NSL[nk])
                nc.scalar.activation(
                    out=a1T[:, mt, NSL[nk]], in_=mm, func=AF.Relu,
                    scale=1.0 / WS,
                    bias=(adb[:, mt:mt + 1] if adb_nz else zero_col))

        # ---------------- MLP (fp8 DR, resident weights) ----------------
        for nk in range(2):
            h1 = sb.tile([P, KM, HALF], fp8, tag="h1", bufs=1)
            for mt in range(KM):
                mm = pp.tile([P, HALF], f32, tag="acc", bufs=4)
                dr_mms(mm, f1sb[:, mt], xn2T, NSL[nk])
                nc.scalar.activation(
                    out=h1[:, mt], in_=mm, func=AF.Gelu, scale=1.0 / WS,
                    bias=(f1b[:, mt:mt + 1] if f1b_nz else zero_col))
            for mt in range(KC):
                mm = pp.tile([P, HALF], f32, tag="acc", bufs=4)
                for j in range(KM // 2):
                    nc.tensor.matmul(mm, lhsT=f2sb[:, mt, 2 * j:2 * j + 2, :],
                                     rhs=h1[:, 2 * j:2 * j + 2, :],
                                     start=(j == 0), stop=False,
                                     perf_mode=PM.DoubleRow)
                nc.tensor.matmul(mm, lhsT=ausb[:, mt],
                                 rhs=a1T[:, 0:KR, NSL[nk]],
                                 start=False, stop=True,
                                 perf_mode=PM.DoubleRow)
                if fb_nz:
                    nc.vector.tensor_scalar_add(mm, mm, vec(V_FB)[:, mt:mt + 1])
                nc.vector.scalar_tensor_tensor(
                    out=xres[:, mt, NSL[nk]], in0=mm,
                    scalar=vec(V_G2)[:, mt:mt + 1],
                    in1=xres[:, mt, NSL[nk]], op0=OP.mult, op1=OP.add)
                nc.sync.dma_start(out=out_d[:, mt, NSL[nk]],
                                  in_=xres[:, mt, NSL[nk]])

    if not nc.is_finalized():
        nc.finalize()
    return nc


def _pack_w6(wT, km, kk):
    """[K, M] (K=contraction, M=out) -> [M//128, 128, K//128, 128] fp8 tiles
    laid out so each DMA partition read is contiguous."""
    K, M = wT.shape
    assert K == kk * P and M == km * P
    a = wT.reshape(kk, P, km, P)          # [ks, p, mt, col]
    return np.ascontiguousarray(a.transpose(2, 1, 0, 3)).astype(FP8)


def _pack_res(wT, km, kk):
    """[K, M] -> [128, M//128, K//128, 128] partition-major for one-shot
    SBUF-resident DMA (contiguous per-partition reads)."""
    K, M = wT.shape
    assert K == kk * P and M == km * P
    a = wT.reshape(kk, P, km, P)          # [ks, p, mt, col]
    return np.ascontiguousarray(a.transpose(1, 2, 0, 3)).astype(FP8)


def _stripe(v, k):
    """[k*128] -> [128, k] with v[ks*128+p] at [p, ks]."""
    return np.ascontiguousarray(v.reshape(k, P).T).astype(np.float32)


def prepare_core_inputs(x, mask, rpb, ln1_g, ln1_b, qkv_w, q_bias, v_bias,
                        proj_w, proj_b, gamma1, ln2_g, ln2_b, fc1_w, fc1_b,
                        fc2_w, fc2_b, ad_dw, ad_db, ad_uw, ad_ub, gamma2):
    """Host-side layout prep. Returns (per_core_maps, flags)."""
    f32 = np.float32

    qkv_w = np.asarray(qkv_w, f32)
    wq = qkv_w[:C] * ALPHA            # q-side carries alpha (incl. D^-.5*K2)
    wk = qkv_w[C:2 * C] * BETA
    wv = qkv_w[2 * C:] * WS
    wqkT = np.concatenate([wq, wk], 0).T          # [C, 1536]
    wqk = _pack_w6(wqkT, 12, KC)
    wv_packed = np.ascontiguousarray(
        wv.T.reshape(KC, P, C).transpose(1, 0, 2)).astype(FP8)

    projw = _pack_w6(np.asarray(proj_w, f32).T * WS, KC, KC)
    fc1w = _pack_res(np.asarray(fc1_w, f32).T * WS, KM, KC)
    fc2w = _pack_res(np.asarray(fc2_w, f32).T * WS, KC, KM)
    adw = _pack_w6(np.asarray(ad_dw, f32).T * WS, KR, KC)
    auw = _pack_res(np.asarray(ad_uw, f32).T * WS, KC, KR)

    # rpb as exponent steps: int8 round(K8*rpb)+64 for the DVE schraudolph
    # path (the +64 fp8-exponent bias rides along), fp8 K2*rpb for the
    # identity-matmul (ACT) path where the bias comes from the ACT op
    rpbT = np.ascontiguousarray(np.asarray(rpb, f32).transpose(0, 2, 1))
    rpb8 = np.clip(np.round(K8 * rpbT) + 64.0, -127, 127).astype(np.int8)
    rpbf = (K2 * rpbT).astype(FP8)
    ident = np.eye(P).astype(FP8)

    q_bias_s = np.asarray(q_bias, f32) * ALPHA
    fb = (np.asarray(fc2_b, f32) + np.asarray(ad_ub, f32)) * WS

    vecs = np.stack([
        _stripe(np.asarray(gamma1, f32) / (OSC * WS), KC),
        _stripe(np.asarray(gamma2, f32) / WS, KC),
        _stripe(q_bias_s, KC),
        _stripe(np.asarray(proj_b, f32) * OSC * WS, KC),
        _stripe(fb, KC),
        _stripe(np.asarray(ln1_g, f32), KC),
        _stripe(np.asarray(ln1_b, f32), KC),
        _stripe(np.asarray(ln2_g, f32), KC),
        _stripe(np.asarray(ln2_b, f32), KC),
    ], 0)  # [NVEC, 128, KC]

    f1b = _stripe(np.asarray(fc1_b, f32), KM)
    adb = _stripe(np.asarray(ad_db, f32), KR)
    vb = (np.asarray(v_bias, f32) * WS).reshape(1, C).astype(f32)

    mask = np.asarray(mask)
    has_mask = not bool(mask.all())

    flags = (
        has_mask,
        bool(np.any(q_bias_s)),
        bool(np.any(v_bias)),
        bool(np.any(proj_b)),
        bool(np.any(fc1_b)),
        bool(np.any(fb)),
        bool(np.any(ad_db)),
        bool(np.all(ln1_g == 1.0) and not np.any(ln1_b)),
        bool(np.all(ln2_g == 1.0) and not np.any(ln2_b)),
    )

    shared = {
        "rpb8": rpb8, "rpbf": rpbf, "ident": ident,
        "wqk": wqk, "wv": wv_packed, "projw": projw,
        "fc1w": fc1w, "fc2w": fc2w, "adw": adw, "auw": auw,
        "vecs": vecs, "fc1b": f1b, "adb": adb, "vbias": vb,
    }

    x = np.asarray(x, f32)
    per_core = []
    for b in range(B):
        xT = np.ascontiguousarray(
            x[b].T.reshape(KC, P, N).transpose(1, 0, 2)).astype(f32)
        if has_mask:
            mb = np.where(mask[b], 1.0, 0.0).astype(f32)    # [N] over keys m
            mb = np.ascontiguousarray(mb.reshape(NT, P).T)  # [128, NT]
        else:
            mb = np.zeros((P, NT), f32)
        m = dict(shared)
        m["xT"] = xT
        m["maskb"] = mb
        per_core.append(m)
    return per_core, flags


def _ensure_ntff_hook():
    """The agent image lacks ``antenv.axon_hooks``; provide it and register
    the ctypes NTFF profile hook so trace=True works under axon."""
    import types
    try:
        from antenv.axon_hooks import get_axon_ntff_profile_hook  # noqa: F401
        return
    except ImportError:
        pass
    import antenv
    mod = types.ModuleType("antenv.axon_hooks")
    _h = {"hook": None}
    mod.set_axon_ntff_profile_hook = lambda h: _h.__setitem__("hook", h)
    mod.get_axon_ntff_profile_hook = lambda: _h["hook"]
    sys.modules["antenv.axon_hooks"] = mod
    antenv.axon_hooks = mod
    try:
        from trn_agent_boot.trn_boot import _ntff_profile_via_ctypes
        hook = _ntff_profile_via_ctypes("/opt/axon/libaxon_pjrt.so")
        if hook is not None:
            mod.set_axon_ntff_profile_hook(hook)
    except Exception as e:  # profiling degrades, run still works
        print("ntff hook setup failed:", e)


def run_sharded(inputs, trace=False, trace_kwargs=None):
    """Compile (cached) + run on 8 cores. Returns (out [B,N,C] f32, results)."""
    from concourse.bass_utils import run_bass_kernel_spmd
    if trace:
        _ensure_ntff_hook()

    per_core, flags = prepare_core_inputs(**inputs)
    if flags not in _PROG_CACHE:
        _PROG_CACHE[flags] = _build(flags)
    nc = _PROG_CACHE[flags]

    kw = {}
    if trace:
        kw["trace"] = True
        kw["trace_cores"] = [0]
        if trace_kwargs:
            kw["trace_kwargs"] = trace_kwargs
    res = run_bass_kernel_spmd(nc, per_core, core_ids=list(range(B)), **kw)

    out = np.empty((B, N, C), np.float32)
    for b in range(B):
        oT = res.results[b]["outT"]          # [128, KC, N]
        out[b] = oT.transpose(1, 0, 2).reshape(C, N).T
    return out, res


def kernel(**inputs):
    out, _ = run_sharded(inputs, trace=False)
    return out


# revision 56
# speedup vs baseline: 1.7294x; 1.0383x over previous
"""Trainium2 Bass kernel for nn_Block_89361089561275 (dense transformer block).

Sharding: data-parallel over batch B=8 -> one batch element per NeuronCore.
No collectives. Feature-transposed layout (features on SBUF partitions,
tokens on the free dim) throughout.

Key speed tricks over the bf16 baseline:
  * All big matmuls (qkv, v, proj, fc1, fc2, adapters) run as fp8e4m3
    DoubleRow matmuls: lhsT [128,2,M] + rhs [128,2,N] contract 256 K per
    instruction (~2x tensor-engine throughput). Weights are host-scaled
    (x32-ish) into fp8's sweet spot; inverse scales fold into downstream
    per-feature vectors / activation `scale=` operands.
  * Softmax exp is a Schraudolph bit-trick fused into the mandatory
    PSUM-evacuation op: the q/k weight scales carry alpha*beta =
    128*log2(e)*D^-0.5 so the score PSUM is already K2*s; one DVE
    scalar_tensor_tensor computes int16(round(st + 8192 + rpb_i8)) whose
    bf16 bit-pattern IS exp(s+r) up to a global power-of-2 scale that
    cancels in softmax. rpb ships as int8 = round(K2*rpb) (+8192 via the
    freed scalar slot). ScalarE does no exp at all.
  * The softmax denominator falls out of the AV matmul via a ones-column
    appended to v (M=65 accumulators, row 64 = colsum).
  * LN mean/var still come from ones-matmuls, but the finish chain runs on
    [1,N] rows and broadcasts rstd/-mean*rstd back across partitions with a
    K=1 ones matmul (no DRAM bounce round-trips).
  * fc1/fc2 weights are SBUF-resident (loaded once, streamed during
    attention), gelu/relu descale by 1/32 via the activation scale operand.
"""

import sys

for _p in ("/opt/trn_rl_repo",):
    if _p not in sys.path:
        sys.path.insert(0, _p)

import numpy as np
import ml_dtypes

BF16 = ml_dtypes.bfloat16
FP8 = ml_dtypes.float8_e4m3

B, N, C, H = 8, 1024, 768, 12
D = C // H            # 64
MLP = 4 * C           # 3072
RED = C // 3          # 256
EPS = 1e-5
P = 128
KC = C // P           # 6   c-chunks
KM = MLP // P         # 24  mlp-chunks
KR = RED // P         # 2   adapter chunks
NT = N // P           # 8   token tiles
HALF = 512
NSL = (slice(0, HALF), slice(HALF, N))

K2 = 128.0 / float(np.log(2.0))   # exponent-steps-per-unit-score for bf16
K8 = 8.0 / float(np.log(2.0))     # same for fp8e4m3 (3 mantissa bits)
IBIAS = 64.0                      # int8 exponent offset (global 2^-7 scale)
SCALE = D ** -0.5
AB = K2 * SCALE                   # alpha*beta for the q/k weight pair
ALPHA = float(np.sqrt(AB))        # q-side host scale
BETA = float(np.sqrt(AB))         # k-side host scale
WS = 32.0                         # generic fp8 weight scale
OSC = 256.0                       # oT = OSC * o_true (8.0 folded in evac)

_PROG_CACHE: dict = {}

# indices into the packed [n, 128, KC] per-feature vector table
V_G1, V_G2, V_QB, V_PB, V_FB, V_L1G, V_L1B, V_L2G, V_L2B = range(9)
NVEC = 9


def _build(flags):
    """Build the single-core Bass program. flags is a tuple of bools:
    (has_mask, qb_nz, vb_nz, pb_nz, f1b_nz, fb_nz, adb_nz,
     ln1_triv, ln2_triv)
    """
    (has_mask, qb_nz, vb_nz, pb_nz, f1b_nz, fb_nz, adb_nz,
     ln1_triv, ln2_triv) = flags

    import concourse.tile as tile
    from concourse import bacc, mybir
    from contextlib import ExitStack

    f32 = mybir.dt.float32
    bf16 = mybir.dt.bfloat16
    fp8 = mybir.dt.float8e4
    i16 = mybir.dt.int16
    i8 = mybir.dt.int8
    AF = mybir.ActivationFunctionType
    OP = mybir.AluOpType
    PM = mybir.MatmulPerfMode

    nc = bacc.Bacc("TRN2")

    # ---- external I/O ----
    x_d = nc.declare_dram_parameter("xT", [P, KC, N], f32, isOutput=False)
    rpb8_d = nc.declare_dram_parameter("rpb8", [H, N, N], i8, isOutput=False)
    rpbf_d = nc.declare_dram_parameter("rpbf", [H, N, N], fp8, isOutput=False)
    id_d = nc.declare_dram_parameter("ident", [P, P], fp8, isOutput=False)
    wqk_d = nc.declare_dram_parameter("wqk", [12, P, KC, P], fp8, isOutput=False)
    wv_d = nc.declare_dram_parameter("wv", [P, KC, C], fp8, isOutput=False)
    pw_d = nc.declare_dram_parameter("projw", [KC, P, KC, P], fp8, isOutput=False)
    f1_d = nc.declare_dram_parameter("fc1w", [P, KM, KC, P], fp8, isOutput=False)
    f2_d = nc.declare_dram_parameter("fc2w", [P, KC, KM, P], fp8, isOutput=False)
    ad_d = nc.declare_dram_parameter("adw", [KR, P, KC, P], fp8, isOutput=False)
    au_d = nc.declare_dram_parameter("auw", [P, KC, KR, P], fp8, isOutput=False)
    vec_d = nc.declare_dram_parameter("vecs", [NVEC, P, KC], f32, isOutput=False)
    f1b_d = nc.declare_dram_parameter("fc1b", [P, KM], f32, isOutput=False)
    adb_d = nc.declare_dram_parameter("adb", [P, KR], f32, isOutput=False)
    vb_d = nc.declare_dram_parameter("vbias", [1, C], f32, isOutput=False)
    mb_d = nc.declare_dram_parameter("maskb", [P, NT], f32, isOutput=False)
    out_d = nc.declare_dram_parameter("outT", [P, KC, N], f32, isOutput=True)

    with tile.TileContext(nc) as tc, ExitStack() as ctx:
        sb = ctx.enter_context(tc.tile_pool(name="sb", bufs=1))
        pp = ctx.enter_context(tc.tile_pool(name="pp", bufs=1, space="PSUM"))
        dram = ctx.enter_context(tc.tile_pool(name="dram", bufs=2, space="DRAM"))

        # ---- persistent tiles ----
        VP = 80  # vaug row pitch: D+1 padded so the DoubleRow mt-pair
        #          stride (H*VP bytes) stays 16B-aligned
        xres = sb.tile([P, KC, N], f32, tag="xres", bufs=1)
        qkT = sb.tile([P, 12, N], bf16, tag="qkT", bufs=1)
        vaug = sb.tile([P, NT, H, VP], fp8, tag="vaug", bufs=1)
        oT = sb.tile([P, KC, N], fp8, tag="oT", bufs=1)
        xnT = sb.tile([P, KC, N], fp8, tag="xnT", bufs=1)
        xn2T = sb.tile([P, KC, N], fp8, tag="xn2T", bufs=1)
        a1T = sb.tile([P, KR, N], fp8, tag="a1", bufs=1)

        for ch in range(KC):  # per-chunk loads so LN1 stats start early
            nc.sync.dma_start(out=xres[:, ch], in_=x_d[:, ch])

        ones_bf = sb.tile([P, 1], bf16, tag="ones", bufs=1)
        nc.vector.memset(ones_bf, 1.0)
        ones_row = sb.tile([1, P], bf16, tag="onesr", bufs=1)
        nc.vector.memset(ones_row, 1.0)
        # one contiguous memset initializes every byte (ones column + pad);
        # v evacuations overwrite the [0:D] slices later
        nc.vector.memset(vaug.rearrange("p t h c -> p (t h c)"), 1.0)

        zero_col = sb.tile([P, 1], f32, tag="zcol", bufs=1)
        nc.vector.memset(zero_col, 0.0)
        eps_col = sb.tile([P, 1], f32, tag="ecol", bufs=1)
        nc.vector.memset(eps_col, float(EPS))

        vecs = sb.tile([P, NVEC, KC], f32, tag="vecs", bufs=1)
        nc.sync.dma_start(out=vecs, in_=vec_d[:].rearrange("v p k -> p v k"))

        def vec(i):
            return vecs[:, i]  # [128, KC]

        if f1b_nz:
            f1b = sb.tile([P, KM], f32, tag="f1b", bufs=1)
            nc.sync.dma_start(out=f1b, in_=f1b_d[:])
        if adb_nz:
            adb = sb.tile([P, KR], f32, tag="adb", bufs=1)
            nc.sync.dma_start(out=adb, in_=adb_d[:])
        if vb_nz:
            vb1 = sb.tile([1, C], f32, tag="vb1", bufs=1)
            nc.sync.dma_start(out=vb1, in_=vb_d[:])
            vb_b = sb.tile([P, C], f32, tag="vb_b", bufs=1)
            scratch = dram.tile([1, C], f32, tag="bscratch", bufs=2)
            nc.sync.dma_start(out=scratch, in_=vb1)
            nc.sync.dma_start(out=vb_b, in_=scratch.to_broadcast(vb_b.shape))
        if has_mask:
            maskb = sb.tile([P, NT], f32, tag="maskb", bufs=1)
            nc.sync.dma_start(out=maskb, in_=mb_d[:])

        # ---------------- layernorm (feature-transposed) ----------------
        # stats via ones-matmuls into base-0 psum rows (sum, sum-sq), one
        # [1,512] tile per (stat, nk) so they fit the shared "st" slots --
        # engine ops need all operands on the same partitions.
        def ln_stats_alloc(pfx):
            return [pp.tile([1, HALF], f32, tag="st", bufs=4,
                            name=f"{pfx}{i}") for i in range(4)]

        def ln_stats_ch(stq, ch, xbs, src):
            xb = sb.tile([P, N], bf16, tag="xb", bufs=KC)
            xbs.append(xb)
            nc.vector.tensor_copy(out=xb, in_=src[:, ch])
            x2 = sb.tile([P, N], bf16, tag="x2", bufs=2)
            nc.vector.tensor_mul(x2, xb, xb)
            for nk in range(2):
                nc.tensor.matmul(stq[nk][0:1, :], lhsT=ones_bf,
                                 rhs=xb[:, NSL[nk]],
                                 start=(ch == 0), stop=(ch == KC - 1))
                nc.tensor.matmul(stq[2 + nk][0:1, :], lhsT=ones_bf,
                                 rhs=x2[:, NSL[nk]],
                                 start=(ch == 0), stop=(ch == KC - 1))

        def ln_finish(stq, xbs, dst, g_i, b_i, triv):
            # evacuate sum rows via ACT (with the 1/C fold), scatter the
            # per-token stats across 128 lanes via DMA so sqrt/reciprocal
            # run wide, gather back, K=1 PE broadcast.
            mq = sb.tile([1, 2, N], f32, tag="lnrow", bufs=1)
            for nk in range(2):
                nc.scalar.activation(out=mq[0:1, 0, NSL[nk]], in_=stq[nk],
                                     func=AF.Copy, scale=1.0 / C)
                nc.scalar.activation(out=mq[0:1, 1, NSL[nk]], in_=stq[2 + nk],
                                     func=AF.Copy, scale=1.0 / C)
            s128 = sb.tile([P, 2, NT], f32, tag="s128", bufs=2)
            nc.sync.dma_start(out=s128[:, 0], in_=mq[0:1, 0])
            nc.sync.dma_start(out=s128[:, 1], in_=mq[0:1, 1])
            t128 = sb.tile([P, NT], f32, tag="t128", bufs=4)
            nc.vector.tensor_mul(t128, s128[:, 0], s128[:, 0])
            v128 = sb.tile([P, NT], f32, tag="t128", bufs=4)
            nc.vector.tensor_sub(v128, s128[:, 1], t128)
            sd = sb.tile([P, NT], f32, tag="t128", bufs=4)
            nc.scalar.activation(out=sd, in_=v128, func=AF.Sqrt,
                                 bias=eps_col)
            r2 = sb.tile([P, 2, NT], bf16, tag="r128", bufs=2)
            with nc.allow_low_precision(reason="bf16 rstd broadcast"):
                nc.vector.reciprocal(r2[:, 0], sd)             # rstd
            nc.vector.scalar_tensor_tensor(out=r2[:, 1], in0=s128[:, 0],
                                           scalar=-1.0, in1=r2[:, 0],
                                           op0=OP.mult, op1=OP.mult)
            # gather to one SBUF row, broadcast across partitions with K=1
            # matmuls (PSUM is free during the LN chains), evacuate via ACT
            rn = sb.tile([1, 2, N], bf16, tag="lnrn", bufs=1)
            nc.sync.dma_start(out=rn[0:1, 0], in_=r2[:, 0])
            nc.sync.dma_start(out=rn[0:1, 1], in_=r2[:, 1])
            # dummy ones-matmuls keep the PE HAM clock warm while the
            # scatter/sqrt/gather chain runs (results are never read)
            for wi in range(8):
                warm = pp.tile([1, HALF], f32, tag="st", bufs=4, name="warm")
                nc.tensor.matmul(warm, lhsT=ones_bf,
                                 rhs=xbs[wi % KC][:, NSL[wi % 2]],
                                 start=True, stop=True)
            ab = sb.tile([P, 2, N], bf16, tag="lnab", bufs=1)
            for i in range(2):
                for nk in range(2):
                    rb = pp.tile([P, HALF], f32, tag="st", bufs=4,
                                 name="lnrb")
                    nc.tensor.matmul(rb, lhsT=ones_row,
                                     rhs=rn[0:1, i, NSL[nk]],
                                     start=True, stop=True)
                    nc.scalar.activation(out=ab[:, i, NSL[nk]], in_=rb,
                                         func=AF.Copy)
            # apply in nk-halves, nk0 for every chunk first, so consumers
            # of the first half can start ~5us earlier
            for nk in range(2):
                for ch in range(KC):
                    t1 = sb.tile([P, HALF], bf16, tag="x2h", bufs=2)
                    nc.vector.tensor_mul(t1, xbs[ch][:, NSL[nk]],
                                         ab[:, 0, NSL[nk]])
                    if triv:
                        nc.vector.tensor_add(dst[:, ch, NSL[nk]], t1,
                                             ab[:, 1, NSL[nk]])
                    else:
                        nc.vector.tensor_add(t1, t1, ab[:, 1, NSL[nk]])
                        nc.vector.tensor_scalar(
                            out=dst[:, ch, NSL[nk]], in0=t1,
                            scalar1=vec(g_i)[:, ch:ch + 1],
                            scalar2=vec(b_i)[:, ch:ch + 1],
                            op0=OP.mult, op1=OP.add)

        # ---------------- LN1 -> xnT (fp8) ----------------
        stq1 = ln_stats_alloc("ln1")
        xbs1 = []
        for ch in range(KC):
            ln_stats_ch(stq1, ch, xbs1, xres)
        ln_finish(stq1, xbs1, xnT, V_L1G, V_L1B, ln1_triv)

        # ---------------- QKV (fp8 DoubleRow) ----------------
        def dr_mms(acc, wt, rhs_t, nsl, kd=KC):
            for j in range(kd // 2):
                nc.tensor.matmul(acc, lhsT=wt[:, 2 * j:2 * j + 2, :],
                                 rhs=rhs_t[:, 2 * j:2 * j + 2, nsl],
                                 start=(j == 0), stop=(j == kd // 2 - 1),
                                 perf_mode=PM.DoubleRow)

        # qkv work is emitted as "fill" groups, mostly interleaved into the
        # attention stream so the PE always has runnable instructions while
        # the DVE/ACT produce softmax terms (keeps the HAM clock warm).
        # Accumulators come from the shared 1-bank "st" slots.
        wv_sb = sb.tile([P, KC, C], fp8, tag="h1", bufs=1)
        nc.sync.dma_start(out=wv_sb, in_=wv_d[:])
        wqk_sb = sb.tile([P, 12, KC, P], fp8, tag="wqk", bufs=1)
        nc.sync.dma_start(out=wqk_sb, in_=wqk_d[:])

        def qk_group(blk, nk):
            mm = pp.tile([P, HALF], f32, tag="st", bufs=4, name="qkacc")
            dr_mms(mm, wqk_sb[:, blk], xnT, NSL[nk])
            dst = qkT[:, blk, NSL[nk]]
            if blk < 6 and qb_nz:
                nc.vector.tensor_scalar_add(dst, mm, vec(V_QB)[:, blk:blk + 1])
            else:
                nc.scalar.copy(out=dst, in_=mm)

        def v_group(t, off):
            cw = HALF if off == 0 else C - HALF
            mm = pp.tile([P, HALF], f32, tag="st", bufs=4, name="vacc")
            for j in range(KC // 2):
                nc.tensor.matmul(
                    mm[:, :cw], lhsT=xnT[:, 2 * j:2 * j + 2, t * P:(t + 1) * P],
                    rhs=wv_sb[:, 2 * j:2 * j + 2, off:off + cw],
                    start=(j == 0), stop=(j == KC // 2 - 1),
                    perf_mode=PM.DoubleRow)
            dst = vaug[:, t, off // D:(off + cw) // D, :D]
            src = mm[:, :cw].rearrange("p (h d) -> p h d", d=D)
            if vb_nz:
                nc.vector.tensor_add(
                    dst, src,
                    vb_b[:, off:off + cw].rearrange("p (h d) -> p h d", d=D))
            else:
                nc.scalar.copy(out=dst, in_=src)  # ACT has the spare port

        # pre-attention: q/k blocks 0+6 and all v tiles (the strided vaug
        # writes must stay in simple program order before the AV reads --
        # Tile's range tracker misses the cross-engine dep otherwise)
        for blk in (0, 6):
            for nk in range(2):
                qk_group(blk, nk)
        for t in range(NT):
            v_group(t, 0)
            v_group(t, 1)

        # fill queue of remaining q/k blocks for the attention loop: one
        # group per mt keeps the PE fed while softmax terms are produced
        fill: list = []
        for bq in (1, 2, 3, 4, 5):
            for blk in (bq, 6 + bq):
                for nk in range(2):
                    fill.append(lambda b=blk, n=nk: qk_group(b, n))

        # resident MLP weights: queue the DMAs here so they land during
        # attention (after the first head pairs' rpb tiles)
        f1sb = sb.tile([P, KM, KC, P], fp8, tag="f1sb", bufs=1)
        f2sb = sb.tile([P, KC, KM, P], fp8, tag="f2sb", bufs=1)
        ausb = sb.tile([P, KC, KR, P], fp8, tag="ausb", bufs=1)

        # ---------------- attention ----------------
        ident_sb = sb.tile([P, P], fp8, tag="ident", bufs=1)
        nc.sync.dma_start(out=ident_sb, in_=id_d[:])

        def evac_head(o_ps, hp, hh):
            # ACT evacuates the accumulators (frees PSUM fast); colsum row
            # scatters across 128 lanes for the reciprocal, gathers back,
            # K=1 PE broadcast, one bf16 2x DVE multiply.
            ou = sb.tile([D + 1, N], bf16, tag="ou", bufs=2)
            for nk in range(2):
                nc.scalar.activation(out=ou[:, NSL[nk]], in_=o_ps[nk],
                                     func=AF.Copy)
            cs = sb.tile([P, NT], bf16, tag="cs", bufs=2)
            nc.sync.dma_start(out=cs, in_=ou[D:D + 1, :])
            rcp = sb.tile([P, NT], bf16, tag="cs", bufs=2)
            with nc.allow_low_precision(reason="bf16 softmax denom"):
                # 8/colsum (the OSC/WS fold rides the reciprocal input: the
                # scatter wrote colsum; scale via tensor_scalar on the way)
                nc.vector.tensor_scalar_mul(rcp, cs, float(WS / OSC))
                nc.vector.reciprocal(rcp, rcp)
            scr = dram.tile([1, N], bf16, tag="evscr", bufs=4)
            nc.sync.dma_start(out=scr, in_=rcp)
            rbs = sb.tile([P, N], bf16, tag="rbs", bufs=2)
            nc.sync.dma_start(out=rbs, in_=scr.to_broadcast(rbs.shape))
            if hh == 0:
                nc.vector.tensor_mul(oT[0:D, hp, :], ou[0:D, :], rbs[0:D, :])
            else:
                ot_tmp = sb.tile([D, N], fp8, tag="ott", bufs=2)
                nc.vector.tensor_mul(ot_tmp, ou[0:D, :], rbs[0:D, :])
                nc.sync.dma_start(out=oT[D:P, hp, :], in_=ot_tmp)

        for hp in range(H // 2):
            qh = [qkT[hh * D:(hh + 1) * D, hp, :] for hh in range(2)]
            kh = [qkT[hh * D:(hh + 1) * D, 6 + hp, :] for hh in range(2)]
            o_ps = [[pp.tile([D + 1, HALF], f32, tag="acc", bufs=4,
                             name=f"o_ps{hh}{nk}") for nk in range(2)]
                    for hh in range(2)]
            if hp == 2:  # let early rpb tiles through the queue first
                nc.sync.dma_start(out=f1sb, in_=f1_d[:])
            if hp == 3:
                nc.sync.dma_start(out=f2sb, in_=f2_d[:])
                nc.sync.dma_start(out=ausb, in_=au_d[:])
            if hp == 4:  # prefetch proj weights so proj starts instantly
                pwts = []
                for mt in range(KC):
                    wt = sb.tile([P, KC, P], fp8, tag="wp", bufs=KC)
                    nc.sync.dma_start(out=wt, in_=pw_d[mt])
                    pwts.append(wt)
            def emit_av(ptp, pi):
                # fp8 DoubleRow: one MM contracts an mt-pair (256 tokens)
                for hh in range(2):
                    for nk in range(2):
                        nc.tensor.matmul(
                            o_ps[hh][nk][:, :],
                            lhsT=vaug[:, 2 * pi:2 * pi + 2, 2 * hp + hh, :D + 1],
                            rhs=ptp[hh][:, :, NSL[nk]].bitcast(fp8),
                            start=(pi == 0), stop=(pi == NT // 2 - 1),
                            perf_mode=PM.DoubleRow)

            pend = None  # software-pipelined: AV(pair-1) issues while the
            ptp = None
            for mt in range(NT):  # DVE/ACT produce pts(mt)
                # hh=1 tiles mostly take the "ACT path": rpb is accumulated
                # into the score PSUM by an fp8 identity matmul, and the
                # scale/+64/int8 conversion runs on the scalar engine; the
                # rest use the DVE scalar_tensor_tensor with int8 rpb. This
                # splits the unavoidable fp32-PSUM evacuation across both
                # engines. Scores live in per-(hh,nk) 1-bank quarters so
                # the pipeline is 4 deep.
                act_path = [False, mt % 4 != 3]
                stq, rps = [[None, None], [None, None]], []
                for hh in range(2):
                    h = 2 * hp + hh
                    msl = slice(mt * P, (mt + 1) * P)
                    if act_path[hh]:
                        rp = sb.tile([P, N], fp8, tag="rpf", bufs=4)
                        nc.sync.dma_start(out=rp, in_=rpbf_d[h, msl, :])
                    else:
                        rp = sb.tile([P, N], i8, tag="rpb", bufs=5)
                        nc.sync.dma_start(out=rp, in_=rpb8_d[h, msl, :])
                    rps.append(rp)
                if mt % 2 == 0:
                    if pend is not None:
                        emit_av(*pend)
                    ptp = [sb.tile([P, 2, N], i8, tag="pt", bufs=4,
                                   name=f"ptp{_hh}")
                           for _hh in range(2)]
                for nk in range(2):  # hh-pairs issue adjacently -> row-group
                    for hh in range(2):  # concurrency on the PE
                        stq[hh][nk] = pp.tile([P, HALF], f32, tag="st",
                                              bufs=4, name="stq")
                        nc.tensor.matmul(stq[hh][nk],
                                         lhsT=kh[hh][:, mt * P:(mt + 1) * P],
                                         rhs=qh[hh][:, NSL[nk]],
                                         start=True, stop=not act_path[hh])
                    for hh in range(2):
                        if act_path[hh]:
                            nc.tensor.matmul(stq[hh][nk], lhsT=ident_sb,
                                             rhs=rps[hh][:, NSL[nk]],
                                             start=False, stop=True)
                    for hh in range(2):
                        dst8 = ptp[hh][:, mt % 2, NSL[nk]]
                        # int8(st/16 + 64 + K8*rpb): bit-pattern is the
                        # fp8e4m3 encoding of exp(s+r) up to a global
                        # power-of-2 scale that cancels in softmax
                        if act_path[hh]:
                            nc.scalar.activation(out=dst8, in_=stq[hh][nk],
                                                 func=AF.Copy, bias=IBIAS,
                                                 scale=1.0 / 16.0)
                        else:
                            nc.vector.scalar_tensor_tensor(
                                out=dst8, in0=stq[hh][nk], scalar=1.0 / 16.0,
                                in1=rps[hh][:, NSL[nk]],
                                op0=OP.mult, op1=OP.add)
                        if has_mask:
                            nc.vector.tensor_scalar_mul(dst8, dst8,
                                                        maskb[:, mt:mt + 1])
                if fill:  # keep the PE fed while softmax terms are produced
                    fill.pop(0)()
                if mt % 2 == 1:
                    pend = (ptp, mt // 2)
            emit_av(*pend)
            evac_head(o_ps[0], hp, 0)
            evac_head(o_ps[1], hp, 1)

        # ---------------- proj + residual 1 + LN2 stats ----------------
        stq2 = ln_stats_alloc("ln2")
        xbs2 = []
        for mt in range(KC):
            for nk in range(2):
                mm = pp.tile([P, HALF], f32, tag="acc", bufs=4)
                dr_mms(mm, pwts[mt], oT, NSL[nk])
                if pb_nz:
                    nc.vector.tensor_scalar_add(mm, mm, vec(V_PB)[:, mt:mt + 1])
                nc.vector.scalar_tensor_tensor(
                    out=xres[:, mt, NSL[nk]], in0=mm,
                    scalar=vec(V_G1)[:, mt:mt + 1],
                    in1=xres[:, mt, NSL[nk]], op0=OP.mult, op1=OP.add)
            if mt >= 1:  # stats lag the proj chunks so the PE never waits
                ln_stats_ch(stq2, mt - 1, xbs2, xres)
        ln_stats_ch(stq2, KC - 1, xbs2, xres)
        ln_finish(stq2, xbs2, xn2T, V_L2G, V_L2B, ln2_triv)

        # ---------------- adapter down (relu via ACT, 1/32 descale) -----
        for mt in range(KR):
            wt = sb.tile([P, KC, P], fp8, tag="w6", bufs=2)
            nc.sync.dma_start(out=wt, in_=ad_d[mt])
            for nk in range(2):
                mm = pp.tile([P, HALF], f32, tag="acc", bufs=4)
                dr_mms(mm, wt, xn2T, NSL[nk])
                nc.scalar.activation(
                    out=a1T[:, mt, NSL[nk]], in_=mm, func=AF.Relu,
                    scale=1.0 / WS,
                    bias=(adb[:, mt:mt + 1] if adb_nz else zero_col))

        # ---------------- MLP (fp8 DR, resident weights) ----------------
        for nk in range(2):
            h1 = sb.tile([P, KM, HALF], fp8, tag="h1", bufs=1)
            for mt in range(KM):
                mm = pp.tile([P, HALF], f32, tag="acc", bufs=4)
                dr_mms(mm, f1sb[:, mt], xn2T, NSL[nk])
                nc.scalar.activation(
                    out=h1[:, mt], in_=mm, func=AF.Gelu, scale=1.0 / WS,
                    bias=(f1b[:, mt:mt + 1] if f1b_nz else zero_col))
            for mt in range(KC):
                mm = pp.tile([P, HALF], f32, tag="acc", bufs=4)
                for j in range(KM // 2):
                    nc.tensor.matmul(mm, lhsT=f2sb[:, mt, 2 * j:2 * j + 2, :],
                                     rhs=h1[:, 2 * j:2 * j + 2, :],
                                     start=(j == 0), stop=False,
                                     perf_mode=PM.DoubleRow)
                nc.tensor.matmul(mm, lhsT=ausb[:, mt],
                                 rhs=a1T[:, 0:KR, NSL[nk]],
                                 start=False, stop=True,
                                 perf_mode=PM.DoubleRow)
                if fb_nz:
                    nc.vector.tensor_scalar_add(mm, mm, vec(V_FB)[:, mt:mt + 1])
                nc.vector.scalar_tensor_tensor(
                    out=xres[:, mt, NSL[nk]], in0=mm,
                    scalar=vec(V_G2)[:, mt:mt + 1],
                    in1=xres[:, mt, NSL[nk]], op0=OP.mult, op1=OP.add)
                nc.sync.dma_start(out=out_d[:, mt, NSL[nk]],
                                  in_=xres[:, mt, NSL[nk]])

    if not nc.is_finalized():
        nc.finalize()
    return nc


def _pack_w6(wT, km, kk):
    """[K, M] (K=contraction, M=out) -> [M//128, 128, K//128, 128] fp8 tiles
    laid out so each DMA partition read is contiguous."""
    K, M = wT.shape
    assert K == kk * P and M == km * P
    a = wT.reshape(kk, P, km, P)          # [ks, p, mt, col]
    return np.ascontiguousarray(a.transpose(2, 1, 0, 3)).astype(FP8)


def _pack_res(wT, km, kk):
    """[K, M] -> [128, M//128, K//128, 128] partition-major for one-shot
    SBUF-resident DMA (contiguous per-partition reads)."""
    K, M = wT.shape
    assert K == kk * P and M == km * P
    a = wT.reshape(kk, P, km, P)          # [ks, p, mt, col]
    return np.ascontiguousarray(a.transpose(1, 2, 0, 3)).astype(FP8)


def _stripe(v, k):
    """[k*128] -> [128, k] with v[ks*128+p] at [p, ks]."""
    return np.ascontiguousarray(v.reshape(k, P).T).astype(np.float32)


def prepare_core_inputs(x, mask, rpb, ln1_g, ln1_b, qkv_w, q_bias, v_bias,
                        proj_w, proj_b, gamma1, ln2_g, ln2_b, fc1_w, fc1_b,
                        fc2_w, fc2_b, ad_dw, ad_db, ad_uw, ad_ub, gamma2):
    """Host-side layout prep. Returns (per_core_maps, flags)."""
    f32 = np.float32

    qkv_w = np.asarray(qkv_w, f32)
    wq = qkv_w[:C] * ALPHA            # q-side carries alpha (incl. D^-.5*K2)
    wk = qkv_w[C:2 * C] * BETA
    wv = qkv_w[2 * C:] * WS
    wqkT = np.concatenate([wq, wk], 0).T          # [C, 1536]
    wqk = _pack_w6(wqkT, 12, KC)
    wv_packed = np.ascontiguousarray(
        wv.T.reshape(KC, P, C).transpose(1, 0, 2)).astype(FP8)

    projw = _pack_w6(np.asarray(proj_w, f32).T * WS, KC, KC)
    fc1w = _pack_res(np.asarray(fc1_w, f32).T * WS, KM, KC)
    fc2w = _pack_res(np.asarray(fc2_w, f32).T * WS, KC, KM)
    adw = _pack_w6(np.asarray(ad_dw, f32).T * WS, KR, KC)
    auw = _pack_res(np.asarray(ad_uw, f32).T * WS, KC, KR)

    # rpb as exponent steps: int8 round(K8*rpb)+64 for the DVE schraudolph
    # path (the +64 fp8-exponent bias rides along), fp8 K2*rpb for the
    # identity-matmul (ACT) path where the bias comes from the ACT op
    rpbT = np.ascontiguousarray(np.asarray(rpb, f32).transpose(0, 2, 1))
    rpb8 = np.clip(np.round(K8 * rpbT) + 64.0, -127, 127).astype(np.int8)
    rpbf = (K2 * rpbT).astype(FP8)
    ident = np.eye(P).astype(FP8)

    q_bias_s = np.asarray(q_bias, f32) * ALPHA
    fb = (np.asarray(fc2_b, f32) + np.asarray(ad_ub, f32)) * WS

    vecs = np.stack([
        _stripe(np.asarray(gamma1, f32) / (OSC * WS), KC),
        _stripe(np.asarray(gamma2, f32) / WS, KC),
        _stripe(q_bias_s, KC),
        _stripe(np.asarray(proj_b, f32) * OSC * WS, KC),
        _stripe(fb, KC),
        _stripe(np.asarray(ln1_g, f32), KC),
        _stripe(np.asarray(ln1_b, f32), KC),
        _stripe(np.asarray(ln2_g, f32), KC),
        _stripe(np.asarray(ln2_b, f32), KC),
    ], 0)  # [NVEC, 128, KC]

    f1b = _stripe(np.asarray(fc1_b, f32), KM)
    adb = _stripe(np.asarray(ad_db, f32), KR)
    vb = (np.asarray(v_bias, f32) * WS).reshape(1, C).astype(f32)

    mask = np.asarray(mask)
    has_mask = not bool(mask.all())

    flags = (
        has_mask,
        bool(np.any(q_bias_s)),
        bool(np.any(v_bias)),
        bool(np.any(proj_b)),
        bool(np.any(fc1_b)),
        bool(np.any(fb)),
        bool(np.any(ad_db)),
        bool(np.all(ln1_g == 1.0) and not np.any(ln1_b)),
        bool(np.all(ln2_g == 1.0) and not np.any(ln2_b)),
    )

    shared = {
        "rpb8": rpb8, "rpbf": rpbf, "ident": ident,
        "wqk": wqk, "wv": wv_packed, "projw": projw,
        "fc1w": fc1w, "fc2w": fc2w, "adw": adw, "auw": auw,
        "vecs": vecs, "fc1b": f1b, "adb": adb, "vbias": vb,
    }

    x = np.asarray(x, f32)
    per_core = []
    for b in range(B):
        xT = np.ascontiguousarray(
            x[b].T.reshape(KC, P, N).transpose(1, 0, 2)).astype(f32)
        if has_mask:
            mb = np.where(mask[b], 1.0, 0.0).astype(f32)    # [N] over keys m
            mb = np.ascontiguousarray(mb.reshape(NT, P).T)  # [128, NT]
        else:
            mb = np.zeros((P, NT), f32)
        m = dict(shared)
        m["xT"] = xT
        m["maskb"] = mb
        per_core.append(m)
    return per_core, flags


def _ensure_ntff_hook():
    """The agent image lacks ``antenv.axon_hooks``; provide it and register
    the ctypes NTFF profile hook so trace=True works under axon."""
    import types
    try:
        from antenv.axon_hooks import get_axon_ntff_profile_hook  # noqa: F401
        return
    except ImportError:
        pass
    import antenv
    mod = types.ModuleType("antenv.axon_hooks")
    _h = {"hook": None}
    mod.set_axon_ntff_profile_hook = lambda h: _h.__setitem__("hook", h)
    mod.get_axon_ntff_profile_hook = lambda: _h["hook"]
    sys.modules["antenv.axon_hooks"] = mod
    antenv.axon_hooks = mod
    try:
        from trn_agent_boot.trn_boot import _ntff_profile_via_ctypes
        hook = _ntff_profile_via_ctypes("/opt/axon/libaxon_pjrt.so")
        if hook is not None:
            mod.set_axon_ntff_profile_hook(hook)
    except Exception as e:  # profiling degrades, run still works
        print("ntff hook setup failed:", e)


def run_sharded(inputs, trace=False, trace_kwargs=None):
    """Compile (cached) + run on 8 cores. Returns (out [B,N,C] f32, results)."""
    from concourse.bass_utils import run_bass_kernel_spmd
    if trace:
        _ensure_ntff_hook()

    per_core, flags = prepare_core_inputs(**inputs)
    if flags not in _PROG_CACHE:
        _PROG_CACHE[flags] = _build(flags)
    nc = _PROG_CACHE[flags]

    kw = {}
    if trace:
        kw["trace"] = True
        kw["trace_cores"] = [0]
        if trace_kwargs:
            kw["trace_kwargs"] = trace_kwargs
    res = run_bass_kernel_spmd(nc, per_core, core_ids=list(range(B)), **kw)

    out = np.empty((B, N, C), np.float32)
    for b in range(B):
        oT = res.results[b]["outT"]          # [128, KC, N]
        out[b] = oT.transpose(1, 0, 2).reshape(C, N).T
    return out, res


def kernel(**inputs):
    out, _ = run_sharded(inputs, trace=False)
    return out
